# revision 1
# baseline (speedup 1.0000x reference)
"""GAT (2-layer, heads=1) + pooling + MLP on 8 Trainium2 NeuronCores.

Strategy:
- Nodes are mapped to per-graph padded slots (SLOT = align128(max graph size)),
  graphs are sharded 16-per-core, and within each core slots are striped so
  that dst-block i holds slots s with s % NB == i.  Every 128-slot block then
  contains exactly 8 slots of each of the core's 16 graphs (partition p ->
  local graph p//8), which makes pooling segment boundaries compile-time.
- Edge aggregation: per-edge rows [h|s] are fetched with dma_gather (int16
  indices -> 4 src buckets); attention weights w = exp(leakyrelu(s_src +
  d_dst)) are computed on-chip (d expanded per edge via a one-hot *
  broadcast-d reduce); the segment softmax + feature sum is one matmul per
  128-edge tile accumulating [sum(w*h) | sum(w)] into a per-block PSUM.
- Self-loop edges are applied densely at finalize (no gather).
- Node tables ([h|s] rows, bf16) are built sharded and AllGathered; pooled
  [G,2H] is assembled with a single small AllGather; the final MLP is
  replicated.
"""
import sys

sys.path.insert(0, "/opt/trn_rl_repo")

import numpy as np
import ml_dtypes

import concourse.bacc as bacc
import concourse.bass as bass
import concourse.mybir as mybir
import concourse.tile as tile
from concourse import bass_utils
from concourse.masks import make_identity

bf16 = ml_dtypes.bfloat16
F32 = mybir.dt.float32
BF = mybir.dt.bfloat16
I16 = mybir.dt.int16
AL = mybir.AluOpType
ACT = mybir.ActivationFunctionType

NCORES = 8
G = 128
N = 100000
FIN = 64
H = 64
NEG = 0.2
NBUCKET = 4
GPC = G // NCORES  # graphs per core = 16
NEG_BIG = -1.0e30


# ---------------------------------------------------------------- host prep
def _host_prep(inputs):
    x = np.asarray(inputs["x"], np.float32)
    ei = np.asarray(inputs["edge_index"]).astype(np.int64)
    bid = np.asarray(inputs["batch_ids"]).astype(np.int64)

    cnt = np.bincount(bid, minlength=G).astype(np.int64)
    SLOT = int(np.ceil(max(cnt.max(), 128) / 128) * 128)
    NSLOT = GPC * SLOT
    NB = NSLOT // 128
    assert NB % 16 == 0, NB
    NGRP = NB // 16
    NSLOT_G = NCORES * NSLOT
    BUCKET = NSLOT_G // NBUCKET
    assert BUCKET * NBUCKET == NSLOT_G and BUCKET <= 32768

    gstart = np.zeros(G + 1, np.int64)
    gstart[1:] = np.cumsum(cnt)
    rank = np.arange(N, dtype=np.int64) - gstart[bid]
    slot_of = bid * SLOT + rank  # graph-padded slot, 0..NSLOT_G

    def pi(s):
        c, sl = s // NSLOT, s % NSLOT
        return c * NSLOT, (sl % NB) * 128 + sl // NB

    core_base, loc = pi(slot_of)
    pi_of = core_base + loc  # global pi row of each node

    # permuted x per core (pad rows zero)
    x_pi = np.zeros((NCORES, NSLOT, FIN), np.float32)
    x_pi[pi_of // NSLOT, pi_of % NSLOT] = x

    # masks / counts per core
    # local slot (block i, partition p) <-> graph-slot p*NB + i
    pp, ii = np.meshgrid(np.arange(128), np.arange(NB), indexing="ij")
    gslot = pp * NB + ii  # [128, NB] graph-padded local slot
    within = gslot % SLOT  # rank within graph
    lg = gslot // SLOT  # local graph 0..15 (== pp//8)
    mask01 = np.zeros((NCORES, 128, NB), np.float32)
    maskneg = np.zeros((NCORES, 128, NB), np.float32)
    for c in range(NCORES):
        real = within < cnt[c * GPC + lg]
        mask01[c] = real.astype(np.float32)
        maskneg[c] = np.where(real, 0.0, NEG_BIG).astype(np.float32)

    # edges (no self loops in the gather path)
    src, dst = ei[0], ei[1]
    ps = pi_of[src]
    pd = pi_of[dst]
    core = pd // NSLOT
    blk = (pd % NSLOT) // 128
    bkt = ps // BUCKET
    grp = blk // 16

    cnts = np.zeros((NCORES, NB, NBUCKET), np.int64)
    np.add.at(cnts, (core, blk, bkt), 1)
    # uniform tiles-per-block within each (group, bucket)
    tpb = np.zeros((NGRP, NBUCKET), np.int64)  # tiles per block
    for g in range(NGRP):
        for b in range(NBUCKET):
            m = cnts[:, 16 * g:16 * g + 16, b].max()
            tpb[g, b] = max(1, int(np.ceil(m / 128)))
    seg_tiles = (16 * tpb).astype(np.int64)  # tiles per (g,b) segment
    TT = int(seg_tiles.sum())  # total tiles per core per layer
    TOTSLOT = TT * 128

    # slot offsets: order (g, b, block-within-group, slot)
    seg_off = np.zeros((NGRP, NBUCKET), np.int64)
    acc = 0
    for g in range(NGRP):
        for b in range(NBUCKET):
            seg_off[g, b] = acc
            acc += seg_tiles[g, b] * 128

    order = np.lexsort((bkt, blk))  # edges sorted by (blk, bkt); core split below
    src_local = np.zeros((NCORES, 128, TOTSLOT // 128), np.int16)
    dst_loc = np.full((NCORES, 128, TOTSLOT // 128), -1.0, np.float32)
    for c in range(NCORES):
        sel = order[core[order] == c]
        sblk, sbkt = blk[sel], bkt[sel]
        sps, spd = ps[sel], pd[sel]
        # slot index for each edge: within its (g,b,block) run
        # run start: seg_off[g,b] + (blk%16)*tpb[g,b]*128; position = rank in run
        key = sblk * NBUCKET + sbkt
        # stable order already (blk, bkt); rank within run:
        runstart_mark = np.r_[True, key[1:] != key[:-1]]
        runid = np.cumsum(runstart_mark) - 1
        nruns = int(runid[-1]) + 1 if len(runid) else 0
        first = np.full(nruns, len(sel), np.int64)
        np.minimum.at(first, runid, np.arange(len(sel)))
        pos = np.arange(len(sel)) - first[runid]
        gg = sblk // 16
        slot = (seg_off[gg, sbkt] + (sblk % 16) * tpb[gg, sbkt] * 128 + pos)
        assert (pos < tpb[gg, sbkt] * 128).all()
        p_ = slot % 128
        t_ = slot // 128
        src_local[c, p_, t_] = (sps - sbkt * BUCKET).astype(np.int16)
        dst_loc[c, p_, t_] = (spd % NSLOT % 128).astype(np.float32)

    # wrapped int16 gather indices per (g,b) call, concatenated along cols
    idx_w = np.zeros((NCORES, 128, TOTSLOT // 16), np.int16)
    for c in range(NCORES):
        flat = np.zeros(TOTSLOT, np.int16)
        sl = src_local[c]
        flat[np.arange(TOTSLOT)] = sl[np.arange(TOTSLOT) % 128,
                                      np.arange(TOTSLOT) // 128]
        w = flat.reshape(TOTSLOT // 16, 16).T  # [16, TOTSLOT//16]
        idx_w[c] = np.tile(w, (8, 1))

    # weights
    W1 = np.asarray(inputs["W1"], np.float32)
    W2 = np.asarray(inputs["W2"], np.float32)
    waug1 = np.concatenate(
        [W1, (W1 @ np.asarray(inputs["a_src1"], np.float32))[:, None],
         (W1 @ np.asarray(inputs["a_dst1"], np.float32))[:, None]], axis=1)
    waug2 = np.concatenate(
        [W2, (W2 @ np.asarray(inputs["a_src2"], np.float32))[:, None],
         (W2 @ np.asarray(inputs["a_dst2"], np.float32))[:, None]], axis=1)

    b1 = np.asarray(inputs["b1"], np.float32)
    b2v = np.asarray(inputs["b2"], np.float32)
    lin1_W = np.asarray(inputs["lin1_W"], np.float32)
    lin1_b = np.asarray(inputs["lin1_b"], np.float32)
    lin2_W = np.asarray(inputs["lin2_W"], np.float32)
    lin2_b = np.asarray(inputs["lin2_b"], np.float32)

    npadneg = np.zeros((NCORES, 64, GPC), np.float32)
    invcnt = np.zeros((NCORES, 64, GPC), np.float32)
    for c in range(NCORES):
        npadneg[c] = -np.broadcast_to(
            (SLOT - cnt[c * GPC:(c + 1) * GPC]).astype(np.float32), (64, GPC))
        invcnt[c] = np.broadcast_to(
            1.0 / np.maximum(cnt[c * GPC:(c + 1) * GPC], 1.0), (64, GPC))

    tpl = dict(SLOT=SLOT, NSLOT=NSLOT, NB=NB, NGRP=NGRP, NSLOT_G=NSLOT_G,
               BUCKET=BUCKET, tpb=tpb, seg_tiles=seg_tiles, seg_off=seg_off,
               TT=TT)

    per_core = []
    for c in range(NCORES):
        per_core.append({
            "xT_own": np.ascontiguousarray(x_pi[c].T),
            "idx_w": idx_w[c],
            "dst_loc": dst_loc[c],
            "mask01": mask01[c],
            "maskpad": 1.0 - mask01[c],
            "maskneg": maskneg[c],
            "npadneg": npadneg[c],
            "invcnt": invcnt[c],
            "waug1": waug1,
            "waug2_bf": waug2.astype(bf16),
            "b1_tile": np.broadcast_to(b1, (128, 64)).copy(),
            "b1_tile2": np.broadcast_to(np.asarray(inputs["b2"], np.float32),
                                        (128, 64)).copy(),
            "lin1_W": lin1_W,
            "lin1b_tile": np.broadcast_to(lin1_b, (128, 64)).copy(),
            "lin2row": np.broadcast_to(lin2_W[:, 0], (128, 64)).copy(),
            "b2col": np.full((128, 1), lin2_b[0], np.float32),
            "rb_cols": np.stack([np.maximum(b1, 0.0),
                                 np.maximum(b2v, 0.0)], axis=1),
        })
    return tpl, per_core


# ---------------------------------------------------------------- device bld
DEBUG = False


def _build(tpl):
    NSLOT = tpl["NSLOT"]
    NB = tpl["NB"]
    NGRP = tpl["NGRP"]
    NSLOT_G = tpl["NSLOT_G"]
    BUCKET = tpl["BUCKET"]
    tpb = tpl["tpb"]
    seg_tiles = tpl["seg_tiles"]
    TT = tpl["TT"]

    nc = bacc.Bacc("TRN2", target_bir_lowering=False, debug=False,
                   num_devices=NCORES)

    # inputs
    xT_own = nc.dram_tensor("xT_own", [FIN, NSLOT], F32,
                            kind="ExternalInput")
    idx_w = nc.dram_tensor("idx_w", [128, TT * 8], I16, kind="ExternalInput")
    dst_loc = nc.dram_tensor("dst_loc", [128, TT], F32, kind="ExternalInput")
    mask01 = nc.dram_tensor("mask01", [128, NB], F32, kind="ExternalInput")
    maskpad = nc.dram_tensor("maskpad", [128, NB], F32, kind="ExternalInput")
    npadneg = nc.dram_tensor("npadneg", [64, GPC], F32, kind="ExternalInput")
    invcnt = nc.dram_tensor("invcnt", [64, GPC], F32, kind="ExternalInput")
    waug1 = nc.dram_tensor("waug1", [64, 66], F32, kind="ExternalInput")
    waug2_bf = nc.dram_tensor("waug2_bf", [64, 66], BF, kind="ExternalInput")
    b1_tile = nc.dram_tensor("b1_tile", [128, 64], F32, kind="ExternalInput")
    b1_tile2 = nc.dram_tensor("b1_tile2", [128, 64], F32, kind="ExternalInput")
    lin1_W = nc.dram_tensor("lin1_W", [128, 64], F32, kind="ExternalInput")
    lin1b_tile = nc.dram_tensor("lin1b_tile", [128, 64], F32,
                                kind="ExternalInput")
    lin2row = nc.dram_tensor("lin2row", [128, 64], F32, kind="ExternalInput")
    b2col = nc.dram_tensor("b2col", [128, 1], F32, kind="ExternalInput")
    rb_cols = nc.dram_tensor("rb_cols", [64, 2], F32, kind="ExternalInput")

    out_final = nc.dram_tensor("out_final", [128, 1], F32,
                               kind="ExternalOutput")
    if DEBUG:
        dbg_tab = nc.dram_tensor("dbg_tab", [NSLOT, 65], BF,
                                 kind="ExternalOutput")
        dbg_d = nc.dram_tensor("dbg_d", [NB, 128], F32, kind="ExternalOutput")
        dbg_hT = nc.dram_tensor("dbg_hT", [64, NSLOT], BF,
                                kind="ExternalOutput")
        dbg_pool = nc.dram_tensor("dbg_pool", [64, 4 * GPC], F32,
                                  kind="ExternalOutput")
        dbg_z = nc.dram_tensor("dbg_z", [G, 128], F32, kind="ExternalOutput")

    with tile.TileContext(nc) as tc:
        with (
            tc.tile_pool(name="const", bufs=1) as cp,
            tc.tile_pool(name="stage", bufs=1) as stp,
            tc.tile_pool(name="dram", bufs=1, space="DRAM") as dr,
        ):
            # ---- constants in SBUF
            iota_i = cp.tile([128, 128], mybir.dt.int32)
            nc.gpsimd.iota(iota_i[:], pattern=[[1, 128]], base=0,
                           channel_multiplier=0)
            iota_bf = cp.tile([128, 128], BF)
            nc.vector.tensor_copy(iota_bf[:], iota_i[:])
            ident = cp.tile([128, 128], F32)
            make_identity(nc, ident[:])
            ones_row = cp.tile([1, 128], F32)
            nc.gpsimd.memset(ones_row[:], 1.0)
            ones_bf = cp.tile([128, 1], BF)
            nc.gpsimd.memset(ones_bf[:], 1.0)
            waug1_sb = cp.tile([64, 66], F32)
            nc.sync.dma_start(waug1_sb[:], waug1[:, :])
            waug2_sb = cp.tile([64, 66], BF)
            nc.sync.dma_start(waug2_sb[:], waug2_bf[:, :])
            b1t = cp.tile([128, 64], F32)
            nc.sync.dma_start(b1t[:], b1_tile[:, :])
            b2t = cp.tile([128, 64], F32)
            nc.sync.dma_start(b2t[:], b1_tile2[:, :])
            m01 = cp.tile([128, NB], F32)
            nc.sync.dma_start(m01[:], mask01[:, :])
            mpad = cp.tile([128, NB], F32)
            nc.sync.dma_start(mpad[:], maskpad[:, :])
            dl_all = cp.tile([128, TT], F32)
            nc.sync.dma_start(dl_all[:], dst_loc[:, :])

            # persistent staging
            h_T = stp.tile([64, NSLOT], BF, tag="h_T", name="h_T")  # layer1 out, pi order
            dstage = [stp.tile([128, NB], F32, tag=f"dstage{l}", name=f"dstage{l}")
                      for l in range(2)]
            pool_mx = [stp.tile([64, GPC], F32, tag=f"pmx{l}", name=f"pmx{l}")
                       for l in range(2)]
            pool_sm = [stp.tile([64, GPC], F32, tag=f"psm{l}", name=f"psm{l}")
                       for l in range(2)]
            for l in range(2):
                nc.vector.memset(pool_mx[l][:], NEG_BIG)
                nc.vector.memset(pool_sm[l][:], 0.0)

            # DRAM scratch
            table = [dr.tile([NSLOT_G, 128], BF, tag=f"tab{l}", name=f"tab{l}")
                     for l in range(2)]
            tab_own = [dr.tile([NSLOT, 128], BF, tag=f"tabown{l}", name=f"tabown{l}")
                       for l in range(2)]
            d2d = [dr.tile([NB, 128], F32, tag=f"d2d{l}", name=f"d2d{l}") for l in range(2)]
            pool_bounce_in = dr.tile([GPC, 128], F32)
            pool_bounce_out = dr.tile([G, 128], F32)

            # ===== table 1 build: replicated full + own shard for d =====
            with (
                tc.tile_pool(name="p1", bufs=3) as p1,
                tc.tile_pool(name="p1x", bufs=3) as p1x,
                tc.tile_pool(name="p1ps", bufs=2, space="PSUM") as p1ps,
            ):
                SLAB = 32
                for base in range(0, NB, SLAB):
                    ns = min(SLAB, NB - base)
                    xT = p1x.tile([64, SLAB * 128], F32, tag="xTs")
                    nc.sync.dma_start(
                        xT[:, 0:ns * 128],
                        xT_own[:, 128 * base:128 * (base + ns)])
                    rows = p1.tile([128, SLAB, 65], BF, tag="rows")
                    for j in range(ns):
                        hps = p1ps.tile([128, 66], F32, space="PSUM",
                                        tag="hps")
                        nc.tensor.matmul(
                            hps[:], lhsT=xT[:, 128 * j:128 * (j + 1)],
                            rhs=waug1_sb[:], start=True, stop=True)
                        nc.scalar.activation(rows[:, j, :], hps[:, 0:65],
                                             ACT.Copy)
                        nc.vector.tensor_copy(
                            dstage[0][:, base + j:base + j + 1],
                            hps[:, 65:66])
                    nc.sync.dma_start(
                        tab_own[0].rearrange("(s r) c -> r s c", r=128)[
                            :, base:base + ns, 0:65],
                        rows[:, 0:ns, :])
                # d transpose -> DRAM [NB, 128]
                dt_ps = p1ps.tile([NB, 128], F32, space="PSUM", tag="dt")
                nc.tensor.transpose(dt_ps[:], dstage[0][:], ident[:])
                dt_sb = p1.tile([NB, 128], F32, tag="dts")
                nc.vector.tensor_copy(dt_sb[:], dt_ps[:])
                nc.sync.dma_start(d2d[0][:, :], dt_sb[:])

            nc.gpsimd.collective_compute(
                "AllGather", AL.bypass,
                replica_groups=[list(range(NCORES))],
                ins=[tab_own[0].opt()], outs=[table[0].opt()])

            # ================= edge pass (both layers) =================
            def edge_pass(l):
                tab = table[l]
                with (
                    tc.tile_pool(name=f"eg{l}", bufs=2) as eg,
                    tc.tile_pool(name=f"eq{l}", bufs=2) as eqp,
                    tc.tile_pool(name=f"ep{l}", bufs=1, space="PSUM") as eps,
                    tc.tile_pool(name=f"et{l}", bufs=2, space="PSUM") as ept,
                    tc.tile_pool(name=f"ef{l}", bufs=3) as ef,
                ):
                    for g in range(NGRP):
                        # broadcast d rows of the group's 16 blocks
                        dbc = ef.tile([128, 16, 128], F32, tag="dbc")
                        nc.sync.dma_start(
                            dbc[:],
                            d2d[l][16 * g:16 * (g + 1), :].rearrange(
                                "(o r) c -> o r c", o=1).to_broadcast(
                                [128, 16, 128]))
                        dbc_bf = ef.tile([128, 16, 128], BF, tag="dbcb")
                        nc.vector.tensor_copy(dbc_bf[:], dbc[:])

                        psums = [eps.tile([128, 260], F32, space="PSUM",
                                          tag=f"ps{k}", name=f"ps{k}")
                                 for k in range(4)]
                        for ps_ in psums:
                            nc.vector.memset(ps_[:], 0.0)

                        for b in range(NBUCKET):
                            ntile = int(seg_tiles[g, b])
                            Tb = int(tpb[g, b])
                            off = int(tpl["seg_off"][g, b]) // 128  # tile off
                            # gather the whole (g,b) segment
                            idx_sb = eg.tile([128, ntile * 8], I16, tag="idx")
                            nc.sync.dma_start(
                                idx_sb[:],
                                idx_w[:, off * 8:(off + ntile) * 8])
                            ch = eg.tile([128, ntile, 128], BF, tag="ch")
                            nc.gpsimd.dma_gather(
                                out_ap=ch[:],
                                in_ap=tab[b * BUCKET:(b + 1) * BUCKET, :],
                                idxs_ap=idx_sb[:],
                                num_idxs=ntile * 128,
                                num_idxs_reg=ntile * 128,
                                elem_size=128,
                                single_packet=False)
                            dl = dl_all[:, off:off + ntile]
                            # per tile: lhsT = (iota==dstloc)*|d|  (the |d|
                            # row-scale cancels in the softmax division);
                            # accum of (iota==dstloc)*d gives d_edge.
                            lhsT = eqp.tile([128, ntile, 128], BF, tag="eq")
                            dedge = ef.tile([128, ntile], F32, tag="dedge")
                            for tt in range(ntile):
                                k16 = tt // Tb
                                nc.vector.scalar_tensor_tensor(
                                    out=lhsT[:, tt, :], in0=iota_bf[:],
                                    scalar=dl[:, tt:tt + 1],
                                    in1=dbc_bf[:, k16, :],
                                    op0=AL.is_equal, op1=AL.mult,
                                    accum_out=dedge[:, tt:tt + 1])
                            # z = s + d ; w = exp(max(0.2 z, z))
                            z = ef.tile([128, ntile], F32, tag="z")
                            nc.vector.tensor_tensor(
                                out=z[:], in0=ch[:, :, 64], in1=dedge[:],
                                op=AL.add)
                            nc.vector.scalar_tensor_tensor(
                                out=z[:], in0=z[:], scalar=NEG, in1=z[:],
                                op0=AL.mult, op1=AL.max)
                            w = ef.tile([128, ntile], F32, tag="w")
                            nc.scalar.activation(w[:], z[:], ACT.Exp)
                            # ones into s slot for the denominator column
                            nc.vector.tensor_copy(
                                ch[:, :, 64],
                                ones_bf[:].to_broadcast([128, ntile]))
                            # rhs2 = ch[:, :, 0:65] * w (bulk)
                            ch2 = eqp.tile([128, ntile, 65], BF, tag="dmul",
                                           name="ch2")
                            nc.vector.tensor_tensor(
                                out=ch2[:], in0=ch[:, :, 0:65],
                                in1=w[:].to_broadcast([128, ntile, 65]),
                                op=AL.mult)
                            # matmuls
                            for i in range(16):
                                ps = psums[i // 4]
                                csl = slice(65 * (i % 4), 65 * (i % 4) + 65)
                                for t in range(Tb):
                                    tt = i * Tb + t
                                    nc.tensor.matmul(
                                        ps[:, csl],
                                        lhsT=lhsT[:, tt, :],
                                        rhs=ch2[:, tt, :],
                                        start=False,
                                        stop=(b == NBUCKET - 1 and t == Tb - 1))

                        # ---- finalize the group's 16 blocks
                        for i in range(16):
                            blk_id = 16 * g + i
                            ps = psums[i // 4]
                            csl = slice(65 * (i % 4), 65 * (i % 4) + 65)
                            # self loops (dense): rows of own dst block
                            row = ef.tile([128, 65], BF, tag="slrow")
                            nc.sync.dma_start(
                                row[:], tab_own[l][128 * blk_id:
                                                   128 * blk_id + 128, 0:65])
                            zs = ef.tile([128, 1], F32, tag="zs")
                            nc.vector.tensor_tensor(
                                out=zs[:], in0=row[:, 64:65],
                                in1=dstage[l][:, blk_id:blk_id + 1],
                                op=AL.add)
                            nc.vector.scalar_tensor_tensor(
                                out=zs[:], in0=zs[:], scalar=NEG, in1=zs[:],
                                op0=AL.mult, op1=AL.max)
                            ws = ef.tile([128, 1], F32, tag="ws")
                            nc.scalar.activation(ws[:], zs[:], ACT.Exp)
                            nc.vector.tensor_tensor(
                                out=ws[:], in0=ws[:],
                                in1=m01[:, blk_id:blk_id + 1], op=AL.mult)
                            nc.vector.tensor_tensor(
                                out=ws[:], in0=ws[:],
                                in1=dstage[l][:, blk_id:blk_id + 1],
                                op=AL.mult)
                            nc.vector.tensor_copy(
                                row[:, 64:65], ones_bf[:])
                            nc.vector.scalar_tensor_tensor(
                                out=ps[:, csl], in0=row[:, :],
                                scalar=ws[:], in1=ps[:, csl],
                                op0=AL.mult, op1=AL.add)
                            # divide + bias + relu
                            den = ef.tile([128, 1], F32, tag="den")
                            nc.vector.tensor_tensor(
                                out=den[:],
                                in0=ps[:, csl.start + 64:csl.start + 65],
                                in1=mpad[:, blk_id:blk_id + 1], op=AL.add)
                            rec = ef.tile([128, 1], F32, tag="rec")
                            nc.vector.reciprocal(rec[:], den[:])
                            hmid = ef.tile([128, 64], F32, tag="hmid")
                            nc.vector.scalar_tensor_tensor(
                                out=hmid[:],
                                in0=ps[:, csl.start:csl.start + 64],
                                scalar=rec[:], in1=b1t[:] if l == 0 else b2t[:],
                                op0=AL.mult, op1=AL.add)
                            hout = ef.tile([128, 64], F32, tag="hout")
                            nc.scalar.activation(hout[:], hmid[:], ACT.Relu)
                            # mask pads to exactly 0 (safe for max: h1 >= 0)
                            nc.vector.tensor_scalar_mul(
                                hout[:], hout[:], m01[:, blk_id:blk_id + 1])
                            # transpose (PE) -> [64, 128]
                            hT = ept.tile([64, 128], F32, space="PSUM",
                                          tag="hT")
                            nc.tensor.transpose(hT[:], hout[:], ident[:])
                            if l == 0:
                                nc.vector.tensor_copy(
                                    h_T[:, 128 * blk_id:128 * (blk_id + 1)],
                                    hT[:])
                            # sum pool
                            red = ef.tile([64, GPC], F32, tag="red")
                            nc.vector.tensor_reduce(
                                red[:],
                                hT[:].rearrange("f (g e) -> f g e", g=GPC),
                                axis=mybir.AxisListType.X, op=AL.add)
                            nc.vector.tensor_tensor(
                                out=pool_sm[l][:], in0=pool_sm[l][:],
                                in1=red[:], op=AL.add)
                            # max pool
                            redm = ef.tile([64, GPC], F32, tag="redm")
                            nc.vector.tensor_reduce(
                                redm[:],
                                hT[:].rearrange("f (g e) -> f g e", g=GPC),
                                axis=mybir.AxisListType.X, op=AL.max)
                            nc.vector.tensor_tensor(
                                out=pool_mx[l][:], in0=pool_mx[l][:],
                                in1=redm[:], op=AL.max)

            edge_pass(0)

            # ================= table 2 build =================
            with (
                tc.tile_pool(name="p3", bufs=3) as p3,
                tc.tile_pool(name="p3ps", bufs=2, space="PSUM") as p3ps,
            ):
                for i in range(NB):
                    hps = p3ps.tile([128, 66], F32, space="PSUM", tag="hps2")
                    nc.tensor.matmul(
                        hps[:], lhsT=h_T[:, 128 * i:128 * (i + 1)],
                        rhs=waug2_sb[:], start=True, stop=True)
                    row = p3.tile([128, 65], BF, tag="row2")
                    nc.vector.tensor_copy(row[:], hps[:, 0:65])
                    nc.sync.dma_start(
                        tab_own[1][128 * i:128 * (i + 1), 0:65], row[:])
                    nc.vector.tensor_copy(dstage[1][:, i:i + 1], hps[:, 65:66])
                dt_ps = p3ps.tile([NB, 128], F32, space="PSUM", tag="dt2")
                nc.tensor.transpose(dt_ps[:], dstage[1][:], ident[:])
                dt_sb = p3.tile([NB, 128], F32, tag="dts2")
                nc.vector.tensor_copy(dt_sb[:], dt_ps[:])
                nc.sync.dma_start(d2d[1][:, :], dt_sb[:])

            nc.gpsimd.collective_compute(
                "AllGather", AL.bypass,
                replica_groups=[list(range(NCORES))],
                ins=[tab_own[1].opt()], outs=[table[1].opt()])

            edge_pass(1)

            if DEBUG:
                nc.sync.dma_start(dbg_tab[:, :], tab_own[0][:, 0:65])
                dbg_d_sb = stp.tile([NB, 128], F32, name="dbgd")
                nc.sync.dma_start(dbg_d_sb[:], d2d[0][:, :])
                nc.sync.dma_start(dbg_d[:, :], dbg_d_sb[:])
                nc.sync.dma_start(dbg_hT[:, :], h_T[:])
                for li in range(2):
                    nc.sync.dma_start(
                        dbg_pool[:, li * GPC:(li + 1) * GPC], pool_mx[li][:])
                    nc.sync.dma_start(
                        dbg_pool[:, (2 + li) * GPC:(3 + li) * GPC],
                        pool_sm[li][:])

            # ================= pooling combine + MLP =================
            with (
                tc.tile_pool(name="p5", bufs=2) as p5,
                tc.tile_pool(name="p5ps", bufs=1, space="PSUM") as p5ps,
            ):
                icn = p5.tile([64, GPC], F32)
                nc.sync.dma_start(icn[:], invcnt[:, :])
                mxh = p5.tile([64, GPC], F32)
                nc.vector.tensor_tensor(out=mxh[:], in0=pool_mx[0][:],
                                        in1=pool_mx[1][:], op=AL.add)
                smh = p5.tile([64, GPC], F32)
                nc.vector.tensor_tensor(out=smh[:], in0=pool_sm[0][:],
                                        in1=pool_sm[1][:], op=AL.add)
                nc.vector.tensor_tensor(out=smh[:], in0=smh[:], in1=icn[:],
                                        op=AL.mult)
                # transpose to graph-major [GPC, 128] and AllGather
                zloc = p5.tile([GPC, 128], F32)
                mxT = p5ps.tile([GPC, 64], F32, space="PSUM", tag="mxT")
                nc.tensor.transpose(mxT[:], mxh[:], ident[0:64, 0:64])
                nc.vector.tensor_copy(zloc[:, 0:64], mxT[:])
                smT = p5ps.tile([GPC, 64], F32, space="PSUM", tag="smT")
                nc.tensor.transpose(smT[:], smh[:], ident[0:64, 0:64])
                nc.vector.tensor_copy(zloc[:, 64:128], smT[:])
                nc.sync.dma_start(pool_bounce_in[:, :], zloc[:])
                nc.gpsimd.collective_compute(
                    "AllGather", AL.bypass,
                    replica_groups=[list(range(NCORES))],
                    ins=[pool_bounce_in.opt()], outs=[pool_bounce_out.opt()])
                zg = p5.tile([G, 128], F32)
                nc.sync.dma_start(zg[:], pool_bounce_out[:, :])
                if DEBUG:
                    nc.sync.dma_start(dbg_z[:, :], zg[:])
                zT_ps = p5ps.tile([128, G], F32, space="PSUM", tag="zT")
                nc.tensor.transpose(zT_ps[:], zg[:], ident[:])
                zT = p5.tile([128, G], F32)
                nc.vector.tensor_copy(zT[:], zT_ps[:])
                l1w = p5.tile([128, 64], F32)
                nc.sync.dma_start(l1w[:], lin1_W[:, :])
                mlp_ps = p5ps.tile([G, 64], F32, space="PSUM", tag="mlp")
                nc.tensor.matmul(mlp_ps[:], lhsT=zT[:], rhs=l1w[:],
                                 start=True, stop=True)
                l1b = p5.tile([128, 64], F32)
                nc.sync.dma_start(l1b[:], lin1b_tile[:, :])
                z1 = p5.tile([G, 64], F32)
                nc.vector.tensor_tensor(out=z1[:], in0=mlp_ps[:], in1=l1b[:],
                                        op=AL.add)
                nc.scalar.activation(z1[:], z1[:], ACT.Relu)
                l2r = p5.tile([128, 64], F32)
                nc.sync.dma_start(l2r[:], lin2row[:, :])
                z2 = p5.tile([G, 64], F32)
                nc.vector.tensor_tensor(out=z2[:], in0=z1[:], in1=l2r[:],
                                        op=AL.mult)
                ored = p5.tile([G, 1], F32)
                nc.vector.tensor_reduce(ored[:], z2[:],
                                        axis=mybir.AxisListType.X, op=AL.add)
                b2c = p5.tile([128, 1], F32)
                nc.sync.dma_start(b2c[:], b2col[:, :])
                nc.vector.tensor_tensor(out=ored[:], in0=ored[:], in1=b2c[:],
                                        op=AL.add)
                nc.sync.dma_start(out_final[:, :], ored[:])

    nc.compile()
    return nc


# ---------------------------------------------------------------- entry
def kernel(**inputs) -> np.ndarray:
    tpl, per_core = _host_prep(inputs)
    nc = _build(tpl)
    in_maps = []
    for c in range(NCORES):
        pc = per_core[c]
        in_maps.append({
            "xT_own": pc["xT_own"],
            "idx_w": pc["idx_w"],
            "dst_loc": pc["dst_loc"],
            "mask01": pc["mask01"],
            "maskpad": pc["maskpad"],
            "npadneg": pc["npadneg"],
            "invcnt": pc["invcnt"],
            "waug1": pc["waug1"],
            "waug2_bf": pc["waug2_bf"],
            "b1_tile": pc["b1_tile"],
            "b1_tile2": pc["b1_tile2"],
            "lin1_W": pc["lin1_W"],
            "lin1b_tile": pc["lin1b_tile"],
            "lin2row": pc["lin2row"],
            "b2col": pc["b2col"],
            "rb_cols": pc["rb_cols"],
        })
    res = bass_utils.run_bass_kernel_spmd(
        nc, in_maps, core_ids=list(range(NCORES)))
    out = np.asarray(res.results[0]["out_final"]).reshape(G)
    return out.astype(np.float32)



# revision 5
# speedup vs baseline: 1.7167x; 1.7167x over previous
"""GAT (2-layer, heads=1) + pooling + MLP on 8 Trainium2 NeuronCores.

Strategy:
- Nodes are mapped to per-graph padded slots (SLOT = align128(max graph size)),
  graphs are sharded 16-per-core, and within each core slots are striped so
  that dst-block i holds slots s with s % NB == i.  Every 128-slot block then
  contains exactly 8 slots of each of the core's 16 graphs (partition p ->
  local graph p//8), which makes pooling segment boundaries compile-time.
- Edge aggregation: per-edge rows [h|s] are fetched with dma_gather (int16
  indices -> 4 src buckets); attention weights w = exp(leakyrelu(s_src +
  d_dst)) are computed on-chip (d expanded per edge via a one-hot *
  broadcast-d reduce); the segment softmax + feature sum is one matmul per
  128-edge tile accumulating [sum(w*h) | sum(w)] into a per-block PSUM.
- Self-loop edges are applied densely at finalize (no gather).
- Node tables ([h|s] rows, bf16) are built sharded and AllGathered; pooled
  [G,2H] is assembled with a single small AllGather; the final MLP is
  replicated.
- All host inputs are packed into ONE bf16 blob per core (x in bf16 split
  into two 64-row halves, gather indices stored once instead of 8x, dst
  rows as int8, small tensors bit-packed f32) and unpacked on device --
  the axon tunnel transfer (~46MB/s + ~70ms/array) dominates wall-clock.
"""
import sys

sys.path.insert(0, "/opt/trn_rl_repo")

import numpy as np
import ml_dtypes

import concourse.bacc as bacc
import concourse.bass as bass
import concourse.mybir as mybir
import concourse.tile as tile
from concourse import bass_utils
from concourse.masks import make_identity

bf16 = ml_dtypes.bfloat16
F32 = mybir.dt.float32
BF = mybir.dt.bfloat16
I16 = mybir.dt.int16
I8 = mybir.dt.int8
AL = mybir.AluOpType
ACT = mybir.ActivationFunctionType

NCORES = 8
G = 128
N = 100000
FIN = 64
H = 64
NEG = 0.2
NBUCKET = 4
GPC = G // NCORES  # graphs per core = 16
NEG_BIG = -1.0e30


# ---------------------------------------------------------------- host prep
def _host_prep(inputs):
    x = np.asarray(inputs["x"], np.float32)
    ei = np.asarray(inputs["edge_index"]).astype(np.int64)
    bid = np.asarray(inputs["batch_ids"]).astype(np.int64)

    cnt = np.bincount(bid, minlength=G).astype(np.int64)
    SLOT = int(np.ceil(max(cnt.max(), 128) / 128) * 128)
    NSLOT = GPC * SLOT
    NB = NSLOT // 128
    assert NB % 16 == 0 and NB % 4 == 0, NB
    NGRP = NB // 16
    NSLOT_G = NCORES * NSLOT
    BUCKET = NSLOT_G // NBUCKET
    assert BUCKET * NBUCKET == NSLOT_G and BUCKET <= 32768

    gstart = np.zeros(G + 1, np.int64)
    gstart[1:] = np.cumsum(cnt)
    rank = np.arange(N, dtype=np.int64) - gstart[bid]
    slot_of = bid * SLOT + rank  # graph-padded slot, 0..NSLOT_G

    def pi(s):
        c, sl = s // NSLOT, s % NSLOT
        return c * NSLOT, (sl % NB) * 128 + sl // NB

    core_base, loc = pi(slot_of)
    pi_of = core_base + loc  # global pi row of each node

    # permuted x per core (pad rows zero)
    x_pi = np.zeros((NCORES, NSLOT, FIN), np.float32)
    x_pi[pi_of // NSLOT, pi_of % NSLOT] = x

    # masks / counts per core
    # local slot (block i, partition p) <-> graph-slot p*NB + i
    pp, ii = np.meshgrid(np.arange(128), np.arange(NB), indexing="ij")
    gslot = pp * NB + ii  # [128, NB] graph-padded local slot
    within = gslot % SLOT  # rank within graph
    lg = gslot // SLOT  # local graph 0..15 (== pp//8)
    mask01 = np.zeros((NCORES, 128, NB), np.float32)
    for c in range(NCORES):
        real = within < cnt[c * GPC + lg]
        mask01[c] = real.astype(np.float32)

    # edges (no self loops in the gather path)
    src, dst = ei[0], ei[1]
    ps = pi_of[src]
    pd = pi_of[dst]
    core = pd // NSLOT
    blk = (pd % NSLOT) // 128
    bkt = ps // BUCKET
    grp = blk // 16

    cnts = np.zeros((NCORES, NB, NBUCKET), np.int64)
    np.add.at(cnts, (core, blk, bkt), 1)
    # uniform tiles-per-block within each (group, bucket)
    tpb = np.zeros((NGRP, NBUCKET), np.int64)  # tiles per block
    for g in range(NGRP):
        for b in range(NBUCKET):
            m = cnts[:, 16 * g:16 * g + 16, b].max()
            tpb[g, b] = max(1, int(np.ceil(m / 128)))
    seg_tiles = (16 * tpb).astype(np.int64)  # tiles per (g,b) segment
    TT = int(seg_tiles.sum())  # total tiles per core per layer
    assert TT % 2 == 0, TT
    TOTSLOT = TT * 128

    # slot offsets: order (g, b, block-within-group, slot)
    seg_off = np.zeros((NGRP, NBUCKET), np.int64)
    acc = 0
    for g in range(NGRP):
        for b in range(NBUCKET):
            seg_off[g, b] = acc
            acc += seg_tiles[g, b] * 128

    order = np.lexsort((bkt, blk))  # edges sorted by (blk, bkt); core split below
    src_local = np.zeros((NCORES, 128, TT), np.int16)
    dst_i8 = np.full((NCORES, 128, TT), -1, np.int8)
    for c in range(NCORES):
        sel = order[core[order] == c]
        sblk, sbkt = blk[sel], bkt[sel]
        sps, spd = ps[sel], pd[sel]
        # slot index for each edge: within its (g,b,block) run
        # run start: seg_off[g,b] + (blk%16)*tpb[g,b]*128; position = rank in run
        key = sblk * NBUCKET + sbkt
        # stable order already (blk, bkt); rank within run:
        runstart_mark = np.r_[True, key[1:] != key[:-1]]
        runid = np.cumsum(runstart_mark) - 1
        nruns = int(runid[-1]) + 1 if len(runid) else 0
        first = np.full(nruns, len(sel), np.int64)
        np.minimum.at(first, runid, np.arange(len(sel)))
        pos = np.arange(len(sel)) - first[runid]
        gg = sblk // 16
        slot = (seg_off[gg, sbkt] + (sblk % 16) * tpb[gg, sbkt] * 128 + pos)
        assert (pos < tpb[gg, sbkt] * 128).all()
        p_ = slot % 128
        t_ = slot // 128
        src_local[c, p_, t_] = (sps - sbkt * BUCKET).astype(np.int16)
        dst_i8[c, p_, t_] = (spd % NSLOT % 128).astype(np.int8)

    # wrapped int16 gather indices, stored compactly: per (g,b) segment the
    # [16, ntile*8] wrap is folded into [128, ntile] (row 16a+r holds
    # wseg[r, a*ntile : (a+1)*ntile]); the 8x replication the gather engine
    # wants is re-created on device with 8 DMAs per segment.
    idx_pack = np.zeros((NCORES, 128, TT), np.int16)
    for c in range(NCORES):
        flat = np.zeros(TOTSLOT, np.int16)
        sl = src_local[c]
        flat[np.arange(TOTSLOT)] = sl[np.arange(TOTSLOT) % 128,
                                      np.arange(TOTSLOT) // 128]
        w = flat.reshape(TOTSLOT // 16, 16).T  # [16, TT*8]
        for g in range(NGRP):
            for b in range(NBUCKET):
                off_t = int(seg_off[g, b]) // 128
                ntile = int(seg_tiles[g, b])
                wseg = w[:, off_t * 8:(off_t + ntile) * 8]
                idx_pack[c][:, off_t:off_t + ntile] = (
                    wseg.reshape(16, 8, ntile).transpose(1, 0, 2)
                    .reshape(128, ntile))

    # weights
    W1 = np.asarray(inputs["W1"], np.float32)
    W2 = np.asarray(inputs["W2"], np.float32)
    waug1 = np.concatenate(
        [W1, (W1 @ np.asarray(inputs["a_src1"], np.float32))[:, None],
         (W1 @ np.asarray(inputs["a_dst1"], np.float32))[:, None]], axis=1)
    waug2 = np.concatenate(
        [W2, (W2 @ np.asarray(inputs["a_src2"], np.float32))[:, None],
         (W2 @ np.asarray(inputs["a_dst2"], np.float32))[:, None]], axis=1)

    b1 = np.asarray(inputs["b1"], np.float32)
    b2v = np.asarray(inputs["b2"], np.float32)
    lin1_W = np.asarray(inputs["lin1_W"], np.float32)
    lin1_b = np.asarray(inputs["lin1_b"], np.float32)
    lin2_W = np.asarray(inputs["lin2_W"], np.float32)
    lin2_b = np.asarray(inputs["lin2_b"], np.float32)

    invcnt = np.zeros((NCORES, 64, GPC), np.float32)
    for c in range(NCORES):
        invcnt[c] = np.broadcast_to(
            1.0 / np.maximum(cnt[c * GPC:(c + 1) * GPC], 1.0), (64, GPC))

    # ---- blob layout (bf16 columns)
    XH = NSLOT // 2
    IOFF = XH
    DOFF = XH + TT
    SOFF = XH + TT + TT // 2
    S = NB + 277  # f32 cols in the small region
    CB = SOFF + 2 * S

    per_core = []
    for c in range(NCORES):
        blob = np.zeros((128, CB * 2), np.uint8)
        xT_bf = np.ascontiguousarray(x_pi[c].T).astype(bf16)  # [64, NSLOT]
        blob[0:64, 0:XH * 2] = np.ascontiguousarray(
            xT_bf[:, 0:XH]).view(np.uint8)
        blob[64:128, 0:XH * 2] = np.ascontiguousarray(
            xT_bf[:, XH:]).view(np.uint8)
        blob[:, IOFF * 2:(IOFF + TT) * 2] = idx_pack[c].view(np.uint8)
        blob[:, DOFF * 2:DOFF * 2 + TT] = dst_i8[c].view(np.uint8)
        sm = np.zeros((128, S), np.float32)
        sm[:, 0:NB] = mask01[c]
        sm[:, NB:NB + 64] = lin1_W
        sm[0:64, NB + 64:NB + 130] = waug1
        sm[0:64, NB + 130:NB + 196] = waug2
        sm[0:64, NB + 196:NB + 196 + GPC] = invcnt[c]
        SM_B = NB + 196 + GPC
        sm[0, SM_B:SM_B + 64] = b1
        sm[1, SM_B:SM_B + 64] = b2v
        sm[2, SM_B:SM_B + 64] = lin1_b
        sm[3, SM_B:SM_B + 64] = lin2_W[:, 0]
        sm[0, SM_B + 64] = lin2_b[0]
        blob[:, SOFF * 2:SOFF * 2 + S * 4] = sm.view(np.uint8)
        per_core.append({"blob": blob.view(bf16)})

    tpl = dict(SLOT=SLOT, NSLOT=NSLOT, NB=NB, NGRP=NGRP, NSLOT_G=NSLOT_G,
               BUCKET=BUCKET, tpb=tpb, seg_tiles=seg_tiles, seg_off=seg_off,
               TT=TT, XH=XH, IOFF=IOFF, DOFF=DOFF, SOFF=SOFF, S=S, CB=CB)
    return tpl, per_core


# ---------------------------------------------------------------- device bld
def _build(tpl):
    NSLOT = tpl["NSLOT"]
    NB = tpl["NB"]
    NGRP = tpl["NGRP"]
    NSLOT_G = tpl["NSLOT_G"]
    BUCKET = tpl["BUCKET"]
    tpb = tpl["tpb"]
    seg_tiles = tpl["seg_tiles"]
    TT = tpl["TT"]
    XH = tpl["XH"]
    IOFF = tpl["IOFF"]
    DOFF = tpl["DOFF"]
    SOFF = tpl["SOFF"]
    S = tpl["S"]
    CB = tpl["CB"]
    SM_MASK = 0
    SM_L1W = NB
    SM_WG1 = NB + 64
    SM_WG2 = NB + 130
    SM_ICN = NB + 196
    SM_B = NB + 196 + GPC
    SM_B2 = SM_B + 64

    nc = bacc.Bacc("TRN2", target_bir_lowering=False, debug=False,
                   num_devices=NCORES)

    blob = nc.dram_tensor("blob", [128, CB], BF, kind="ExternalInput")
    out_final = nc.dram_tensor("out_final", [128, 1], F32,
                               kind="ExternalOutput")

    with tile.TileContext(nc) as tc:
        with (
            tc.tile_pool(name="const", bufs=1) as cp,
            tc.tile_pool(name="stage", bufs=1) as stp,
            tc.tile_pool(name="dram", bufs=1, space="DRAM") as dr,
        ):
            # ---- constants in SBUF
            iota_i = cp.tile([128, 128], mybir.dt.int32)
            nc.gpsimd.iota(iota_i[:], pattern=[[1, 128]], base=0,
                           channel_multiplier=0)
            iota_bf = cp.tile([128, 128], BF)
            nc.vector.tensor_copy(iota_bf[:], iota_i[:])
            ident = cp.tile([128, 128], F32)
            make_identity(nc, ident[:])
            ones_bf = cp.tile([128, 1], BF)
            nc.gpsimd.memset(ones_bf[:], 1.0)
            onec = cp.tile([128, 1], F32)
            nc.gpsimd.memset(onec[:], 1.0)

            # ---- unpack the blob
            smalls = cp.tile([128, S], F32)
            nc.sync.dma_start(
                smalls[:], blob[:, SOFF:SOFF + 2 * S].bitcast(F32))
            m01 = smalls[:, SM_MASK:SM_MASK + NB]
            mpad = cp.tile([128, NB], F32)
            nc.vector.scalar_tensor_tensor(
                out=mpad[:], in0=m01, scalar=-1.0,
                in1=onec[:].to_broadcast([128, NB]),
                op0=AL.mult, op1=AL.add)
            waug1_sb = cp.tile([64, 66], BF)
            nc.vector.tensor_copy(waug1_sb[:], smalls[0:64, SM_WG1:SM_WG1 + 66])
            waug2_sb = cp.tile([64, 66], BF)
            nc.vector.tensor_copy(waug2_sb[:], smalls[0:64, SM_WG2:SM_WG2 + 66])
            btiles = cp.tile([128, 4, 64], F32)
            nc.sync.dma_start(
                btiles[:],
                blob[0:4, SOFF + 2 * SM_B:SOFF + 2 * SM_B + 128].bitcast(
                    F32).rearrange("(o r) c -> o r c", o=1).to_broadcast(
                    [128, 4, 64]))
            b1t = btiles[:, 0, :]
            b2t = btiles[:, 1, :]
            l1bt = btiles[:, 2, :]
            l2rt = btiles[:, 3, :]
            b2c = cp.tile([128, 1], F32)
            nc.sync.dma_start(
                b2c[:],
                blob[0:1, SOFF + 2 * SM_B2:SOFF + 2 * SM_B2 + 2].bitcast(
                    F32).to_broadcast([128, 1]))
            dst_i8 = stp.tile([128, TT], I8, name="dsti8")
            nc.sync.dma_start(dst_i8[:], blob[:, DOFF:DOFF + TT // 2].bitcast(I8))
            dl_all = cp.tile([128, TT], F32)
            nc.vector.tensor_copy(dl_all[:], dst_i8[:])

            # persistent staging
            h_T = stp.tile([64, NSLOT], BF, tag="h_T", name="h_T")  # layer1 out, pi order
            dstage = [stp.tile([128, NB], F32, tag=f"dstage{l}", name=f"dstage{l}")
                      for l in range(2)]
            pool_mx = [stp.tile([64, GPC], F32, tag=f"pmx{l}", name=f"pmx{l}")
                       for l in range(2)]
            pool_sm = [stp.tile([64, GPC], F32, tag=f"psm{l}", name=f"psm{l}")
                       for l in range(2)]
            for l in range(2):
                nc.vector.memset(pool_mx[l][:], NEG_BIG)
                nc.vector.memset(pool_sm[l][:], 0.0)

            # DRAM scratch
            table = [dr.tile([NSLOT_G, 128], BF, tag=f"tab{l}", name=f"tab{l}")
                     for l in range(2)]
            tab_own = [dr.tile([NSLOT, 128], BF, tag=f"tabown{l}", name=f"tabown{l}")
                       for l in range(2)]
            d2d = [dr.tile([NB, 128], F32, tag=f"d2d{l}", name=f"d2d{l}") for l in range(2)]
            pool_bounce_in = dr.tile([GPC, 128], F32)
            pool_bounce_out = dr.tile([G, 128], F32)

            # ===== table 1 build: own shard of [h|s] rows + d column =====
            with (
                tc.tile_pool(name="p1", bufs=3) as p1,
                tc.tile_pool(name="p1x", bufs=3) as p1x,
                tc.tile_pool(name="p1ps", bufs=2, space="PSUM") as p1ps,
            ):
                QB = NB // 4
                for s in range(4):
                    half = (s * QB) // (NB // 2)
                    within = (s * QB) % (NB // 2)
                    rows0 = 64 * half
                    coff = within * 128
                    xT = p1x.tile([64, QB * 128], BF, tag="xTs")
                    nc.sync.dma_start(
                        xT[:], blob[rows0:rows0 + 64, coff:coff + QB * 128])
                    rows = p1.tile([128, QB, 65], BF, tag="rows")
                    for j in range(QB):
                        blk_id = s * QB + j
                        hps = p1ps.tile([128, 66], F32, space="PSUM",
                                        tag="hps")
                        nc.tensor.matmul(
                            hps[:], lhsT=xT[:, 128 * j:128 * (j + 1)],
                            rhs=waug1_sb[:], start=True, stop=True)
                        nc.scalar.activation(rows[:, j, :], hps[:, 0:65],
                                             ACT.Copy)
                        nc.vector.tensor_copy(
                            dstage[0][:, blk_id:blk_id + 1],
                            hps[:, 65:66])
                    nc.sync.dma_start(
                        tab_own[0].rearrange("(s r) c -> r s c", r=128)[
                            :, s * QB:(s + 1) * QB, 0:65],
                        rows[:, 0:QB, :])
                # d transpose -> DRAM [NB, 128]
                dt_ps = p1ps.tile([NB, 128], F32, space="PSUM", tag="dt")
                nc.tensor.transpose(dt_ps[:], dstage[0][:], ident[:])
                dt_sb = p1.tile([NB, 128], F32, tag="dts")
                nc.vector.tensor_copy(dt_sb[:], dt_ps[:])
                nc.sync.dma_start(d2d[0][:, :], dt_sb[:])

            nc.gpsimd.collective_compute(
                "AllGather", AL.bypass,
                replica_groups=[list(range(NCORES))],
                ins=[tab_own[0].opt()], outs=[table[0].opt()])

            # ================= edge pass (both layers) =================
            def edge_pass(l):
                tab = table[l]
                with (
                    tc.tile_pool(name=f"eg{l}", bufs=2) as eg,
                    tc.tile_pool(name=f"eq{l}", bufs=2) as eqp,
                    tc.tile_pool(name=f"ep{l}", bufs=1, space="PSUM") as eps,
                    tc.tile_pool(name=f"et{l}", bufs=2, space="PSUM") as ept,
                    tc.tile_pool(name=f"ef{l}", bufs=3) as ef,
                ):
                    for g in range(NGRP):
                        # broadcast d rows of the group's 16 blocks
                        dbc = ef.tile([128, 16, 128], F32, tag="dbc")
                        nc.sync.dma_start(
                            dbc[:],
                            d2d[l][16 * g:16 * (g + 1), :].rearrange(
                                "(o r) c -> o r c", o=1).to_broadcast(
                                [128, 16, 128]))
                        dbc_bf = ef.tile([128, 16, 128], BF, tag="dbcb")
                        nc.vector.tensor_copy(dbc_bf[:], dbc[:])

                        psums = [eps.tile([128, 260], F32, space="PSUM",
                                          tag=f"ps{k}", name=f"ps{k}")
                                 for k in range(4)]
                        for ps_ in psums:
                            nc.vector.memset(ps_[:], 0.0)

                        for b in range(NBUCKET):
                            ntile = int(seg_tiles[g, b])
                            Tb = int(tpb[g, b])
                            off = int(tpl["seg_off"][g, b]) // 128  # tile off
                            # gather the whole (g,b) segment; indices come
                            # from the compact blob region, replicated 8x
                            idx_sb = eg.tile([128, ntile * 8], I16, tag="idx")
                            for k in range(8):
                                nc.sync.dma_start(
                                    idx_sb[16 * k:16 * (k + 1), :].rearrange(
                                        "p (a c) -> p a c", a=8),
                                    blob[:, IOFF + off:IOFF + off + ntile]
                                    .bitcast(I16).rearrange(
                                        "(a p) c -> p a c", p=16))
                            ch = eg.tile([128, ntile, 128], BF, tag="ch")
                            nc.gpsimd.dma_gather(
                                out_ap=ch[:],
                                in_ap=tab[b * BUCKET:(b + 1) * BUCKET, :],
                                idxs_ap=idx_sb[:],
                                num_idxs=ntile * 128,
                                num_idxs_reg=ntile * 128,
                                elem_size=128,
                                single_packet=False)
                            dl = dl_all[:, off:off + ntile]
                            # per tile: lhsT = (iota==dstloc)*|d|  (the |d|
                            # row-scale cancels in the softmax division);
                            # accum of (iota==dstloc)*d gives d_edge.
                            lhsT = eqp.tile([128, ntile, 128], BF, tag="eq")
                            dedge = ef.tile([128, ntile], F32, tag="dedge")
                            for tt in range(ntile):
                                k16 = tt // Tb
                                nc.vector.scalar_tensor_tensor(
                                    out=lhsT[:, tt, :], in0=iota_bf[:],
                                    scalar=dl[:, tt:tt + 1],
                                    in1=dbc_bf[:, k16, :],
                                    op0=AL.is_equal, op1=AL.mult,
                                    accum_out=dedge[:, tt:tt + 1])
                            # z = s + d ; w = exp(max(0.2 z, z))
                            z = ef.tile([128, ntile], F32, tag="z")
                            nc.vector.tensor_tensor(
                                out=z[:], in0=ch[:, :, 64], in1=dedge[:],
                                op=AL.add)
                            nc.vector.scalar_tensor_tensor(
                                out=z[:], in0=z[:], scalar=NEG, in1=z[:],
                                op0=AL.mult, op1=AL.max)
                            w = ef.tile([128, ntile], F32, tag="w")
                            nc.scalar.activation(w[:], z[:], ACT.Exp)
                            # ones into s slot for the denominator column
                            nc.vector.tensor_copy(
                                ch[:, :, 64],
                                ones_bf[:].to_broadcast([128, ntile]))
                            # rhs2 = ch[:, :, 0:65] * w (bulk)
                            ch2 = eqp.tile([128, ntile, 65], BF, tag="dmul",
                                           name="ch2")
                            nc.vector.tensor_tensor(
                                out=ch2[:], in0=ch[:, :, 0:65],
                                in1=w[:].to_broadcast([128, ntile, 65]),
                                op=AL.mult)
                            # matmuls
                            for i in range(16):
                                ps = psums[i // 4]
                                csl = slice(65 * (i % 4), 65 * (i % 4) + 65)
                                for t in range(Tb):
                                    tt = i * Tb + t
                                    nc.tensor.matmul(
                                        ps[:, csl],
                                        lhsT=lhsT[:, tt, :],
                                        rhs=ch2[:, tt, :],
                                        start=False,
                                        stop=(b == NBUCKET - 1 and t == Tb - 1))

                        # ---- finalize the group's 16 blocks
                        for i in range(16):
                            blk_id = 16 * g + i
                            ps = psums[i // 4]
                            csl = slice(65 * (i % 4), 65 * (i % 4) + 65)
                            # self loops (dense): rows of own dst block
                            row = ef.tile([128, 65], BF, tag="slrow")
                            nc.sync.dma_start(
                                row[:], tab_own[l][128 * blk_id:
                                                   128 * blk_id + 128, 0:65])
                            zs = ef.tile([128, 1], F32, tag="zs")
                            nc.vector.tensor_tensor(
                                out=zs[:], in0=row[:, 64:65],
                                in1=dstage[l][:, blk_id:blk_id + 1],
                                op=AL.add)
                            nc.vector.scalar_tensor_tensor(
                                out=zs[:], in0=zs[:], scalar=NEG, in1=zs[:],
                                op0=AL.mult, op1=AL.max)
                            ws = ef.tile([128, 1], F32, tag="ws")
                            nc.scalar.activation(ws[:], zs[:], ACT.Exp)
                            nc.vector.tensor_tensor(
                                out=ws[:], in0=ws[:],
                                in1=m01[:, blk_id:blk_id + 1], op=AL.mult)
                            nc.vector.tensor_tensor(
                                out=ws[:], in0=ws[:],
                                in1=dstage[l][:, blk_id:blk_id + 1],
                                op=AL.mult)
                            nc.vector.tensor_copy(
                                row[:, 64:65], ones_bf[:])
                            nc.vector.scalar_tensor_tensor(
                                out=ps[:, csl], in0=row[:, :],
                                scalar=ws[:], in1=ps[:, csl],
                                op0=AL.mult, op1=AL.add)
                            # divide + bias + relu
                            den = ef.tile([128, 1], F32, tag="den")
                            nc.vector.tensor_tensor(
                                out=den[:],
                                in0=ps[:, csl.start + 64:csl.start + 65],
                                in1=mpad[:, blk_id:blk_id + 1], op=AL.add)
                            rec = ef.tile([128, 1], F32, tag="rec")
                            nc.vector.reciprocal(rec[:], den[:])
                            hmid = ef.tile([128, 64], F32, tag="hmid")
                            nc.vector.scalar_tensor_tensor(
                                out=hmid[:],
                                in0=ps[:, csl.start:csl.start + 64],
                                scalar=rec[:], in1=b1t if l == 0 else b2t,
                                op0=AL.mult, op1=AL.add)
                            hout = ef.tile([128, 64], F32, tag="hout")
                            nc.scalar.activation(hout[:], hmid[:], ACT.Relu)
                            # mask pads to exactly 0 (safe for max: h1 >= 0)
                            nc.vector.tensor_scalar_mul(
                                hout[:], hout[:], m01[:, blk_id:blk_id + 1])
                            # transpose (PE) -> [64, 128]
                            hT = ept.tile([64, 128], F32, space="PSUM",
                                          tag="hT")
                            nc.tensor.transpose(hT[:], hout[:], ident[:])
                            if l == 0:
                                nc.vector.tensor_copy(
                                    h_T[:, 128 * blk_id:128 * (blk_id + 1)],
                                    hT[:])
                            # sum pool
                            red = ef.tile([64, GPC], F32, tag="red")
                            nc.vector.tensor_reduce(
                                red[:],
                                hT[:].rearrange("f (g e) -> f g e", g=GPC),
                                axis=mybir.AxisListType.X, op=AL.add)
                            nc.vector.tensor_tensor(
                                out=pool_sm[l][:], in0=pool_sm[l][:],
                                in1=red[:], op=AL.add)
                            # max pool
                            redm = ef.tile([64, GPC], F32, tag="redm")
                            nc.vector.tensor_reduce(
                                redm[:],
                                hT[:].rearrange("f (g e) -> f g e", g=GPC),
                                axis=mybir.AxisListType.X, op=AL.max)
                            nc.vector.tensor_tensor(
                                out=pool_mx[l][:], in0=pool_mx[l][:],
                                in1=redm[:], op=AL.max)

            edge_pass(0)

            # ================= table 2 build =================
            with (
                tc.tile_pool(name="p3", bufs=3) as p3,
                tc.tile_pool(name="p3ps", bufs=2, space="PSUM") as p3ps,
            ):
                for i in range(NB):
                    hps = p3ps.tile([128, 66], F32, space="PSUM", tag="hps2")
                    nc.tensor.matmul(
                        hps[:], lhsT=h_T[:, 128 * i:128 * (i + 1)],
                        rhs=waug2_sb[:], start=True, stop=True)
                    row = p3.tile([128, 65], BF, tag="row2")
                    nc.vector.tensor_copy(row[:], hps[:, 0:65])
                    nc.sync.dma_start(
                        tab_own[1][128 * i:128 * (i + 1), 0:65], row[:])
                    nc.vector.tensor_copy(dstage[1][:, i:i + 1], hps[:, 65:66])
                dt_ps = p3ps.tile([NB, 128], F32, space="PSUM", tag="dt2")
                nc.tensor.transpose(dt_ps[:], dstage[1][:], ident[:])
                dt_sb = p3.tile([NB, 128], F32, tag="dts2")
                nc.vector.tensor_copy(dt_sb[:], dt_ps[:])
                nc.sync.dma_start(d2d[1][:, :], dt_sb[:])

            nc.gpsimd.collective_compute(
                "AllGather", AL.bypass,
                replica_groups=[list(range(NCORES))],
                ins=[tab_own[1].opt()], outs=[table[1].opt()])

            edge_pass(1)

            # ================= pooling combine + MLP =================
            with (
                tc.tile_pool(name="p5", bufs=2) as p5,
                tc.tile_pool(name="p5ps", bufs=1, space="PSUM") as p5ps,
            ):
                icn = smalls[0:64, SM_ICN:SM_ICN + GPC]
                mxh = p5.tile([64, GPC], F32)
                nc.vector.tensor_tensor(out=mxh[:], in0=pool_mx[0][:],
                                        in1=pool_mx[1][:], op=AL.add)
                smh = p5.tile([64, GPC], F32)
                nc.vector.tensor_tensor(out=smh[:], in0=pool_sm[0][:],
                                        in1=pool_sm[1][:], op=AL.add)
                nc.vector.tensor_tensor(out=smh[:], in0=smh[:], in1=icn,
                                        op=AL.mult)
                # transpose to graph-major [GPC, 128] and AllGather
                zloc = p5.tile([GPC, 128], F32)
                mxT = p5ps.tile([GPC, 64], F32, space="PSUM", tag="mxT")
                nc.tensor.transpose(mxT[:], mxh[:], ident[0:64, 0:64])
                nc.vector.tensor_copy(zloc[:, 0:64], mxT[:])
                smT = p5ps.tile([GPC, 64], F32, space="PSUM", tag="smT")
                nc.tensor.transpose(smT[:], smh[:], ident[0:64, 0:64])
                nc.vector.tensor_copy(zloc[:, 64:128], smT[:])
                nc.sync.dma_start(pool_bounce_in[:, :], zloc[:])
                nc.gpsimd.collective_compute(
                    "AllGather", AL.bypass,
                    replica_groups=[list(range(NCORES))],
                    ins=[pool_bounce_in.opt()], outs=[pool_bounce_out.opt()])
                zg = p5.tile([G, 128], F32)
                nc.sync.dma_start(zg[:], pool_bounce_out[:, :])
                zT_ps = p5ps.tile([128, G], F32, space="PSUM", tag="zT")
                nc.tensor.transpose(zT_ps[:], zg[:], ident[:])
                zT = p5.tile([128, G], F32)
                nc.vector.tensor_copy(zT[:], zT_ps[:])
                mlp_ps = p5ps.tile([G, 64], F32, space="PSUM", tag="mlp")
                nc.tensor.matmul(mlp_ps[:], lhsT=zT[:],
                                 rhs=smalls[:, SM_L1W:SM_L1W + 64],
                                 start=True, stop=True)
                z1 = p5.tile([G, 64], F32)
                nc.vector.tensor_tensor(out=z1[:], in0=mlp_ps[:], in1=l1bt,
                                        op=AL.add)
                nc.scalar.activation(z1[:], z1[:], ACT.Relu)
                z2 = p5.tile([G, 64], F32)
                nc.vector.tensor_tensor(out=z2[:], in0=z1[:], in1=l2rt,
                                        op=AL.mult)
                ored = p5.tile([G, 1], F32)
                nc.vector.tensor_reduce(ored[:], z2[:],
                                        axis=mybir.AxisListType.X, op=AL.add)
                nc.vector.tensor_tensor(out=ored[:], in0=ored[:], in1=b2c[:],
                                        op=AL.add)
                nc.sync.dma_start(out_final[:, :], ored[:])

    nc.compile()
    return nc


# ---------------------------------------------------------------- entry
def kernel(**inputs) -> np.ndarray:
    tpl, per_core = _host_prep(inputs)
    nc = _build(tpl)
    in_maps = [{"blob": per_core[c]["blob"]} for c in range(NCORES)]
    res = bass_utils.run_bass_kernel_spmd(
        nc, in_maps, core_ids=list(range(NCORES)))
    out = np.asarray(res.results[0]["out_final"]).reshape(G)
    return out.astype(np.float32)


# revision 6
# speedup vs baseline: 3.5010x; 2.0394x over previous
"""GAT (2-layer, heads=1) + pooling + MLP on 8 Trainium2 NeuronCores.

Strategy:
- Nodes are mapped to per-graph padded slots (SLOT = align128(max graph size)),
  graphs are sharded 16-per-core, and within each core slots are striped so
  that dst-block i holds slots s with s % NB == i.  Every 128-slot block then
  contains exactly 8 slots of each of the core's 16 graphs (partition p ->
  local graph p//8), which makes pooling segment boundaries compile-time.
- Edge aggregation: per-edge rows [h|s] are fetched with dma_gather (int16
  indices -> 4 src buckets); attention weights w = exp(leakyrelu(s_src +
  d_dst)) are computed on-chip (d expanded per edge via a one-hot *
  broadcast-d reduce); the segment softmax + feature sum is one matmul per
  128-edge tile accumulating [sum(w*h) | sum(w)] into a per-block PSUM.
- Self-loop edges are applied densely at finalize (no gather).
- Node tables ([h|s] rows, bf16) are built sharded and AllGathered; pooled
  [G,2H] is assembled with a single small AllGather; the final MLP is
  replicated.
- All host inputs are packed into ONE bf16 blob per core (x in bf16 split
  into two 64-row halves, gather indices stored once instead of 8x, dst
  rows as int8, small tensors bit-packed f32) and unpacked on device --
  the axon tunnel transfer (~46MB/s + ~70ms/array) dominates wall-clock.
"""
import sys

sys.path.insert(0, "/opt/trn_rl_repo")

import numpy as np
import ml_dtypes

try:  # persistent XLA compile cache: repeat calls skip the NEFF re-wrap
    import jax

    jax.config.update("jax_compilation_cache_dir", "/tmp/jaxcache")
    jax.config.update("jax_persistent_cache_min_entry_size_bytes", -1)
    jax.config.update("jax_persistent_cache_min_compile_time_secs", 0)
except Exception:
    pass

import concourse.bacc as bacc
import concourse.bass as bass
import concourse.mybir as mybir
import concourse.tile as tile
from concourse import bass_utils
from concourse.masks import make_identity

bf16 = ml_dtypes.bfloat16
F32 = mybir.dt.float32
BF = mybir.dt.bfloat16
I16 = mybir.dt.int16
I8 = mybir.dt.int8
AL = mybir.AluOpType
ACT = mybir.ActivationFunctionType

NCORES = 8
G = 128
N = 100000
FIN = 64
H = 64
NEG = 0.2
NBUCKET = 4
GPC = G // NCORES  # graphs per core = 16
NEG_BIG = -1.0e30


# ---------------------------------------------------------------- host prep
def _host_prep(inputs):
    x = np.asarray(inputs["x"], np.float32)
    ei = np.asarray(inputs["edge_index"]).astype(np.int64)
    bid = np.asarray(inputs["batch_ids"]).astype(np.int64)

    cnt = np.bincount(bid, minlength=G).astype(np.int64)
    SLOT = int(np.ceil(max(cnt.max(), 128) / 128) * 128)
    NSLOT = GPC * SLOT
    NB = NSLOT // 128
    assert NB % 16 == 0 and NB % 4 == 0, NB
    NGRP = NB // 16
    NSLOT_G = NCORES * NSLOT
    BUCKET = NSLOT_G // NBUCKET
    assert BUCKET * NBUCKET == NSLOT_G and BUCKET <= 32768

    gstart = np.zeros(G + 1, np.int64)
    gstart[1:] = np.cumsum(cnt)
    rank = np.arange(N, dtype=np.int64) - gstart[bid]
    slot_of = bid * SLOT + rank  # graph-padded slot, 0..NSLOT_G

    def pi(s):
        c, sl = s // NSLOT, s % NSLOT
        return c * NSLOT, (sl % NB) * 128 + sl // NB

    core_base, loc = pi(slot_of)
    pi_of = core_base + loc  # global pi row of each node

    # permuted x per core (pad rows zero)
    x_pi = np.zeros((NCORES, NSLOT, FIN), np.float32)
    x_pi[pi_of // NSLOT, pi_of % NSLOT] = x

    # masks / counts per core
    # local slot (block i, partition p) <-> graph-slot p*NB + i
    pp, ii = np.meshgrid(np.arange(128), np.arange(NB), indexing="ij")
    gslot = pp * NB + ii  # [128, NB] graph-padded local slot
    within = gslot % SLOT  # rank within graph
    lg = gslot // SLOT  # local graph 0..15 (== pp//8)
    mask01 = np.zeros((NCORES, 128, NB), np.float32)
    for c in range(NCORES):
        real = within < cnt[c * GPC + lg]
        mask01[c] = real.astype(np.float32)

    # edges (no self loops in the gather path)
    src, dst = ei[0], ei[1]
    ps = pi_of[src]
    pd = pi_of[dst]
    core = pd // NSLOT
    blk = (pd % NSLOT) // 128
    bkt = ps // BUCKET
    grp = blk // 16

    cnts = np.zeros((NCORES, NB, NBUCKET), np.int64)
    np.add.at(cnts, (core, blk, bkt), 1)
    # uniform tiles-per-block within each (group, bucket)
    tpb = np.zeros((NGRP, NBUCKET), np.int64)  # tiles per block
    for g in range(NGRP):
        for b in range(NBUCKET):
            m = cnts[:, 16 * g:16 * g + 16, b].max()
            tpb[g, b] = max(1, int(np.ceil(m / 128)))
    seg_tiles = (16 * tpb).astype(np.int64)  # tiles per (g,b) segment
    TT = int(seg_tiles.sum())  # total tiles per core per layer
    assert TT % 2 == 0, TT
    TOTSLOT = TT * 128

    # slot offsets: order (g, b, block-within-group, slot)
    seg_off = np.zeros((NGRP, NBUCKET), np.int64)
    acc = 0
    for g in range(NGRP):
        for b in range(NBUCKET):
            seg_off[g, b] = acc
            acc += seg_tiles[g, b] * 128

    order = np.lexsort((bkt, blk))  # edges sorted by (blk, bkt); core split below
    src_local = np.zeros((NCORES, 128, TT), np.int16)
    dst_i8 = np.full((NCORES, 128, TT), -1, np.int8)
    for c in range(NCORES):
        sel = order[core[order] == c]
        sblk, sbkt = blk[sel], bkt[sel]
        sps, spd = ps[sel], pd[sel]
        # slot index for each edge: within its (g,b,block) run
        # run start: seg_off[g,b] + (blk%16)*tpb[g,b]*128; position = rank in run
        key = sblk * NBUCKET + sbkt
        # stable order already (blk, bkt); rank within run:
        runstart_mark = np.r_[True, key[1:] != key[:-1]]
        runid = np.cumsum(runstart_mark) - 1
        nruns = int(runid[-1]) + 1 if len(runid) else 0
        first = np.full(nruns, len(sel), np.int64)
        np.minimum.at(first, runid, np.arange(len(sel)))
        pos = np.arange(len(sel)) - first[runid]
        gg = sblk // 16
        slot = (seg_off[gg, sbkt] + (sblk % 16) * tpb[gg, sbkt] * 128 + pos)
        assert (pos < tpb[gg, sbkt] * 128).all()
        p_ = slot % 128
        t_ = slot // 128
        src_local[c, p_, t_] = (sps - sbkt * BUCKET).astype(np.int16)
        dst_i8[c, p_, t_] = (spd % NSLOT % 128).astype(np.int8)

    # wrapped int16 gather indices, stored compactly: per (g,b) segment the
    # [16, ntile*8] wrap is folded into [128, ntile] (row 16a+r holds
    # wseg[r, a*ntile : (a+1)*ntile]); the 8x replication the gather engine
    # wants is re-created on device with 8 DMAs per segment.
    idx_pack = np.zeros((NCORES, 128, TT), np.int16)
    for c in range(NCORES):
        flat = np.zeros(TOTSLOT, np.int16)
        sl = src_local[c]
        flat[np.arange(TOTSLOT)] = sl[np.arange(TOTSLOT) % 128,
                                      np.arange(TOTSLOT) // 128]
        w = flat.reshape(TOTSLOT // 16, 16).T  # [16, TT*8]
        for g in range(NGRP):
            for b in range(NBUCKET):
                off_t = int(seg_off[g, b]) // 128
                ntile = int(seg_tiles[g, b])
                wseg = w[:, off_t * 8:(off_t + ntile) * 8]
                idx_pack[c][:, off_t:off_t + ntile] = (
                    wseg.reshape(16, 8, ntile).transpose(1, 0, 2)
                    .reshape(128, ntile))

    # weights
    W1 = np.asarray(inputs["W1"], np.float32)
    W2 = np.asarray(inputs["W2"], np.float32)
    waug1 = np.concatenate(
        [W1, (W1 @ np.asarray(inputs["a_src1"], np.float32))[:, None],
         (W1 @ np.asarray(inputs["a_dst1"], np.float32))[:, None]], axis=1)
    waug2 = np.concatenate(
        [W2, (W2 @ np.asarray(inputs["a_src2"], np.float32))[:, None],
         (W2 @ np.asarray(inputs["a_dst2"], np.float32))[:, None]], axis=1)

    b1 = np.asarray(inputs["b1"], np.float32)
    b2v = np.asarray(inputs["b2"], np.float32)
    lin1_W = np.asarray(inputs["lin1_W"], np.float32)
    lin1_b = np.asarray(inputs["lin1_b"], np.float32)
    lin2_W = np.asarray(inputs["lin2_W"], np.float32)
    lin2_b = np.asarray(inputs["lin2_b"], np.float32)

    invcnt = np.zeros((NCORES, 64, GPC), np.float32)
    for c in range(NCORES):
        invcnt[c] = np.broadcast_to(
            1.0 / np.maximum(cnt[c * GPC:(c + 1) * GPC], 1.0), (64, GPC))

    # ---- blob layout (bf16 columns)
    XH = NSLOT // 2
    IOFF = XH
    DOFF = XH + TT
    SOFF = XH + TT + TT // 2
    S = NB + 277  # f32 cols in the small region
    CB = SOFF + 2 * S

    per_core = []
    for c in range(NCORES):
        blob = np.zeros((128, CB * 2), np.uint8)
        xT_bf = np.ascontiguousarray(x_pi[c].T).astype(bf16)  # [64, NSLOT]
        blob[0:64, 0:XH * 2] = np.ascontiguousarray(
            xT_bf[:, 0:XH]).view(np.uint8)
        blob[64:128, 0:XH * 2] = np.ascontiguousarray(
            xT_bf[:, XH:]).view(np.uint8)
        blob[:, IOFF * 2:(IOFF + TT) * 2] = idx_pack[c].view(np.uint8)
        blob[:, DOFF * 2:DOFF * 2 + TT] = dst_i8[c].view(np.uint8)
        sm = np.zeros((128, S), np.float32)
        sm[:, 0:NB] = mask01[c]
        sm[:, NB:NB + 64] = lin1_W
        sm[0:64, NB + 64:NB + 130] = waug1
        sm[0:64, NB + 130:NB + 196] = waug2
        sm[0:64, NB + 196:NB + 196 + GPC] = invcnt[c]
        SM_B = NB + 196 + GPC
        sm[0, SM_B:SM_B + 64] = b1
        sm[1, SM_B:SM_B + 64] = b2v
        sm[2, SM_B:SM_B + 64] = lin1_b
        sm[3, SM_B:SM_B + 64] = lin2_W[:, 0]
        sm[0, SM_B + 64] = lin2_b[0]
        blob[:, SOFF * 2:SOFF * 2 + S * 4] = sm.view(np.uint8)
        per_core.append({"blob": blob.view(bf16)})

    tpl = dict(SLOT=SLOT, NSLOT=NSLOT, NB=NB, NGRP=NGRP, NSLOT_G=NSLOT_G,
               BUCKET=BUCKET, tpb=tpb, seg_tiles=seg_tiles, seg_off=seg_off,
               TT=TT, XH=XH, IOFF=IOFF, DOFF=DOFF, SOFF=SOFF, S=S, CB=CB)
    return tpl, per_core


# ---------------------------------------------------------------- device bld
def _build(tpl):
    NSLOT = tpl["NSLOT"]
    NB = tpl["NB"]
    NGRP = tpl["NGRP"]
    NSLOT_G = tpl["NSLOT_G"]
    BUCKET = tpl["BUCKET"]
    tpb = tpl["tpb"]
    seg_tiles = tpl["seg_tiles"]
    TT = tpl["TT"]
    XH = tpl["XH"]
    IOFF = tpl["IOFF"]
    DOFF = tpl["DOFF"]
    SOFF = tpl["SOFF"]
    S = tpl["S"]
    CB = tpl["CB"]
    SM_MASK = 0
    SM_L1W = NB
    SM_WG1 = NB + 64
    SM_WG2 = NB + 130
    SM_ICN = NB + 196
    SM_B = NB + 196 + GPC
    SM_B2 = SM_B + 64

    nc = bacc.Bacc("TRN2", target_bir_lowering=False, debug=False,
                   num_devices=NCORES)

    blob = nc.dram_tensor("blob", [128, CB], BF, kind="ExternalInput")
    out_final = nc.dram_tensor("out_final", [128, 1], F32,
                               kind="ExternalOutput")

    with tile.TileContext(nc) as tc:
        with (
            tc.tile_pool(name="const", bufs=1) as cp,
            tc.tile_pool(name="stage", bufs=1) as stp,
            tc.tile_pool(name="dram", bufs=1, space="DRAM") as dr,
        ):
            # ---- constants in SBUF
            iota_i = cp.tile([128, 128], mybir.dt.int32)
            nc.gpsimd.iota(iota_i[:], pattern=[[1, 128]], base=0,
                           channel_multiplier=0)
            iota_bf = cp.tile([128, 128], BF)
            nc.vector.tensor_copy(iota_bf[:], iota_i[:])
            ident = cp.tile([128, 128], F32)
            make_identity(nc, ident[:])
            ones_bf = cp.tile([128, 1], BF)
            nc.gpsimd.memset(ones_bf[:], 1.0)
            onec = cp.tile([128, 1], F32)
            nc.gpsimd.memset(onec[:], 1.0)

            # ---- unpack the blob
            smalls = cp.tile([128, S], F32)
            nc.sync.dma_start(
                smalls[:], blob[:, SOFF:SOFF + 2 * S].bitcast(F32))
            m01 = smalls[:, SM_MASK:SM_MASK + NB]
            mpad = cp.tile([128, NB], F32)
            nc.vector.scalar_tensor_tensor(
                out=mpad[:], in0=m01, scalar=-1.0,
                in1=onec[:].to_broadcast([128, NB]),
                op0=AL.mult, op1=AL.add)
            waug1_sb = cp.tile([64, 66], BF)
            nc.vector.tensor_copy(waug1_sb[:], smalls[0:64, SM_WG1:SM_WG1 + 66])
            waug2_sb = cp.tile([64, 66], BF)
            nc.vector.tensor_copy(waug2_sb[:], smalls[0:64, SM_WG2:SM_WG2 + 66])
            btiles = cp.tile([128, 4, 64], F32)
            nc.sync.dma_start(
                btiles[:],
                blob[0:4, SOFF + 2 * SM_B:SOFF + 2 * SM_B + 128].bitcast(
                    F32).rearrange("(o r) c -> o r c", o=1).to_broadcast(
                    [128, 4, 64]))
            b1t = btiles[:, 0, :]
            b2t = btiles[:, 1, :]
            l1bt = btiles[:, 2, :]
            l2rt = btiles[:, 3, :]
            b2c = cp.tile([128, 1], F32)
            nc.sync.dma_start(
                b2c[:],
                blob[0:1, SOFF + 2 * SM_B2:SOFF + 2 * SM_B2 + 2].bitcast(
                    F32).to_broadcast([128, 1]))
            dst_i8 = stp.tile([128, TT], I8, name="dsti8")
            nc.sync.dma_start(dst_i8[:], blob[:, DOFF:DOFF + TT // 2].bitcast(I8))
            dl_all = cp.tile([128, TT], F32)
            nc.vector.tensor_copy(dl_all[:], dst_i8[:])

            # persistent staging
            h_T = stp.tile([64, NSLOT], BF, tag="h_T", name="h_T")  # layer1 out, pi order
            dstage = [stp.tile([128, NB], F32, tag=f"dstage{l}", name=f"dstage{l}")
                      for l in range(2)]
            pool_mx = [stp.tile([64, GPC], F32, tag=f"pmx{l}", name=f"pmx{l}")
                       for l in range(2)]
            pool_sm = [stp.tile([64, GPC], F32, tag=f"psm{l}", name=f"psm{l}")
                       for l in range(2)]
            for l in range(2):
                nc.vector.memset(pool_mx[l][:], NEG_BIG)
                nc.vector.memset(pool_sm[l][:], 0.0)

            # DRAM scratch
            table = [dr.tile([NSLOT_G, 128], BF, tag=f"tab{l}", name=f"tab{l}")
                     for l in range(2)]
            tab_own = [dr.tile([NSLOT, 128], BF, tag=f"tabown{l}", name=f"tabown{l}")
                       for l in range(2)]
            d2d = [dr.tile([NB, 128], F32, tag=f"d2d{l}", name=f"d2d{l}") for l in range(2)]
            pool_bounce_in = dr.tile([GPC, 128], F32)
            pool_bounce_out = dr.tile([G, 128], F32)

            # ===== table 1 build: own shard of [h|s] rows + d column =====
            with (
                tc.tile_pool(name="p1", bufs=3) as p1,
                tc.tile_pool(name="p1x", bufs=3) as p1x,
                tc.tile_pool(name="p1ps", bufs=2, space="PSUM") as p1ps,
            ):
                QB = NB // 4
                for s in range(4):
                    half = (s * QB) // (NB // 2)
                    within = (s * QB) % (NB // 2)
                    rows0 = 64 * half
                    coff = within * 128
                    xT = p1x.tile([64, QB * 128], BF, tag="xTs")
                    nc.sync.dma_start(
                        xT[:], blob[rows0:rows0 + 64, coff:coff + QB * 128])
                    rows = p1.tile([128, QB, 65], BF, tag="rows")
                    for j in range(QB):
                        blk_id = s * QB + j
                        hps = p1ps.tile([128, 66], F32, space="PSUM",
                                        tag="hps")
                        nc.tensor.matmul(
                            hps[:], lhsT=xT[:, 128 * j:128 * (j + 1)],
                            rhs=waug1_sb[:], start=True, stop=True)
                        nc.scalar.activation(rows[:, j, :], hps[:, 0:65],
                                             ACT.Copy)
                        nc.vector.tensor_copy(
                            dstage[0][:, blk_id:blk_id + 1],
                            hps[:, 65:66])
                    nc.sync.dma_start(
                        tab_own[0].rearrange("(s r) c -> r s c", r=128)[
                            :, s * QB:(s + 1) * QB, 0:65],
                        rows[:, 0:QB, :])
                # d transpose -> DRAM [NB, 128]
                dt_ps = p1ps.tile([NB, 128], F32, space="PSUM", tag="dt")
                nc.tensor.transpose(dt_ps[:], dstage[0][:], ident[:])
                dt_sb = p1.tile([NB, 128], F32, tag="dts")
                nc.vector.tensor_copy(dt_sb[:], dt_ps[:])
                nc.sync.dma_start(d2d[0][:, :], dt_sb[:])

            nc.gpsimd.collective_compute(
                "AllGather", AL.bypass,
                replica_groups=[list(range(NCORES))],
                ins=[tab_own[0].opt()], outs=[table[0].opt()])

            # ================= edge pass (both layers) =================
            def edge_pass(l):
                tab = table[l]
                with (
                    tc.tile_pool(name=f"eg{l}", bufs=2) as eg,
                    tc.tile_pool(name=f"eq{l}", bufs=2) as eqp,
                    tc.tile_pool(name=f"ep{l}", bufs=1, space="PSUM") as eps,
                    tc.tile_pool(name=f"et{l}", bufs=2, space="PSUM") as ept,
                    tc.tile_pool(name=f"ef{l}", bufs=3) as ef,
                ):
                    for g in range(NGRP):
                        # broadcast d rows of the group's 16 blocks
                        dbc = ef.tile([128, 16, 128], F32, tag="dbc")
                        nc.sync.dma_start(
                            dbc[:],
                            d2d[l][16 * g:16 * (g + 1), :].rearrange(
                                "(o r) c -> o r c", o=1).to_broadcast(
                                [128, 16, 128]))
                        dbc_bf = ef.tile([128, 16, 128], BF, tag="dbcb")
                        nc.vector.tensor_copy(dbc_bf[:], dbc[:])

                        psums = [eps.tile([128, 260], F32, space="PSUM",
                                          tag=f"ps{k}", name=f"ps{k}")
                                 for k in range(4)]
                        for ps_ in psums:
                            nc.vector.memset(ps_[:], 0.0)

                        for b in range(NBUCKET):
                            ntile = int(seg_tiles[g, b])
                            Tb = int(tpb[g, b])
                            off = int(tpl["seg_off"][g, b]) // 128  # tile off
                            # gather the whole (g,b) segment; indices come
                            # from the compact blob region, replicated 8x
                            idx_sb = eg.tile([128, ntile * 8], I16, tag="idx")
                            for k in range(8):
                                nc.sync.dma_start(
                                    idx_sb[16 * k:16 * (k + 1), :].rearrange(
                                        "p (a c) -> p a c", a=8),
                                    blob[:, IOFF + off:IOFF + off + ntile]
                                    .bitcast(I16).rearrange(
                                        "(a p) c -> p a c", p=16))
                            ch = eg.tile([128, ntile, 128], BF, tag="ch")
                            nc.gpsimd.dma_gather(
                                out_ap=ch[:],
                                in_ap=tab[b * BUCKET:(b + 1) * BUCKET, :],
                                idxs_ap=idx_sb[:],
                                num_idxs=ntile * 128,
                                num_idxs_reg=ntile * 128,
                                elem_size=128,
                                single_packet=False)
                            dl = dl_all[:, off:off + ntile]
                            # per tile: lhsT = (iota==dstloc)*|d|  (the |d|
                            # row-scale cancels in the softmax division);
                            # accum of (iota==dstloc)*d gives d_edge.
                            lhsT = eqp.tile([128, ntile, 128], BF, tag="eq")
                            dedge = ef.tile([128, ntile], F32, tag="dedge")
                            for tt in range(ntile):
                                k16 = tt // Tb
                                nc.vector.scalar_tensor_tensor(
                                    out=lhsT[:, tt, :], in0=iota_bf[:],
                                    scalar=dl[:, tt:tt + 1],
                                    in1=dbc_bf[:, k16, :],
                                    op0=AL.is_equal, op1=AL.mult,
                                    accum_out=dedge[:, tt:tt + 1])
                            # z = s + d ; w = exp(max(0.2 z, z))
                            z = ef.tile([128, ntile], F32, tag="z")
                            nc.vector.tensor_tensor(
                                out=z[:], in0=ch[:, :, 64], in1=dedge[:],
                                op=AL.add)
                            nc.vector.scalar_tensor_tensor(
                                out=z[:], in0=z[:], scalar=NEG, in1=z[:],
                                op0=AL.mult, op1=AL.max)
                            w = ef.tile([128, ntile], F32, tag="w")
                            nc.scalar.activation(w[:], z[:], ACT.Exp)
                            # ones into s slot for the denominator column
                            nc.vector.tensor_copy(
                                ch[:, :, 64],
                                ones_bf[:].to_broadcast([128, ntile]))
                            # rhs2 = ch[:, :, 0:65] * w (bulk)
                            ch2 = eqp.tile([128, ntile, 65], BF, tag="dmul",
                                           name="ch2")
                            nc.vector.tensor_tensor(
                                out=ch2[:], in0=ch[:, :, 0:65],
                                in1=w[:].to_broadcast([128, ntile, 65]),
                                op=AL.mult)
                            # matmuls
                            for i in range(16):
                                ps = psums[i // 4]
                                csl = slice(65 * (i % 4), 65 * (i % 4) + 65)
                                for t in range(Tb):
                                    tt = i * Tb + t
                                    nc.tensor.matmul(
                                        ps[:, csl],
                                        lhsT=lhsT[:, tt, :],
                                        rhs=ch2[:, tt, :],
                                        start=False,
                                        stop=(b == NBUCKET - 1 and t == Tb - 1))

                        # ---- finalize the group's 16 blocks
                        for i in range(16):
                            blk_id = 16 * g + i
                            ps = psums[i // 4]
                            csl = slice(65 * (i % 4), 65 * (i % 4) + 65)
                            # self loops (dense): rows of own dst block
                            row = ef.tile([128, 65], BF, tag="slrow")
                            nc.sync.dma_start(
                                row[:], tab_own[l][128 * blk_id:
                                                   128 * blk_id + 128, 0:65])
                            zs = ef.tile([128, 1], F32, tag="zs")
                            nc.vector.tensor_tensor(
                                out=zs[:], in0=row[:, 64:65],
                                in1=dstage[l][:, blk_id:blk_id + 1],
                                op=AL.add)
                            nc.vector.scalar_tensor_tensor(
                                out=zs[:], in0=zs[:], scalar=NEG, in1=zs[:],
                                op0=AL.mult, op1=AL.max)
                            ws = ef.tile([128, 1], F32, tag="ws")
                            nc.scalar.activation(ws[:], zs[:], ACT.Exp)
                            nc.vector.tensor_tensor(
                                out=ws[:], in0=ws[:],
                                in1=m01[:, blk_id:blk_id + 1], op=AL.mult)
                            nc.vector.tensor_tensor(
                                out=ws[:], in0=ws[:],
                                in1=dstage[l][:, blk_id:blk_id + 1],
                                op=AL.mult)
                            nc.vector.tensor_copy(
                                row[:, 64:65], ones_bf[:])
                            nc.vector.scalar_tensor_tensor(
                                out=ps[:, csl], in0=row[:, :],
                                scalar=ws[:], in1=ps[:, csl],
                                op0=AL.mult, op1=AL.add)
                            # divide + bias + relu
                            den = ef.tile([128, 1], F32, tag="den")
                            nc.vector.tensor_tensor(
                                out=den[:],
                                in0=ps[:, csl.start + 64:csl.start + 65],
                                in1=mpad[:, blk_id:blk_id + 1], op=AL.add)
                            rec = ef.tile([128, 1], F32, tag="rec")
                            nc.vector.reciprocal(rec[:], den[:])
                            hmid = ef.tile([128, 64], F32, tag="hmid")
                            nc.vector.scalar_tensor_tensor(
                                out=hmid[:],
                                in0=ps[:, csl.start:csl.start + 64],
                                scalar=rec[:], in1=b1t if l == 0 else b2t,
                                op0=AL.mult, op1=AL.add)
                            hout = ef.tile([128, 64], F32, tag="hout")
                            nc.scalar.activation(hout[:], hmid[:], ACT.Relu)
                            # mask pads to exactly 0 (safe for max: h1 >= 0)
                            nc.vector.tensor_scalar_mul(
                                hout[:], hout[:], m01[:, blk_id:blk_id + 1])
                            # transpose (PE) -> [64, 128]
                            hT = ept.tile([64, 128], F32, space="PSUM",
                                          tag="hT")
                            nc.tensor.transpose(hT[:], hout[:], ident[:])
                            if l == 0:
                                nc.vector.tensor_copy(
                                    h_T[:, 128 * blk_id:128 * (blk_id + 1)],
                                    hT[:])
                            # sum pool
                            red = ef.tile([64, GPC], F32, tag="red")
                            nc.vector.tensor_reduce(
                                red[:],
                                hT[:].rearrange("f (g e) -> f g e", g=GPC),
                                axis=mybir.AxisListType.X, op=AL.add)
                            nc.vector.tensor_tensor(
                                out=pool_sm[l][:], in0=pool_sm[l][:],
                                in1=red[:], op=AL.add)
                            # max pool
                            redm = ef.tile([64, GPC], F32, tag="redm")
                            nc.vector.tensor_reduce(
                                redm[:],
                                hT[:].rearrange("f (g e) -> f g e", g=GPC),
                                axis=mybir.AxisListType.X, op=AL.max)
                            nc.vector.tensor_tensor(
                                out=pool_mx[l][:], in0=pool_mx[l][:],
                                in1=redm[:], op=AL.max)

            edge_pass(0)

            # ================= table 2 build =================
            with (
                tc.tile_pool(name="p3", bufs=3) as p3,
                tc.tile_pool(name="p3ps", bufs=2, space="PSUM") as p3ps,
            ):
                for i in range(NB):
                    hps = p3ps.tile([128, 66], F32, space="PSUM", tag="hps2")
                    nc.tensor.matmul(
                        hps[:], lhsT=h_T[:, 128 * i:128 * (i + 1)],
                        rhs=waug2_sb[:], start=True, stop=True)
                    row = p3.tile([128, 65], BF, tag="row2")
                    nc.vector.tensor_copy(row[:], hps[:, 0:65])
                    nc.sync.dma_start(
                        tab_own[1][128 * i:128 * (i + 1), 0:65], row[:])
                    nc.vector.tensor_copy(dstage[1][:, i:i + 1], hps[:, 65:66])
                dt_ps = p3ps.tile([NB, 128], F32, space="PSUM", tag="dt2")
                nc.tensor.transpose(dt_ps[:], dstage[1][:], ident[:])
                dt_sb = p3.tile([NB, 128], F32, tag="dts2")
                nc.vector.tensor_copy(dt_sb[:], dt_ps[:])
                nc.sync.dma_start(d2d[1][:, :], dt_sb[:])

            nc.gpsimd.collective_compute(
                "AllGather", AL.bypass,
                replica_groups=[list(range(NCORES))],
                ins=[tab_own[1].opt()], outs=[table[1].opt()])

            edge_pass(1)

            # ================= pooling combine + MLP =================
            with (
                tc.tile_pool(name="p5", bufs=2) as p5,
                tc.tile_pool(name="p5ps", bufs=1, space="PSUM") as p5ps,
            ):
                icn = smalls[0:64, SM_ICN:SM_ICN + GPC]
                mxh = p5.tile([64, GPC], F32)
                nc.vector.tensor_tensor(out=mxh[:], in0=pool_mx[0][:],
                                        in1=pool_mx[1][:], op=AL.add)
                smh = p5.tile([64, GPC], F32)
                nc.vector.tensor_tensor(out=smh[:], in0=pool_sm[0][:],
                                        in1=pool_sm[1][:], op=AL.add)
                nc.vector.tensor_tensor(out=smh[:], in0=smh[:], in1=icn,
                                        op=AL.mult)
                # transpose to graph-major [GPC, 128] and AllGather
                zloc = p5.tile([GPC, 128], F32)
                mxT = p5ps.tile([GPC, 64], F32, space="PSUM", tag="mxT")
                nc.tensor.transpose(mxT[:], mxh[:], ident[0:64, 0:64])
                nc.vector.tensor_copy(zloc[:, 0:64], mxT[:])
                smT = p5ps.tile([GPC, 64], F32, space="PSUM", tag="smT")
                nc.tensor.transpose(smT[:], smh[:], ident[0:64, 0:64])
                nc.vector.tensor_copy(zloc[:, 64:128], smT[:])
                nc.sync.dma_start(pool_bounce_in[:, :], zloc[:])
                nc.gpsimd.collective_compute(
                    "AllGather", AL.bypass,
                    replica_groups=[list(range(NCORES))],
                    ins=[pool_bounce_in.opt()], outs=[pool_bounce_out.opt()])
                zg = p5.tile([G, 128], F32)
                nc.sync.dma_start(zg[:], pool_bounce_out[:, :])
                zT_ps = p5ps.tile([128, G], F32, space="PSUM", tag="zT")
                nc.tensor.transpose(zT_ps[:], zg[:], ident[:])
                zT = p5.tile([128, G], F32)
                nc.vector.tensor_copy(zT[:], zT_ps[:])
                mlp_ps = p5ps.tile([G, 64], F32, space="PSUM", tag="mlp")
                nc.tensor.matmul(mlp_ps[:], lhsT=zT[:],
                                 rhs=smalls[:, SM_L1W:SM_L1W + 64],
                                 start=True, stop=True)
                z1 = p5.tile([G, 64], F32)
                nc.vector.tensor_tensor(out=z1[:], in0=mlp_ps[:], in1=l1bt,
                                        op=AL.add)
                nc.scalar.activation(z1[:], z1[:], ACT.Relu)
                z2 = p5.tile([G, 64], F32)
                nc.vector.tensor_tensor(out=z2[:], in0=z1[:], in1=l2rt,
                                        op=AL.mult)
                ored = p5.tile([G, 1], F32)
                nc.vector.tensor_reduce(ored[:], z2[:],
                                        axis=mybir.AxisListType.X, op=AL.add)
                nc.vector.tensor_tensor(out=ored[:], in0=ored[:], in1=b2c[:],
                                        op=AL.add)
                nc.sync.dma_start(out_final[:, :], ored[:])

    nc.compile()
    return nc


# ---------------------------------------------------------------- entry
def kernel(**inputs) -> np.ndarray:
    tpl, per_core = _host_prep(inputs)
    nc = _build(tpl)
    in_maps = [{"blob": per_core[c]["blob"]} for c in range(NCORES)]
    res = bass_utils.run_bass_kernel_spmd(
        nc, in_maps, core_ids=list(range(NCORES)))
    out = np.asarray(res.results[0]["out_final"]).reshape(G)
    return out.astype(np.float32)


# revision 11
# speedup vs baseline: 4.2890x; 1.2251x over previous
"""GAT (2-layer, heads=1) + pooling + MLP on 8 Trainium2 NeuronCores.

Strategy:
- Nodes are mapped to per-graph padded slots (SLOT = align128(max graph size)),
  graphs are sharded 16-per-core, and within each core slots are striped so
  that dst-block i holds slots s with s % NB == i.  Every 128-slot block then
  contains exactly 8 slots of each of the core's 16 graphs (partition p ->
  local graph p//8), which makes pooling segment boundaries compile-time.
- Edge aggregation: per-edge rows [h|s] are fetched with dma_gather (int16
  indices -> 4 src buckets); attention weights w = exp(leakyrelu(s_src +
  d_dst)) are computed on-chip (d expanded per edge via a one-hot *
  broadcast-d reduce); the segment softmax + feature sum is one matmul per
  128-edge tile accumulating [sum(w*h) | sum(w)] into a per-block PSUM.
- Self-loop edges are applied densely at finalize (no gather).
- Node tables ([h|s] rows, bf16) are built sharded and AllGathered; pooled
  [G,2H] is assembled with a single small AllGather; the final MLP is
  replicated.
- All host inputs are packed into ONE bf16 blob per core (x in bf16 split
  into two 64-row halves, gather indices stored once instead of 8x, dst
  rows as int8, small tensors bit-packed f32) and unpacked on device --
  the axon tunnel transfer (~46MB/s + ~70ms/array) dominates wall-clock.
"""
import sys

sys.path.insert(0, "/opt/trn_rl_repo")

import numpy as np
import ml_dtypes

try:  # persistent XLA compile cache: repeat calls skip the NEFF re-wrap
    import jax

    jax.config.update("jax_compilation_cache_dir", "/tmp/jaxcache")
    jax.config.update("jax_persistent_cache_min_entry_size_bytes", -1)
    jax.config.update("jax_persistent_cache_min_compile_time_secs", 0)
except Exception:
    pass

import concourse.bacc as bacc
import concourse.bass as bass
import concourse.mybir as mybir
import concourse.tile as tile
from concourse import bass_utils
from concourse.masks import make_identity

bf16 = ml_dtypes.bfloat16
F32 = mybir.dt.float32
BF = mybir.dt.bfloat16
I16 = mybir.dt.int16
I8 = mybir.dt.int8
AL = mybir.AluOpType
ACT = mybir.ActivationFunctionType

NCORES = 8
G = 128
N = 100000
FIN = 64
H = 64
NEG = 0.2
NBUCKET = 4
GPC = G // NCORES  # graphs per core = 16
NEG_BIG = -1.0e30
X_INT8 = True  # ship x as int8 (scale folded into waug1 on host)


# ---------------------------------------------------------------- host prep
def _host_prep(inputs):
    x = np.asarray(inputs["x"], np.float32)
    ei = np.asarray(inputs["edge_index"]).astype(np.int64)
    bid = np.asarray(inputs["batch_ids"]).astype(np.int64)

    cnt = np.bincount(bid, minlength=G).astype(np.int64)
    SLOT = int(np.ceil(max(cnt.max(), 128) / 128) * 128)
    NSLOT = GPC * SLOT
    NB = NSLOT // 128
    assert NB % 16 == 0 and NB % 4 == 0, NB
    NGRP = NB // 16
    NSLOT_G = NCORES * NSLOT
    BUCKET = NSLOT_G // NBUCKET
    assert BUCKET * NBUCKET == NSLOT_G and BUCKET <= 32768

    gstart = np.zeros(G + 1, np.int64)
    gstart[1:] = np.cumsum(cnt)
    rank = np.arange(N, dtype=np.int64) - gstart[bid]
    slot_of = bid * SLOT + rank  # graph-padded slot, 0..NSLOT_G

    def pi(s):
        c, sl = s // NSLOT, s % NSLOT
        return c * NSLOT, (sl % NB) * 128 + sl // NB

    core_base, loc = pi(slot_of)
    pi_of = core_base + loc  # global pi row of each node

    # permuted x per core (pad rows zero)
    if X_INT8:
        s_q = float(np.abs(x).max()) / 127.0
        xq = np.clip(np.rint(x / s_q), -127, 127).astype(np.int8)
        x_pi = np.zeros((NCORES, NSLOT, FIN), np.int8)
        x_pi[pi_of // NSLOT, pi_of % NSLOT] = xq
    else:
        s_q = 1.0
        x_pi = np.zeros((NCORES, NSLOT, FIN), np.float32)
        x_pi[pi_of // NSLOT, pi_of % NSLOT] = x

    # masks / counts per core
    # local slot (block i, partition p) <-> graph-slot p*NB + i
    pp, ii = np.meshgrid(np.arange(128), np.arange(NB), indexing="ij")
    gslot = pp * NB + ii  # [128, NB] graph-padded local slot
    within = gslot % SLOT  # rank within graph
    lg = gslot // SLOT  # local graph 0..15 (== pp//8)
    mask01 = np.zeros((NCORES, 128, NB), np.float32)
    for c in range(NCORES):
        real = within < cnt[c * GPC + lg]
        mask01[c] = real.astype(np.float32)

    # edges (no self loops in the gather path)
    src, dst = ei[0], ei[1]
    ps = pi_of[src]
    pd = pi_of[dst]
    core = pd // NSLOT
    blk = (pd % NSLOT) // 128
    bkt = ps // BUCKET
    grp = blk // 16

    cnts = np.zeros((NCORES, NB, NBUCKET), np.int64)
    np.add.at(cnts, (core, blk, bkt), 1)
    # uniform tiles-per-block within each (group, bucket)
    tpb = np.zeros((NGRP, NBUCKET), np.int64)  # tiles per block
    for g in range(NGRP):
        for b in range(NBUCKET):
            m = cnts[:, 16 * g:16 * g + 16, b].max()
            tpb[g, b] = max(1, int(np.ceil(m / 128)))
    seg_tiles = (16 * tpb).astype(np.int64)  # tiles per (g,b) segment
    TT = int(seg_tiles.sum())  # total tiles per core per layer
    assert TT % 2 == 0, TT
    TOTSLOT = TT * 128

    # slot offsets: order (g, b, block-within-group, slot)
    seg_off = np.zeros((NGRP, NBUCKET), np.int64)
    acc = 0
    for g in range(NGRP):
        for b in range(NBUCKET):
            seg_off[g, b] = acc
            acc += seg_tiles[g, b] * 128

    order = np.lexsort((bkt, blk))  # edges sorted by (blk, bkt); core split below
    src_local = np.zeros((NCORES, 128, TT), np.int16)
    dst_i8 = np.full((NCORES, 128, TT), -1, np.int8)
    for c in range(NCORES):
        sel = order[core[order] == c]
        sblk, sbkt = blk[sel], bkt[sel]
        sps, spd = ps[sel], pd[sel]
        # slot index for each edge: within its (g,b,block) run
        # run start: seg_off[g,b] + (blk%16)*tpb[g,b]*128; position = rank in run
        key = sblk * NBUCKET + sbkt
        # stable order already (blk, bkt); rank within run:
        runstart_mark = np.r_[True, key[1:] != key[:-1]]
        runid = np.cumsum(runstart_mark) - 1
        nruns = int(runid[-1]) + 1 if len(runid) else 0
        first = np.full(nruns, len(sel), np.int64)
        np.minimum.at(first, runid, np.arange(len(sel)))
        pos = np.arange(len(sel)) - first[runid]
        gg = sblk // 16
        slot = (seg_off[gg, sbkt] + (sblk % 16) * tpb[gg, sbkt] * 128 + pos)
        assert (pos < tpb[gg, sbkt] * 128).all()
        p_ = slot % 128
        t_ = slot // 128
        src_local[c, p_, t_] = (sps - sbkt * BUCKET).astype(np.int16)
        dst_i8[c, p_, t_] = (spd % NSLOT % 128).astype(np.int8)

    # wrapped int16 gather indices, stored compactly: per (g,b) segment the
    # [16, ntile*8] wrap is folded into [128, ntile] (row 16a+r holds
    # wseg[r, a*ntile : (a+1)*ntile]); the 8x replication the gather engine
    # wants is re-created on device with 8 DMAs per segment.
    idx_pack = np.zeros((NCORES, 128, TT), np.int16)
    for c in range(NCORES):
        flat = np.zeros(TOTSLOT, np.int16)
        sl = src_local[c]
        flat[np.arange(TOTSLOT)] = sl[np.arange(TOTSLOT) % 128,
                                      np.arange(TOTSLOT) // 128]
        w = flat.reshape(TOTSLOT // 16, 16).T  # [16, TT*8]
        for g in range(NGRP):
            for b in range(NBUCKET):
                off_t = int(seg_off[g, b]) // 128
                ntile = int(seg_tiles[g, b])
                wseg = w[:, off_t * 8:(off_t + ntile) * 8]
                idx_pack[c][:, off_t:off_t + ntile] = (
                    wseg.reshape(16, 8, ntile).transpose(1, 0, 2)
                    .reshape(128, ntile))

    # weights
    W1 = np.asarray(inputs["W1"], np.float32)
    W2 = np.asarray(inputs["W2"], np.float32)
    waug1 = s_q * np.concatenate(
        [W1, (W1 @ np.asarray(inputs["a_src1"], np.float32))[:, None],
         (W1 @ np.asarray(inputs["a_dst1"], np.float32))[:, None]], axis=1)
    waug2 = np.concatenate(
        [W2, (W2 @ np.asarray(inputs["a_src2"], np.float32))[:, None],
         (W2 @ np.asarray(inputs["a_dst2"], np.float32))[:, None]], axis=1)

    b1 = np.asarray(inputs["b1"], np.float32)
    b2v = np.asarray(inputs["b2"], np.float32)
    lin1_W = np.asarray(inputs["lin1_W"], np.float32)
    lin1_b = np.asarray(inputs["lin1_b"], np.float32)
    lin2_W = np.asarray(inputs["lin2_W"], np.float32)
    lin2_b = np.asarray(inputs["lin2_b"], np.float32)

    invcnt = np.zeros((NCORES, 64, GPC), np.float32)
    for c in range(NCORES):
        invcnt[c] = np.broadcast_to(
            1.0 / np.maximum(cnt[c * GPC:(c + 1) * GPC], 1.0), (64, GPC))

    # ---- blob layout (bf16 columns); x is stored as two 64-row halves
    XH = NSLOT // 2  # x columns per half
    XCOLS = (NSLOT // 4) if X_INT8 else (NSLOT // 2)  # bf16 cols of X region
    IOFF = XCOLS
    DOFF = XCOLS + TT
    SOFF = XCOLS + TT + TT // 2
    S = NB + 277  # f32 cols in the small region
    CB = SOFF + 2 * S

    per_core = []
    for c in range(NCORES):
        blob = np.zeros((128, CB * 2), np.uint8)
        if X_INT8:
            xT_q = np.ascontiguousarray(x_pi[c].T)  # [64, NSLOT] int8
            blob[0:64, 0:XH] = np.ascontiguousarray(
                xT_q[:, 0:XH]).view(np.uint8)
            blob[64:128, 0:XH] = np.ascontiguousarray(
                xT_q[:, XH:]).view(np.uint8)
        else:
            xT_bf = np.ascontiguousarray(x_pi[c].T).astype(bf16)
            blob[0:64, 0:XH * 2] = np.ascontiguousarray(
                xT_bf[:, 0:XH]).view(np.uint8)
            blob[64:128, 0:XH * 2] = np.ascontiguousarray(
                xT_bf[:, XH:]).view(np.uint8)
        blob[:, IOFF * 2:(IOFF + TT) * 2] = idx_pack[c].view(np.uint8)
        blob[:, DOFF * 2:DOFF * 2 + TT] = dst_i8[c].view(np.uint8)
        sm = np.zeros((128, S), np.float32)
        sm[:, 0:NB] = mask01[c]
        sm[:, NB:NB + 64] = lin1_W
        sm[0:64, NB + 64:NB + 130] = waug1
        sm[0:64, NB + 130:NB + 196] = waug2
        sm[0:64, NB + 196:NB + 196 + GPC] = invcnt[c]
        SM_B = NB + 196 + GPC
        sm[0, SM_B:SM_B + 64] = b1
        sm[1, SM_B:SM_B + 64] = b2v
        sm[2, SM_B:SM_B + 64] = lin1_b
        sm[3, SM_B:SM_B + 64] = lin2_W[:, 0]
        sm[0, SM_B + 64] = lin2_b[0]
        blob[:, SOFF * 2:SOFF * 2 + S * 4] = sm.view(np.uint8)
        per_core.append({"blob": blob.view(bf16)})

    tpl = dict(SLOT=SLOT, NSLOT=NSLOT, NB=NB, NGRP=NGRP, NSLOT_G=NSLOT_G,
               BUCKET=BUCKET, tpb=tpb, seg_tiles=seg_tiles, seg_off=seg_off,
               TT=TT, XH=XH, IOFF=IOFF, DOFF=DOFF, SOFF=SOFF, S=S, CB=CB)
    return tpl, per_core


# ---------------------------------------------------------------- device bld
def _build(tpl):
    NSLOT = tpl["NSLOT"]
    NB = tpl["NB"]
    NGRP = tpl["NGRP"]
    NSLOT_G = tpl["NSLOT_G"]
    BUCKET = tpl["BUCKET"]
    tpb = tpl["tpb"]
    seg_tiles = tpl["seg_tiles"]
    TT = tpl["TT"]
    XH = tpl["XH"]
    IOFF = tpl["IOFF"]
    DOFF = tpl["DOFF"]
    SOFF = tpl["SOFF"]
    S = tpl["S"]
    CB = tpl["CB"]
    SM_MASK = 0
    SM_L1W = NB
    SM_WG1 = NB + 64
    SM_WG2 = NB + 130
    SM_ICN = NB + 196
    SM_B = NB + 196 + GPC
    SM_B2 = SM_B + 64

    nc = bacc.Bacc("TRN2", target_bir_lowering=False, debug=False,
                   num_devices=NCORES)

    blob = nc.dram_tensor("blob", [128, CB], BF, kind="ExternalInput")
    out_final = nc.dram_tensor("out_final", [128, 1], F32,
                               kind="ExternalOutput")

    with tile.TileContext(nc) as tc:
        with (
            tc.tile_pool(name="const", bufs=1) as cp,
            tc.tile_pool(name="stage", bufs=1) as stp,
            tc.tile_pool(name="dram", bufs=1, space="DRAM") as dr,
        ):
            # ---- constants in SBUF
            iota_i = cp.tile([128, 128], mybir.dt.int32)
            nc.gpsimd.iota(iota_i[:], pattern=[[1, 128]], base=0,
                           channel_multiplier=0)
            iota_bf = cp.tile([128, 128], BF)
            nc.vector.tensor_copy(iota_bf[:], iota_i[:])
            ident = cp.tile([128, 128], F32)
            make_identity(nc, ident[:])
            ones_bf = cp.tile([128, 1], BF)
            nc.gpsimd.memset(ones_bf[:], 1.0)
            onec = cp.tile([128, 1], F32)
            nc.gpsimd.memset(onec[:], 1.0)

            # ---- unpack the blob
            smalls = cp.tile([128, S], F32)
            nc.sync.dma_start(
                smalls[:], blob[:, SOFF:SOFF + 2 * S].bitcast(F32))
            m01 = smalls[:, SM_MASK:SM_MASK + NB]
            mpad = cp.tile([128, NB], F32)
            nc.vector.scalar_tensor_tensor(
                out=mpad[:], in0=m01, scalar=-1.0,
                in1=onec[:].to_broadcast([128, NB]),
                op0=AL.mult, op1=AL.add)
            waug1_sb = cp.tile([64, 66], BF)
            nc.vector.tensor_copy(waug1_sb[:], smalls[0:64, SM_WG1:SM_WG1 + 66])
            waug2_sb = cp.tile([64, 66], BF)
            nc.vector.tensor_copy(waug2_sb[:], smalls[0:64, SM_WG2:SM_WG2 + 66])
            btiles = cp.tile([128, 4, 64], F32)
            nc.sync.dma_start(
                btiles[:],
                blob[0:4, SOFF + 2 * SM_B:SOFF + 2 * SM_B + 128].bitcast(
                    F32).rearrange("(o r) c -> o r c", o=1).to_broadcast(
                    [128, 4, 64]))
            b1t = btiles[:, 0, :]
            b2t = btiles[:, 1, :]
            l1bt = btiles[:, 2, :]
            l2rt = btiles[:, 3, :]
            b2c = cp.tile([128, 1], F32)
            nc.sync.dma_start(
                b2c[:],
                blob[0:1, SOFF + 2 * SM_B2:SOFF + 2 * SM_B2 + 2].bitcast(
                    F32).to_broadcast([128, 1]))
            dst_i8 = stp.tile([128, TT], I8, name="dsti8")
            nc.sync.dma_start(dst_i8[:], blob[:, DOFF:DOFF + TT // 2].bitcast(I8))
            dl_all = cp.tile([128, TT], F32)
            nc.vector.tensor_copy(dl_all[:], dst_i8[:])

            # persistent staging
            h_T = stp.tile([64, NSLOT], BF, tag="h_T", name="h_T")  # layer1 out, pi order
            dstage = [stp.tile([128, NB], F32, tag=f"dstage{l}", name=f"dstage{l}")
                      for l in range(2)]
            pool_mx = [stp.tile([64, GPC], F32, tag=f"pmx{l}", name=f"pmx{l}")
                       for l in range(2)]
            pool_sm = [stp.tile([64, GPC], F32, tag=f"psm{l}", name=f"psm{l}")
                       for l in range(2)]
            for l in range(2):
                nc.vector.memset(pool_mx[l][:], NEG_BIG)
                nc.vector.memset(pool_sm[l][:], 0.0)

            # DRAM scratch
            table = [dr.tile([NSLOT_G, 128], BF, tag=f"tab{l}", name=f"tab{l}")
                     for l in range(2)]
            tab_own = [dr.tile([NSLOT, 128], BF, tag=f"tabown{l}", name=f"tabown{l}")
                       for l in range(2)]
            d2d = [dr.tile([NB, 128], F32, tag=f"d2d{l}", name=f"d2d{l}") for l in range(2)]
            pool_bounce_in = dr.tile([GPC, 128], F32)
            pool_bounce_out = dr.tile([G, 128], F32)

            # ===== table 1 build: own shard of [h|s] rows + d column =====
            with (
                tc.tile_pool(name="p1", bufs=3) as p1,
                tc.tile_pool(name="p1x", bufs=3) as p1x,
                tc.tile_pool(name="p1ps", bufs=2, space="PSUM") as p1ps,
            ):
                QB = NB // 4
                for s in range(4):
                    half = (s * QB) // (NB // 2)
                    within = (s * QB) % (NB // 2)
                    rows0 = 64 * half
                    xT = p1x.tile([64, QB * 128], BF, tag="xTs")
                    if X_INT8:
                        coff = within * 64
                        xq8 = p1x.tile([64, QB * 128], I8, tag="xq8")
                        nc.sync.dma_start(
                            xq8[:],
                            blob[rows0:rows0 + 64,
                                 coff:coff + QB * 64].bitcast(I8))
                        nc.vector.tensor_copy(xT[:], xq8[:])
                    else:
                        coff = within * 128
                        nc.sync.dma_start(
                            xT[:],
                            blob[rows0:rows0 + 64, coff:coff + QB * 128])
                    rows = p1.tile([128, QB, 65], BF, tag="rows")
                    for j in range(QB):
                        blk_id = s * QB + j
                        hps = p1ps.tile([128, 66], F32, space="PSUM",
                                        tag="hps")
                        nc.tensor.matmul(
                            hps[:], lhsT=xT[:, 128 * j:128 * (j + 1)],
                            rhs=waug1_sb[:], start=True, stop=True)
                        nc.scalar.activation(rows[:, j, :], hps[:, 0:65],
                                             ACT.Copy)
                        nc.vector.tensor_copy(
                            dstage[0][:, blk_id:blk_id + 1],
                            hps[:, 65:66])
                    nc.sync.dma_start(
                        tab_own[0].rearrange("(s r) c -> r s c", r=128)[
                            :, s * QB:(s + 1) * QB, 0:65],
                        rows[:, 0:QB, :])
                # d transpose -> DRAM [NB, 128]
                dt_ps = p1ps.tile([NB, 128], F32, space="PSUM", tag="dt")
                nc.tensor.transpose(dt_ps[:], dstage[0][:], ident[:])
                dt_sb = p1.tile([NB, 128], F32, tag="dts")
                nc.vector.tensor_copy(dt_sb[:], dt_ps[:])
                nc.sync.dma_start(d2d[0][:, :], dt_sb[:])

            nc.gpsimd.collective_compute(
                "AllGather", AL.bypass,
                replica_groups=[list(range(NCORES))],
                ins=[tab_own[0].opt()], outs=[table[0].opt()])

            # ================= edge pass (both layers) =================
            def edge_pass(l):
                tab = table[l]
                with (
                    tc.tile_pool(name=f"eg{l}", bufs=2) as eg,
                    tc.tile_pool(name=f"eq{l}", bufs=2) as eqp,
                    tc.tile_pool(name=f"ep{l}", bufs=1, space="PSUM") as eps,
                    tc.tile_pool(name=f"et{l}", bufs=2, space="PSUM") as ept,
                    tc.tile_pool(name=f"ef{l}", bufs=3) as ef,
                ):
                    for g in range(NGRP):
                        # broadcast d rows of the group's 16 blocks
                        dbc = ef.tile([128, 16, 128], F32, tag="dbc")
                        nc.sync.dma_start(
                            dbc[:],
                            d2d[l][16 * g:16 * (g + 1), :].rearrange(
                                "(o r) c -> o r c", o=1).to_broadcast(
                                [128, 16, 128]))
                        dbc_bf = ef.tile([128, 16, 128], BF, tag="dbcb")
                        nc.vector.tensor_copy(dbc_bf[:], dbc[:])

                        psums = [eps.tile([128, 260], F32, space="PSUM",
                                          tag=f"ps{k}", name=f"ps{k}")
                                 for k in range(4)]
                        for ps_ in psums:
                            nc.vector.memset(ps_[:], 0.0)

                        for b in range(NBUCKET):
                            ntile = int(seg_tiles[g, b])
                            Tb = int(tpb[g, b])
                            off = int(tpl["seg_off"][g, b]) // 128  # tile off
                            # gather the whole (g,b) segment; indices come
                            # from the compact blob region, replicated 8x
                            idx_sb = eg.tile([128, ntile * 8], I16, tag="idx")
                            for k in range(8):
                                nc.sync.dma_start(
                                    idx_sb[16 * k:16 * (k + 1), :].rearrange(
                                        "p (a c) -> p a c", a=8),
                                    blob[:, IOFF + off:IOFF + off + ntile]
                                    .bitcast(I16).rearrange(
                                        "(a p) c -> p a c", p=16))
                            ch = eg.tile([128, ntile, 128], BF, tag="ch")
                            nc.gpsimd.dma_gather(
                                out_ap=ch[:],
                                in_ap=tab[b * BUCKET:(b + 1) * BUCKET, :],
                                idxs_ap=idx_sb[:],
                                num_idxs=ntile * 128,
                                num_idxs_reg=ntile * 128,
                                elem_size=128,
                                single_packet=False)
                            dl = dl_all[:, off:off + ntile]
                            # per tile: lhsT = (iota==dstloc)*|d|  (the |d|
                            # row-scale cancels in the softmax division);
                            # accum of (iota==dstloc)*d gives d_edge.
                            lhsT = eqp.tile([128, ntile, 128], BF, tag="eq")
                            dedge = ef.tile([128, ntile], F32, tag="dedge")
                            for tt in range(ntile):
                                k16 = tt // Tb
                                nc.vector.scalar_tensor_tensor(
                                    out=lhsT[:, tt, :], in0=iota_bf[:],
                                    scalar=dl[:, tt:tt + 1],
                                    in1=dbc_bf[:, k16, :],
                                    op0=AL.is_equal, op1=AL.mult,
                                    accum_out=dedge[:, tt:tt + 1])
                            # z = s + d ; w = exp(max(0.2 z, z))
                            z = ef.tile([128, ntile], F32, tag="z")
                            nc.vector.tensor_tensor(
                                out=z[:], in0=ch[:, :, 64], in1=dedge[:],
                                op=AL.add)
                            nc.vector.scalar_tensor_tensor(
                                out=z[:], in0=z[:], scalar=NEG, in1=z[:],
                                op0=AL.mult, op1=AL.max)
                            w = ef.tile([128, ntile], F32, tag="w")
                            nc.scalar.activation(w[:], z[:], ACT.Exp)
                            # ones into s slot for the denominator column
                            nc.vector.tensor_copy(
                                ch[:, :, 64],
                                ones_bf[:].to_broadcast([128, ntile]))
                            # rhs2 = ch[:, :, 0:65] * w (bulk)
                            ch2 = eqp.tile([128, ntile, 65], BF, tag="dmul",
                                           name="ch2")
                            nc.vector.tensor_tensor(
                                out=ch2[:], in0=ch[:, :, 0:65],
                                in1=w[:].to_broadcast([128, ntile, 65]),
                                op=AL.mult)
                            # matmuls
                            for i in range(16):
                                ps = psums[i // 4]
                                csl = slice(65 * (i % 4), 65 * (i % 4) + 65)
                                for t in range(Tb):
                                    tt = i * Tb + t
                                    nc.tensor.matmul(
                                        ps[:, csl],
                                        lhsT=lhsT[:, tt, :],
                                        rhs=ch2[:, tt, :],
                                        start=False,
                                        stop=(b == NBUCKET - 1 and t == Tb - 1))

                        # ---- finalize the group's 16 blocks
                        for i in range(16):
                            blk_id = 16 * g + i
                            ps = psums[i // 4]
                            csl = slice(65 * (i % 4), 65 * (i % 4) + 65)
                            # self loops (dense): rows of own dst block
                            row = ef.tile([128, 65], BF, tag="slrow")
                            nc.sync.dma_start(
                                row[:], tab_own[l][128 * blk_id:
                                                   128 * blk_id + 128, 0:65])
                            zs = ef.tile([128, 1], F32, tag="zs")
                            nc.vector.tensor_tensor(
                                out=zs[:], in0=row[:, 64:65],
                                in1=dstage[l][:, blk_id:blk_id + 1],
                                op=AL.add)
                            nc.vector.scalar_tensor_tensor(
                                out=zs[:], in0=zs[:], scalar=NEG, in1=zs[:],
                                op0=AL.mult, op1=AL.max)
                            ws = ef.tile([128, 1], F32, tag="ws")
                            nc.scalar.activation(ws[:], zs[:], ACT.Exp)
                            nc.vector.tensor_tensor(
                                out=ws[:], in0=ws[:],
                                in1=m01[:, blk_id:blk_id + 1], op=AL.mult)
                            nc.vector.tensor_tensor(
                                out=ws[:], in0=ws[:],
                                in1=dstage[l][:, blk_id:blk_id + 1],
                                op=AL.mult)
                            nc.vector.tensor_copy(
                                row[:, 64:65], ones_bf[:])
                            nc.vector.scalar_tensor_tensor(
                                out=ps[:, csl], in0=row[:, :],
                                scalar=ws[:], in1=ps[:, csl],
                                op0=AL.mult, op1=AL.add)
                            # divide + bias + relu
                            den = ef.tile([128, 1], F32, tag="den")
                            nc.vector.tensor_tensor(
                                out=den[:],
                                in0=ps[:, csl.start + 64:csl.start + 65],
                                in1=mpad[:, blk_id:blk_id + 1], op=AL.add)
                            rec = ef.tile([128, 1], F32, tag="rec")
                            nc.vector.reciprocal(rec[:], den[:])
                            hmid = ef.tile([128, 64], F32, tag="hmid")
                            nc.vector.scalar_tensor_tensor(
                                out=hmid[:],
                                in0=ps[:, csl.start:csl.start + 64],
                                scalar=rec[:], in1=b1t if l == 0 else b2t,
                                op0=AL.mult, op1=AL.add)
                            hout = ef.tile([128, 64], F32, tag="hout")
                            nc.scalar.activation(hout[:], hmid[:], ACT.Relu)
                            # mask pads to exactly 0 (safe for max: h1 >= 0)
                            nc.vector.tensor_scalar_mul(
                                hout[:], hout[:], m01[:, blk_id:blk_id + 1])
                            # transpose (PE) -> [64, 128]
                            hT = ept.tile([64, 128], F32, space="PSUM",
                                          tag="hT")
                            nc.tensor.transpose(hT[:], hout[:], ident[:])
                            if l == 0:
                                nc.vector.tensor_copy(
                                    h_T[:, 128 * blk_id:128 * (blk_id + 1)],
                                    hT[:])
                            # sum pool
                            red = ef.tile([64, GPC], F32, tag="red")
                            nc.vector.tensor_reduce(
                                red[:],
                                hT[:].rearrange("f (g e) -> f g e", g=GPC),
                                axis=mybir.AxisListType.X, op=AL.add)
                            nc.vector.tensor_tensor(
                                out=pool_sm[l][:], in0=pool_sm[l][:],
                                in1=red[:], op=AL.add)
                            # max pool
                            redm = ef.tile([64, GPC], F32, tag="redm")
                            nc.vector.tensor_reduce(
                                redm[:],
                                hT[:].rearrange("f (g e) -> f g e", g=GPC),
                                axis=mybir.AxisListType.X, op=AL.max)
                            nc.vector.tensor_tensor(
                                out=pool_mx[l][:], in0=pool_mx[l][:],
                                in1=redm[:], op=AL.max)

            edge_pass(0)

            # ================= table 2 build =================
            with (
                tc.tile_pool(name="p3", bufs=3) as p3,
                tc.tile_pool(name="p3ps", bufs=2, space="PSUM") as p3ps,
            ):
                for i in range(NB):
                    hps = p3ps.tile([128, 66], F32, space="PSUM", tag="hps2")
                    nc.tensor.matmul(
                        hps[:], lhsT=h_T[:, 128 * i:128 * (i + 1)],
                        rhs=waug2_sb[:], start=True, stop=True)
                    row = p3.tile([128, 65], BF, tag="row2")
                    nc.vector.tensor_copy(row[:], hps[:, 0:65])
                    nc.sync.dma_start(
                        tab_own[1][128 * i:128 * (i + 1), 0:65], row[:])
                    nc.vector.tensor_copy(dstage[1][:, i:i + 1], hps[:, 65:66])
                dt_ps = p3ps.tile([NB, 128], F32, space="PSUM", tag="dt2")
                nc.tensor.transpose(dt_ps[:], dstage[1][:], ident[:])
                dt_sb = p3.tile([NB, 128], F32, tag="dts2")
                nc.vector.tensor_copy(dt_sb[:], dt_ps[:])
                nc.sync.dma_start(d2d[1][:, :], dt_sb[:])

            nc.gpsimd.collective_compute(
                "AllGather", AL.bypass,
                replica_groups=[list(range(NCORES))],
                ins=[tab_own[1].opt()], outs=[table[1].opt()])

            edge_pass(1)

            # ================= pooling combine + MLP =================
            with (
                tc.tile_pool(name="p5", bufs=2) as p5,
                tc.tile_pool(name="p5ps", bufs=1, space="PSUM") as p5ps,
            ):
                icn = smalls[0:64, SM_ICN:SM_ICN + GPC]
                mxh = p5.tile([64, GPC], F32)
                nc.vector.tensor_tensor(out=mxh[:], in0=pool_mx[0][:],
                                        in1=pool_mx[1][:], op=AL.add)
                smh = p5.tile([64, GPC], F32)
                nc.vector.tensor_tensor(out=smh[:], in0=pool_sm[0][:],
                                        in1=pool_sm[1][:], op=AL.add)
                nc.vector.tensor_tensor(out=smh[:], in0=smh[:], in1=icn,
                                        op=AL.mult)
                # transpose to graph-major [GPC, 128] and AllGather
                zloc = p5.tile([GPC, 128], F32)
                mxT = p5ps.tile([GPC, 64], F32, space="PSUM", tag="mxT")
                nc.tensor.transpose(mxT[:], mxh[:], ident[0:64, 0:64])
                nc.vector.tensor_copy(zloc[:, 0:64], mxT[:])
                smT = p5ps.tile([GPC, 64], F32, space="PSUM", tag="smT")
                nc.tensor.transpose(smT[:], smh[:], ident[0:64, 0:64])
                nc.vector.tensor_copy(zloc[:, 64:128], smT[:])
                nc.sync.dma_start(pool_bounce_in[:, :], zloc[:])
                nc.gpsimd.collective_compute(
                    "AllGather", AL.bypass,
                    replica_groups=[list(range(NCORES))],
                    ins=[pool_bounce_in.opt()], outs=[pool_bounce_out.opt()])
                zg = p5.tile([G, 128], F32)
                nc.sync.dma_start(zg[:], pool_bounce_out[:, :])
                zT_ps = p5ps.tile([128, G], F32, space="PSUM", tag="zT")
                nc.tensor.transpose(zT_ps[:], zg[:], ident[:])
                zT = p5.tile([128, G], F32)
                nc.vector.tensor_copy(zT[:], zT_ps[:])
                mlp_ps = p5ps.tile([G, 64], F32, space="PSUM", tag="mlp")
                nc.tensor.matmul(mlp_ps[:], lhsT=zT[:],
                                 rhs=smalls[:, SM_L1W:SM_L1W + 64],
                                 start=True, stop=True)
                z1 = p5.tile([G, 64], F32)
                nc.vector.tensor_tensor(out=z1[:], in0=mlp_ps[:], in1=l1bt,
                                        op=AL.add)
                nc.scalar.activation(z1[:], z1[:], ACT.Relu)
                z2 = p5.tile([G, 64], F32)
                nc.vector.tensor_tensor(out=z2[:], in0=z1[:], in1=l2rt,
                                        op=AL.mult)
                ored = p5.tile([G, 1], F32)
                nc.vector.tensor_reduce(ored[:], z2[:],
                                        axis=mybir.AxisListType.X, op=AL.add)
                nc.vector.tensor_tensor(out=ored[:], in0=ored[:], in1=b2c[:],
                                        op=AL.add)
                nc.sync.dma_start(out_final[:, :], ored[:])

    nc.compile()
    return nc


# ---------------------------------------------------------------- entry
def kernel(**inputs) -> np.ndarray:
    tpl, per_core = _host_prep(inputs)
    nc = _build(tpl)
    in_maps = [{"blob": per_core[c]["blob"]} for c in range(NCORES)]
    res = bass_utils.run_bass_kernel_spmd(
        nc, in_maps, core_ids=list(range(NCORES)))
    out = np.asarray(res.results[0]["out_final"]).reshape(G)
    return out.astype(np.float32)


# revision 12
# speedup vs baseline: 6.9321x; 1.6163x over previous
"""GAT (2-layer, heads=1) + pooling + MLP on 8 Trainium2 NeuronCores.

Strategy:
- Nodes are mapped to per-graph padded slots (SLOT = align128(max graph size)),
  graphs are sharded 16-per-core, and within each core slots are striped so
  that dst-block i holds slots s with s % NB == i.  Every 128-slot block then
  contains exactly 8 slots of each of the core's 16 graphs (partition p ->
  local graph p//8), which makes pooling segment boundaries compile-time.
- Edge aggregation: per-edge rows [h|s] are fetched with dma_gather (int16
  indices -> 4 src buckets); attention weights w = exp(leakyrelu(s_src +
  d_dst)) are computed on-chip (d expanded per edge via a one-hot *
  broadcast-d reduce); the segment softmax + feature sum is one matmul per
  128-edge tile accumulating [sum(w*h) | sum(w)] into a per-block PSUM.
- Self-loop edges are applied densely at finalize (no gather).
- Node tables ([h|s] rows, bf16) are built sharded and AllGathered; pooled
  [G,2H] is assembled with a single small AllGather; the final MLP is
  replicated.
- All host inputs are packed into ONE bf16 blob per core (x in bf16 split
  into two 64-row halves, gather indices stored once instead of 8x, dst
  rows as int8, small tensors bit-packed f32) and unpacked on device --
  the axon tunnel transfer (~46MB/s + ~70ms/array) dominates wall-clock.
"""
import sys

sys.path.insert(0, "/opt/trn_rl_repo")

import numpy as np
import ml_dtypes

try:  # persistent XLA compile cache: repeat calls skip the NEFF re-wrap
    import jax

    jax.config.update("jax_compilation_cache_dir", "/tmp/jaxcache")
    jax.config.update("jax_persistent_cache_min_entry_size_bytes", -1)
    jax.config.update("jax_persistent_cache_min_compile_time_secs", 0)
except Exception:
    pass

import concourse.bacc as bacc
import concourse.bass as bass
import concourse.mybir as mybir
import concourse.tile as tile
from concourse import bass_utils
from concourse.masks import make_identity

bf16 = ml_dtypes.bfloat16
F32 = mybir.dt.float32
BF = mybir.dt.bfloat16
I16 = mybir.dt.int16
I8 = mybir.dt.int8
AL = mybir.AluOpType
ACT = mybir.ActivationFunctionType

NCORES = 8
G = 128
N = 100000
FIN = 64
H = 64
NEG = 0.2
NBUCKET = 4
GPC = G // NCORES  # graphs per core = 16
NEG_BIG = -1.0e30
X_INT8 = True  # ship x as int8 (scale folded into waug1 on host)


# ---------------------------------------------------------------- host prep
def _host_prep(inputs):
    x = np.asarray(inputs["x"], np.float32)
    ei = np.asarray(inputs["edge_index"]).astype(np.int64)
    bid = np.asarray(inputs["batch_ids"]).astype(np.int64)

    cnt = np.bincount(bid, minlength=G).astype(np.int64)
    SLOT = int(np.ceil(max(cnt.max(), 128) / 128) * 128)
    NSLOT = GPC * SLOT
    NB = NSLOT // 128
    assert NB % 16 == 0 and NB % 4 == 0, NB
    NGRP = NB // 16
    NSLOT_G = NCORES * NSLOT
    BUCKET = NSLOT_G // NBUCKET
    assert BUCKET * NBUCKET == NSLOT_G and BUCKET <= 32768

    gstart = np.zeros(G + 1, np.int64)
    gstart[1:] = np.cumsum(cnt)
    rank = np.arange(N, dtype=np.int64) - gstart[bid]
    slot_of = bid * SLOT + rank  # graph-padded slot, 0..NSLOT_G

    def pi(s):
        c, sl = s // NSLOT, s % NSLOT
        return c * NSLOT, (sl % NB) * 128 + sl // NB

    core_base, loc = pi(slot_of)
    pi_of = core_base + loc  # global pi row of each node

    # permuted x per core (pad rows zero)
    if X_INT8:
        s_q = float(np.abs(x).max()) / 127.0
        xq = np.clip(np.rint(x / s_q), -127, 127).astype(np.int8)
        x_pi = np.zeros((NCORES, NSLOT, FIN), np.int8)
        x_pi[pi_of // NSLOT, pi_of % NSLOT] = xq
    else:
        s_q = 1.0
        x_pi = np.zeros((NCORES, NSLOT, FIN), np.float32)
        x_pi[pi_of // NSLOT, pi_of % NSLOT] = x

    # masks / counts per core
    # local slot (block i, partition p) <-> graph-slot p*NB + i
    pp, ii = np.meshgrid(np.arange(128), np.arange(NB), indexing="ij")
    gslot = pp * NB + ii  # [128, NB] graph-padded local slot
    within = gslot % SLOT  # rank within graph
    lg = gslot // SLOT  # local graph 0..15 (== pp//8)
    mask01 = np.zeros((NCORES, 128, NB), np.float32)
    for c in range(NCORES):
        real = within < cnt[c * GPC + lg]
        mask01[c] = real.astype(np.float32)

    # edges (no self loops in the gather path)
    src, dst = ei[0], ei[1]
    ps = pi_of[src]
    pd = pi_of[dst]
    core = pd // NSLOT
    blk = (pd % NSLOT) // 128
    bkt = ps // BUCKET
    grp = blk // 16

    cnts = np.zeros((NCORES, NB, NBUCKET), np.int64)
    np.add.at(cnts, (core, blk, bkt), 1)
    # uniform tiles-per-block within each (group, bucket)
    tpb = np.zeros((NGRP, NBUCKET), np.int64)  # tiles per block
    for g in range(NGRP):
        for b in range(NBUCKET):
            m = cnts[:, 16 * g:16 * g + 16, b].max()
            tpb[g, b] = max(1, int(np.ceil(m / 128)))
    seg_tiles = (16 * tpb).astype(np.int64)  # tiles per (g,b) segment
    TT = int(seg_tiles.sum())  # total tiles per core per layer
    assert TT % 2 == 0, TT
    TOTSLOT = TT * 128

    # slot offsets: order (g, b, block-within-group, slot)
    seg_off = np.zeros((NGRP, NBUCKET), np.int64)
    acc = 0
    for g in range(NGRP):
        for b in range(NBUCKET):
            seg_off[g, b] = acc
            acc += seg_tiles[g, b] * 128

    order = np.lexsort((bkt, blk))  # edges sorted by (blk, bkt); core split below
    src_local = np.zeros((NCORES, 128, TT), np.int16)
    dst_i8 = np.full((NCORES, 128, TT), -1, np.int8)
    for c in range(NCORES):
        sel = order[core[order] == c]
        sblk, sbkt = blk[sel], bkt[sel]
        sps, spd = ps[sel], pd[sel]
        # slot index for each edge: within its (g,b,block) run
        # run start: seg_off[g,b] + (blk%16)*tpb[g,b]*128; position = rank in run
        key = sblk * NBUCKET + sbkt
        # stable order already (blk, bkt); rank within run:
        runstart_mark = np.r_[True, key[1:] != key[:-1]]
        runid = np.cumsum(runstart_mark) - 1
        nruns = int(runid[-1]) + 1 if len(runid) else 0
        first = np.full(nruns, len(sel), np.int64)
        np.minimum.at(first, runid, np.arange(len(sel)))
        pos = np.arange(len(sel)) - first[runid]
        gg = sblk // 16
        slot = (seg_off[gg, sbkt] + (sblk % 16) * tpb[gg, sbkt] * 128 + pos)
        assert (pos < tpb[gg, sbkt] * 128).all()
        p_ = slot % 128
        t_ = slot // 128
        src_local[c, p_, t_] = (sps - sbkt * BUCKET).astype(np.int16)
        dst_i8[c, p_, t_] = (spd % NSLOT % 128).astype(np.int8)

    # wrapped int16 gather indices, stored compactly: per (g,b) segment the
    # [16, ntile*8] wrap is folded into [128, ntile] (row 16a+r holds
    # wseg[r, a*ntile : (a+1)*ntile]); the 8x replication the gather engine
    # wants is re-created on device with 8 DMAs per segment.
    idx_pack = np.zeros((NCORES, 128, TT), np.int16)
    for c in range(NCORES):
        flat = np.zeros(TOTSLOT, np.int16)
        sl = src_local[c]
        flat[np.arange(TOTSLOT)] = sl[np.arange(TOTSLOT) % 128,
                                      np.arange(TOTSLOT) // 128]
        w = flat.reshape(TOTSLOT // 16, 16).T  # [16, TT*8]
        for g in range(NGRP):
            for b in range(NBUCKET):
                off_t = int(seg_off[g, b]) // 128
                ntile = int(seg_tiles[g, b])
                wseg = w[:, off_t * 8:(off_t + ntile) * 8]
                idx_pack[c][:, off_t:off_t + ntile] = (
                    wseg.reshape(16, 8, ntile).transpose(1, 0, 2)
                    .reshape(128, ntile))

    # weights
    W1 = np.asarray(inputs["W1"], np.float32)
    W2 = np.asarray(inputs["W2"], np.float32)
    waug1 = s_q * np.concatenate(
        [W1, (W1 @ np.asarray(inputs["a_src1"], np.float32))[:, None],
         (W1 @ np.asarray(inputs["a_dst1"], np.float32))[:, None]], axis=1)
    waug2 = np.concatenate(
        [W2, (W2 @ np.asarray(inputs["a_src2"], np.float32))[:, None],
         (W2 @ np.asarray(inputs["a_dst2"], np.float32))[:, None]], axis=1)

    b1 = np.asarray(inputs["b1"], np.float32)
    b2v = np.asarray(inputs["b2"], np.float32)
    lin1_W = np.asarray(inputs["lin1_W"], np.float32)
    lin1_b = np.asarray(inputs["lin1_b"], np.float32)
    lin2_W = np.asarray(inputs["lin2_W"], np.float32)
    lin2_b = np.asarray(inputs["lin2_b"], np.float32)

    invcnt = np.zeros((NCORES, 64, GPC), np.float32)
    for c in range(NCORES):
        invcnt[c] = np.broadcast_to(
            1.0 / np.maximum(cnt[c * GPC:(c + 1) * GPC], 1.0), (64, GPC))

    # ---- blob layout (bf16 columns); x is stored as two 64-row halves
    XH = NSLOT // 2  # x columns per half
    XCOLS = (NSLOT // 4) if X_INT8 else (NSLOT // 2)  # bf16 cols of X region
    IOFF = XCOLS
    DOFF = XCOLS + TT
    SOFF = XCOLS + TT + TT // 2
    S = NB + 277  # f32 cols in the small region
    CB = SOFF + 2 * S

    per_core = []
    for c in range(NCORES):
        blob = np.zeros((128, CB * 2), np.uint8)
        if X_INT8:
            xT_q = np.ascontiguousarray(x_pi[c].T)  # [64, NSLOT] int8
            blob[0:64, 0:XH] = np.ascontiguousarray(
                xT_q[:, 0:XH]).view(np.uint8)
            blob[64:128, 0:XH] = np.ascontiguousarray(
                xT_q[:, XH:]).view(np.uint8)
        else:
            xT_bf = np.ascontiguousarray(x_pi[c].T).astype(bf16)
            blob[0:64, 0:XH * 2] = np.ascontiguousarray(
                xT_bf[:, 0:XH]).view(np.uint8)
            blob[64:128, 0:XH * 2] = np.ascontiguousarray(
                xT_bf[:, XH:]).view(np.uint8)
        blob[:, IOFF * 2:(IOFF + TT) * 2] = idx_pack[c].view(np.uint8)
        blob[:, DOFF * 2:DOFF * 2 + TT] = dst_i8[c].view(np.uint8)
        sm = np.zeros((128, S), np.float32)
        sm[:, 0:NB] = mask01[c]
        sm[:, NB:NB + 64] = lin1_W
        sm[0:64, NB + 64:NB + 130] = waug1
        sm[0:64, NB + 130:NB + 196] = waug2
        sm[0:64, NB + 196:NB + 196 + GPC] = invcnt[c]
        SM_B = NB + 196 + GPC
        sm[0, SM_B:SM_B + 64] = b1
        sm[1, SM_B:SM_B + 64] = b2v
        sm[2, SM_B:SM_B + 64] = lin1_b
        sm[3, SM_B:SM_B + 64] = lin2_W[:, 0]
        sm[0, SM_B + 64] = lin2_b[0]
        blob[:, SOFF * 2:SOFF * 2 + S * 4] = sm.view(np.uint8)
        per_core.append({"blob": blob.view(bf16)})

    tpl = dict(SLOT=SLOT, NSLOT=NSLOT, NB=NB, NGRP=NGRP, NSLOT_G=NSLOT_G,
               BUCKET=BUCKET, tpb=tpb, seg_tiles=seg_tiles, seg_off=seg_off,
               TT=TT, XH=XH, IOFF=IOFF, DOFF=DOFF, SOFF=SOFF, S=S, CB=CB)
    return tpl, per_core


# ---------------------------------------------------------------- device bld
def _build(tpl):
    NSLOT = tpl["NSLOT"]
    NB = tpl["NB"]
    NGRP = tpl["NGRP"]
    NSLOT_G = tpl["NSLOT_G"]
    BUCKET = tpl["BUCKET"]
    tpb = tpl["tpb"]
    seg_tiles = tpl["seg_tiles"]
    TT = tpl["TT"]
    XH = tpl["XH"]
    IOFF = tpl["IOFF"]
    DOFF = tpl["DOFF"]
    SOFF = tpl["SOFF"]
    S = tpl["S"]
    CB = tpl["CB"]
    SM_MASK = 0
    SM_L1W = NB
    SM_WG1 = NB + 64
    SM_WG2 = NB + 130
    SM_ICN = NB + 196
    SM_B = NB + 196 + GPC
    SM_B2 = SM_B + 64

    nc = bacc.Bacc("TRN2", target_bir_lowering=False, debug=False,
                   num_devices=NCORES)

    blob = nc.dram_tensor("blob", [128, CB], BF, kind="ExternalInput")
    out_final = nc.dram_tensor("out_final", [128, 1], F32,
                               kind="ExternalOutput")

    with tile.TileContext(nc) as tc:
        with (
            tc.tile_pool(name="const", bufs=1) as cp,
            tc.tile_pool(name="stage", bufs=1) as stp,
            tc.tile_pool(name="dram", bufs=1, space="DRAM") as dr,
        ):
            # ---- constants in SBUF
            iota_i = cp.tile([128, 128], mybir.dt.int32)
            nc.gpsimd.iota(iota_i[:], pattern=[[1, 128]], base=0,
                           channel_multiplier=0)
            iota_bf = cp.tile([128, 128], BF)
            nc.vector.tensor_copy(iota_bf[:], iota_i[:])
            ident = cp.tile([128, 128], F32)
            make_identity(nc, ident[:])
            ones_bf = cp.tile([128, 1], BF)
            nc.gpsimd.memset(ones_bf[:], 1.0)
            onec = cp.tile([128, 1], F32)
            nc.gpsimd.memset(onec[:], 1.0)

            # ---- unpack the blob
            smalls = cp.tile([128, S], F32)
            nc.sync.dma_start(
                smalls[:], blob[:, SOFF:SOFF + 2 * S].bitcast(F32))
            m01 = smalls[:, SM_MASK:SM_MASK + NB]
            mpad = cp.tile([128, NB], F32)
            nc.vector.scalar_tensor_tensor(
                out=mpad[:], in0=m01, scalar=-1.0,
                in1=onec[:].to_broadcast([128, NB]),
                op0=AL.mult, op1=AL.add)
            waug1_sb = cp.tile([64, 66], BF)
            nc.vector.tensor_copy(waug1_sb[:], smalls[0:64, SM_WG1:SM_WG1 + 66])
            waug2_sb = cp.tile([64, 66], BF)
            nc.vector.tensor_copy(waug2_sb[:], smalls[0:64, SM_WG2:SM_WG2 + 66])
            btiles = cp.tile([128, 4, 64], F32)
            nc.sync.dma_start(
                btiles[:],
                blob[0:4, SOFF + 2 * SM_B:SOFF + 2 * SM_B + 128].bitcast(
                    F32).rearrange("(o r) c -> o r c", o=1).to_broadcast(
                    [128, 4, 64]))
            b1t = btiles[:, 0, :]
            b2t = btiles[:, 1, :]
            l1bt = btiles[:, 2, :]
            l2rt = btiles[:, 3, :]
            b2c = cp.tile([128, 1], F32)
            nc.sync.dma_start(
                b2c[:],
                blob[0:1, SOFF + 2 * SM_B2:SOFF + 2 * SM_B2 + 2].bitcast(
                    F32).to_broadcast([128, 1]))
            dst_i8 = stp.tile([128, TT], I8, name="dsti8")
            nc.sync.dma_start(dst_i8[:], blob[:, DOFF:DOFF + TT // 2].bitcast(I8))
            dl_all = cp.tile([128, TT], F32)
            nc.vector.tensor_copy(dl_all[:], dst_i8[:])

            # persistent staging
            h_T = stp.tile([64, NSLOT], BF, tag="h_T", name="h_T")  # layer1 out, pi order
            dstage = [stp.tile([128, NB], F32, tag=f"dstage{l}", name=f"dstage{l}")
                      for l in range(2)]
            pool_mx = [stp.tile([64, GPC], F32, tag=f"pmx{l}", name=f"pmx{l}")
                       for l in range(2)]
            pool_sm = [stp.tile([64, GPC], F32, tag=f"psm{l}", name=f"psm{l}")
                       for l in range(2)]
            for l in range(2):
                nc.vector.memset(pool_mx[l][:], NEG_BIG)
                nc.vector.memset(pool_sm[l][:], 0.0)

            # DRAM scratch
            table = [dr.tile([NSLOT_G, 128], BF, tag=f"tab{l}", name=f"tab{l}")
                     for l in range(2)]
            tab_own = [dr.tile([NSLOT, 128], BF, tag=f"tabown{l}", name=f"tabown{l}")
                       for l in range(2)]
            d2d = [dr.tile([NB, 128], F32, tag=f"d2d{l}", name=f"d2d{l}") for l in range(2)]
            pool_bounce_in = dr.tile([GPC, 128], F32)
            pool_bounce_out = dr.tile([G, 128], F32)

            # ===== table 1 build: own shard of [h|s] rows + d column =====
            with (
                tc.tile_pool(name="p1", bufs=3) as p1,
                tc.tile_pool(name="p1x", bufs=3) as p1x,
                tc.tile_pool(name="p1ps", bufs=2, space="PSUM") as p1ps,
            ):
                QB = NB // 4
                for s in range(4):
                    half = (s * QB) // (NB // 2)
                    within = (s * QB) % (NB // 2)
                    rows0 = 64 * half
                    xT = p1x.tile([64, QB * 128], BF, tag="xTs")
                    if X_INT8:
                        coff = within * 64
                        xq8 = p1x.tile([64, QB * 128], I8, tag="xq8")
                        nc.sync.dma_start(
                            xq8[:],
                            blob[rows0:rows0 + 64,
                                 coff:coff + QB * 64].bitcast(I8))
                        nc.vector.tensor_copy(xT[:], xq8[:])
                    else:
                        coff = within * 128
                        nc.sync.dma_start(
                            xT[:],
                            blob[rows0:rows0 + 64, coff:coff + QB * 128])
                    rows = p1.tile([128, QB, 65], BF, tag="rows")
                    for j in range(QB):
                        blk_id = s * QB + j
                        hps = p1ps.tile([128, 66], F32, space="PSUM",
                                        tag="hps")
                        nc.tensor.matmul(
                            hps[:], lhsT=xT[:, 128 * j:128 * (j + 1)],
                            rhs=waug1_sb[:], start=True, stop=True)
                        nc.scalar.activation(rows[:, j, :], hps[:, 0:65],
                                             ACT.Copy)
                        nc.vector.tensor_copy(
                            dstage[0][:, blk_id:blk_id + 1],
                            hps[:, 65:66])
                    nc.sync.dma_start(
                        tab_own[0].rearrange("(s r) c -> r s c", r=128)[
                            :, s * QB:(s + 1) * QB, 0:65],
                        rows[:, 0:QB, :])
                # d transpose -> DRAM [NB, 128]
                dt_ps = p1ps.tile([NB, 128], F32, space="PSUM", tag="dt")
                nc.tensor.transpose(dt_ps[:], dstage[0][:], ident[:])
                dt_sb = p1.tile([NB, 128], F32, tag="dts")
                nc.vector.tensor_copy(dt_sb[:], dt_ps[:])
                nc.sync.dma_start(d2d[0][:, :], dt_sb[:])

            nc.gpsimd.collective_compute(
                "AllGather", AL.bypass,
                replica_groups=[list(range(NCORES))],
                ins=[tab_own[0].opt()], outs=[table[0].opt()])

            # ================= edge pass (both layers) =================
            def edge_pass(l):
                tab = table[l]
                with (
                    tc.tile_pool(name=f"eg{l}", bufs=2) as eg,
                    tc.tile_pool(name=f"eq{l}", bufs=2) as eqp,
                    tc.tile_pool(name=f"ep{l}", bufs=1, space="PSUM") as eps,
                    tc.tile_pool(name=f"et{l}", bufs=2, space="PSUM") as ept,
                    tc.tile_pool(name=f"ef{l}", bufs=3) as ef,
                ):
                    for g in range(NGRP):
                        # broadcast d rows of the group's 16 blocks
                        dbc = ef.tile([128, 16, 128], F32, tag="dbc")
                        nc.sync.dma_start(
                            dbc[:],
                            d2d[l][16 * g:16 * (g + 1), :].rearrange(
                                "(o r) c -> o r c", o=1).to_broadcast(
                                [128, 16, 128]))
                        dbc_bf = ef.tile([128, 16, 128], BF, tag="dbcb")
                        nc.vector.tensor_copy(dbc_bf[:], dbc[:])

                        psums = [eps.tile([128, 260], F32, space="PSUM",
                                          tag=f"ps{k}", name=f"ps{k}")
                                 for k in range(4)]
                        for ps_ in psums:
                            nc.vector.memset(ps_[:], 0.0)

                        for b in range(NBUCKET):
                            ntile = int(seg_tiles[g, b])
                            Tb = int(tpb[g, b])
                            off = int(tpl["seg_off"][g, b]) // 128  # tile off
                            # gather the whole (g,b) segment; indices come
                            # from the compact blob region, replicated 8x
                            idx_sb = eg.tile([128, ntile * 8], I16, tag="idx")
                            for k in range(8):
                                nc.sync.dma_start(
                                    idx_sb[16 * k:16 * (k + 1), :].rearrange(
                                        "p (a c) -> p a c", a=8),
                                    blob[:, IOFF + off:IOFF + off + ntile]
                                    .bitcast(I16).rearrange(
                                        "(a p) c -> p a c", p=16))
                            ch = eg.tile([128, ntile, 128], BF, tag="ch")
                            nc.gpsimd.dma_gather(
                                out_ap=ch[:],
                                in_ap=tab[b * BUCKET:(b + 1) * BUCKET, :],
                                idxs_ap=idx_sb[:],
                                num_idxs=ntile * 128,
                                num_idxs_reg=ntile * 128,
                                elem_size=128,
                                single_packet=False)
                            dl = dl_all[:, off:off + ntile]
                            # per tile: lhsT = (iota==dstloc)*|d|  (the |d|
                            # row-scale cancels in the softmax division);
                            # accum of (iota==dstloc)*d gives d_edge.
                            lhsT = eqp.tile([128, ntile, 128], BF, tag="eq")
                            dedge = ef.tile([128, ntile], F32, tag="dedge")
                            for tt in range(ntile):
                                k16 = tt // Tb
                                nc.vector.scalar_tensor_tensor(
                                    out=lhsT[:, tt, :], in0=iota_bf[:],
                                    scalar=dl[:, tt:tt + 1],
                                    in1=dbc_bf[:, k16, :],
                                    op0=AL.is_equal, op1=AL.mult,
                                    accum_out=dedge[:, tt:tt + 1])
                            # z = s + d ; w = exp(max(0.2 z, z))
                            z = ef.tile([128, ntile], F32, tag="z")
                            nc.vector.tensor_tensor(
                                out=z[:], in0=ch[:, :, 64], in1=dedge[:],
                                op=AL.add)
                            nc.vector.scalar_tensor_tensor(
                                out=z[:], in0=z[:], scalar=NEG, in1=z[:],
                                op0=AL.mult, op1=AL.max)
                            w = ef.tile([128, ntile], F32, tag="w")
                            nc.scalar.activation(w[:], z[:], ACT.Exp)
                            # ones into s slot for the denominator column
                            nc.vector.tensor_copy(
                                ch[:, :, 64],
                                ones_bf[:].to_broadcast([128, ntile]))
                            # rhs2 = ch[:, :, 0:65] * w (bulk)
                            ch2 = eqp.tile([128, ntile, 65], BF, tag="dmul",
                                           name="ch2")
                            nc.vector.tensor_tensor(
                                out=ch2[:], in0=ch[:, :, 0:65],
                                in1=w[:].to_broadcast([128, ntile, 65]),
                                op=AL.mult)
                            # matmuls
                            for i in range(16):
                                ps = psums[i // 4]
                                csl = slice(65 * (i % 4), 65 * (i % 4) + 65)
                                for t in range(Tb):
                                    tt = i * Tb + t
                                    nc.tensor.matmul(
                                        ps[:, csl],
                                        lhsT=lhsT[:, tt, :],
                                        rhs=ch2[:, tt, :],
                                        start=False,
                                        stop=(b == NBUCKET - 1 and t == Tb - 1))

                        # ---- finalize the group's 16 blocks
                        for i in range(16):
                            blk_id = 16 * g + i
                            ps = psums[i // 4]
                            csl = slice(65 * (i % 4), 65 * (i % 4) + 65)
                            # self loops (dense): rows of own dst block
                            row = ef.tile([128, 65], BF, tag="slrow")
                            nc.sync.dma_start(
                                row[:], tab_own[l][128 * blk_id:
                                                   128 * blk_id + 128, 0:65])
                            zs = ef.tile([128, 1], F32, tag="zs")
                            nc.vector.tensor_tensor(
                                out=zs[:], in0=row[:, 64:65],
                                in1=dstage[l][:, blk_id:blk_id + 1],
                                op=AL.add)
                            nc.vector.scalar_tensor_tensor(
                                out=zs[:], in0=zs[:], scalar=NEG, in1=zs[:],
                                op0=AL.mult, op1=AL.max)
                            ws = ef.tile([128, 1], F32, tag="ws")
                            nc.scalar.activation(ws[:], zs[:], ACT.Exp)
                            nc.vector.tensor_tensor(
                                out=ws[:], in0=ws[:],
                                in1=m01[:, blk_id:blk_id + 1], op=AL.mult)
                            nc.vector.tensor_tensor(
                                out=ws[:], in0=ws[:],
                                in1=dstage[l][:, blk_id:blk_id + 1],
                                op=AL.mult)
                            nc.vector.tensor_copy(
                                row[:, 64:65], ones_bf[:])
                            nc.vector.scalar_tensor_tensor(
                                out=ps[:, csl], in0=row[:, :],
                                scalar=ws[:], in1=ps[:, csl],
                                op0=AL.mult, op1=AL.add)
                            # divide + bias + relu
                            den = ef.tile([128, 1], F32, tag="den")
                            nc.vector.tensor_tensor(
                                out=den[:],
                                in0=ps[:, csl.start + 64:csl.start + 65],
                                in1=mpad[:, blk_id:blk_id + 1], op=AL.add)
                            rec = ef.tile([128, 1], F32, tag="rec")
                            nc.vector.reciprocal(rec[:], den[:])
                            hmid = ef.tile([128, 64], F32, tag="hmid")
                            nc.vector.scalar_tensor_tensor(
                                out=hmid[:],
                                in0=ps[:, csl.start:csl.start + 64],
                                scalar=rec[:], in1=b1t if l == 0 else b2t,
                                op0=AL.mult, op1=AL.add)
                            hout = ef.tile([128, 64], F32, tag="hout")
                            nc.scalar.activation(hout[:], hmid[:], ACT.Relu)
                            # mask pads to exactly 0 (safe for max: h1 >= 0)
                            nc.vector.tensor_scalar_mul(
                                hout[:], hout[:], m01[:, blk_id:blk_id + 1])
                            # transpose (PE) -> [64, 128]
                            hT = ept.tile([64, 128], F32, space="PSUM",
                                          tag="hT")
                            nc.tensor.transpose(hT[:], hout[:], ident[:])
                            if l == 0:
                                nc.vector.tensor_copy(
                                    h_T[:, 128 * blk_id:128 * (blk_id + 1)],
                                    hT[:])
                            # sum pool
                            red = ef.tile([64, GPC], F32, tag="red")
                            nc.vector.tensor_reduce(
                                red[:],
                                hT[:].rearrange("f (g e) -> f g e", g=GPC),
                                axis=mybir.AxisListType.X, op=AL.add)
                            nc.vector.tensor_tensor(
                                out=pool_sm[l][:], in0=pool_sm[l][:],
                                in1=red[:], op=AL.add)
                            # max pool
                            redm = ef.tile([64, GPC], F32, tag="redm")
                            nc.vector.tensor_reduce(
                                redm[:],
                                hT[:].rearrange("f (g e) -> f g e", g=GPC),
                                axis=mybir.AxisListType.X, op=AL.max)
                            nc.vector.tensor_tensor(
                                out=pool_mx[l][:], in0=pool_mx[l][:],
                                in1=redm[:], op=AL.max)

            edge_pass(0)

            # ================= table 2 build =================
            with (
                tc.tile_pool(name="p3", bufs=3) as p3,
                tc.tile_pool(name="p3ps", bufs=2, space="PSUM") as p3ps,
            ):
                for i in range(NB):
                    hps = p3ps.tile([128, 66], F32, space="PSUM", tag="hps2")
                    nc.tensor.matmul(
                        hps[:], lhsT=h_T[:, 128 * i:128 * (i + 1)],
                        rhs=waug2_sb[:], start=True, stop=True)
                    row = p3.tile([128, 65], BF, tag="row2")
                    nc.vector.tensor_copy(row[:], hps[:, 0:65])
                    nc.sync.dma_start(
                        tab_own[1][128 * i:128 * (i + 1), 0:65], row[:])
                    nc.vector.tensor_copy(dstage[1][:, i:i + 1], hps[:, 65:66])
                dt_ps = p3ps.tile([NB, 128], F32, space="PSUM", tag="dt2")
                nc.tensor.transpose(dt_ps[:], dstage[1][:], ident[:])
                dt_sb = p3.tile([NB, 128], F32, tag="dts2")
                nc.vector.tensor_copy(dt_sb[:], dt_ps[:])
                nc.sync.dma_start(d2d[1][:, :], dt_sb[:])

            nc.gpsimd.collective_compute(
                "AllGather", AL.bypass,
                replica_groups=[list(range(NCORES))],
                ins=[tab_own[1].opt()], outs=[table[1].opt()])

            edge_pass(1)

            # ================= pooling combine + MLP =================
            with (
                tc.tile_pool(name="p5", bufs=2) as p5,
                tc.tile_pool(name="p5ps", bufs=1, space="PSUM") as p5ps,
            ):
                icn = smalls[0:64, SM_ICN:SM_ICN + GPC]
                mxh = p5.tile([64, GPC], F32)
                nc.vector.tensor_tensor(out=mxh[:], in0=pool_mx[0][:],
                                        in1=pool_mx[1][:], op=AL.add)
                smh = p5.tile([64, GPC], F32)
                nc.vector.tensor_tensor(out=smh[:], in0=pool_sm[0][:],
                                        in1=pool_sm[1][:], op=AL.add)
                nc.vector.tensor_tensor(out=smh[:], in0=smh[:], in1=icn,
                                        op=AL.mult)
                # transpose to graph-major [GPC, 128] and AllGather
                zloc = p5.tile([GPC, 128], F32)
                mxT = p5ps.tile([GPC, 64], F32, space="PSUM", tag="mxT")
                nc.tensor.transpose(mxT[:], mxh[:], ident[0:64, 0:64])
                nc.vector.tensor_copy(zloc[:, 0:64], mxT[:])
                smT = p5ps.tile([GPC, 64], F32, space="PSUM", tag="smT")
                nc.tensor.transpose(smT[:], smh[:], ident[0:64, 0:64])
                nc.vector.tensor_copy(zloc[:, 64:128], smT[:])
                nc.sync.dma_start(pool_bounce_in[:, :], zloc[:])
                nc.gpsimd.collective_compute(
                    "AllGather", AL.bypass,
                    replica_groups=[list(range(NCORES))],
                    ins=[pool_bounce_in.opt()], outs=[pool_bounce_out.opt()])
                zg = p5.tile([G, 128], F32)
                nc.sync.dma_start(zg[:], pool_bounce_out[:, :])
                zT_ps = p5ps.tile([128, G], F32, space="PSUM", tag="zT")
                nc.tensor.transpose(zT_ps[:], zg[:], ident[:])
                zT = p5.tile([128, G], F32)
                nc.vector.tensor_copy(zT[:], zT_ps[:])
                mlp_ps = p5ps.tile([G, 64], F32, space="PSUM", tag="mlp")
                nc.tensor.matmul(mlp_ps[:], lhsT=zT[:],
                                 rhs=smalls[:, SM_L1W:SM_L1W + 64],
                                 start=True, stop=True)
                z1 = p5.tile([G, 64], F32)
                nc.vector.tensor_tensor(out=z1[:], in0=mlp_ps[:], in1=l1bt,
                                        op=AL.add)
                nc.scalar.activation(z1[:], z1[:], ACT.Relu)
                z2 = p5.tile([G, 64], F32)
                nc.vector.tensor_tensor(out=z2[:], in0=z1[:], in1=l2rt,
                                        op=AL.mult)
                ored = p5.tile([G, 1], F32)
                nc.vector.tensor_reduce(ored[:], z2[:],
                                        axis=mybir.AxisListType.X, op=AL.add)
                nc.vector.tensor_tensor(out=ored[:], in0=ored[:], in1=b2c[:],
                                        op=AL.add)
                nc.sync.dma_start(out_final[:, :], ored[:])

    nc.compile()
    return nc


# ---------------------------------------------------------------- runner
# run_bass_kernel_spmd rebuilds jax.jit(shard_map(...)) on every call, which
# re-deserializes and re-loads the NEFF executable (~0.3s/call under axon).
# Build the same jit once and reuse it; the execution path is identical.
def _make_runner(nc):
    import jax
    from jax.sharding import Mesh, PartitionSpec
    from jax.experimental.shard_map import shard_map
    from concourse.bass2jax import (_bass_exec_p, partition_id_tensor,
                                    install_neuronx_cc_hook)

    install_neuronx_cc_hook()
    partition_name = (nc.partition_id_tensor.name
                      if nc.partition_id_tensor else None)
    in_names, out_names, out_avals, zero_shapes = [], [], [], []
    for alloc in nc.m.functions[0].allocations:
        if not isinstance(alloc, mybir.MemoryLocationSet):
            continue
        name = alloc.memorylocations[0].name
        if alloc.kind == "ExternalInput":
            if name != partition_name:
                in_names.append(name)
        elif alloc.kind == "ExternalOutput":
            out_names.append(name)
            shape = tuple(alloc.tensor_shape)
            dtype = mybir.dt.np(alloc.dtype)
            out_avals.append(jax.core.ShapedArray(shape, dtype))
            zero_shapes.append((shape, dtype))
    n_params = len(in_names)
    n_outs = len(out_avals)
    all_in_names = list(in_names) + list(out_names)
    if partition_name is not None:
        all_in_names.append(partition_name)

    def _body(*args):
        operands = list(args)
        if partition_name is not None:
            operands.append(partition_id_tensor())
        outs = _bass_exec_p.bind(
            *operands, out_avals=tuple(out_avals),
            in_names=tuple(all_in_names), out_names=tuple(out_names),
            lowering_input_output_aliases=(), sim_require_finite=True,
            sim_require_nnan=True, nc=nc)
        return tuple(outs)

    devices = jax.devices()[:NCORES]
    mesh = Mesh(np.asarray(devices), ("core",))
    sharded = jax.jit(
        shard_map(_body, mesh=mesh,
                  in_specs=(PartitionSpec("core"),) * (n_params + n_outs),
                  out_specs=(PartitionSpec("core"),) * n_outs,
                  check_rep=False),
        donate_argnums=tuple(range(n_params, n_params + n_outs)),
        keep_unused=True)

    def run(in_maps):
        concat_in = [
            np.concatenate([np.asarray(m[n]) for m in in_maps], axis=0)
            for n in in_names]
        concat_zeros = [np.zeros((NCORES * s[0], *s[1:]), d)
                        for s, d in zero_shapes]
        outs = sharded(*concat_in, *concat_zeros)
        return [
            {name: np.asarray(outs[i]).reshape(NCORES, *out_avals[i].shape)[c]
             for i, name in enumerate(out_names)}
            for c in range(NCORES)]

    return run


def _get_runner(nc):
    try:
        return _make_runner(nc)
    except Exception:
        return lambda in_maps: bass_utils.run_bass_kernel_spmd(
            nc, in_maps, core_ids=list(range(NCORES))).results


# ---------------------------------------------------------------- entry
def kernel(**inputs) -> np.ndarray:
    tpl, per_core = _host_prep(inputs)
    nc = _build(tpl)
    in_maps = [{"blob": per_core[c]["blob"]} for c in range(NCORES)]
    results = _get_runner(nc)(in_maps)
    out = np.asarray(results[0]["out_final"]).reshape(G)
    return out.astype(np.float32)


# revision 20
# speedup vs baseline: 7.2249x; 1.0422x over previous
"""GAT (2-layer, heads=1) + pooling + MLP on 8 Trainium2 NeuronCores.

Strategy:
- Nodes are mapped to per-graph padded slots (SLOT = align128(max graph size)),
  graphs are sharded 16-per-core, and within each core slots are striped so
  that dst-block i holds slots s with s % NB == i.  Every 128-slot block then
  contains exactly 8 slots of each of the core's 16 graphs (partition p ->
  local graph p//8), which makes pooling segment boundaries compile-time.
- Edge aggregation: per-edge rows [h|s] are fetched with dma_gather (int16
  indices -> 4 src buckets); attention weights w = exp(leakyrelu(s_src +
  d_dst)) are computed on-chip (d expanded per edge via a one-hot *
  broadcast-d reduce); the segment softmax + feature sum is one matmul per
  128-edge tile accumulating [sum(w*h) | sum(w)] into a per-block PSUM.
- Self-loop edges are applied densely at finalize (no gather).
- Node tables ([h|s] rows, bf16) are built sharded and AllGathered; pooled
  [G,2H] is assembled with a single small AllGather; the final MLP is
  replicated.
- All host inputs are packed into ONE bf16 blob per core (x in bf16 split
  into two 64-row halves, gather indices stored once instead of 8x, dst
  rows as int8, small tensors bit-packed f32) and unpacked on device --
  the axon tunnel transfer (~46MB/s + ~70ms/array) dominates wall-clock.
"""
import sys

sys.path.insert(0, "/opt/trn_rl_repo")

import numpy as np
import ml_dtypes

try:  # persistent XLA compile cache: repeat calls skip the NEFF re-wrap
    import jax

    jax.config.update("jax_compilation_cache_dir", "/tmp/jaxcache")
    jax.config.update("jax_persistent_cache_min_entry_size_bytes", -1)
    jax.config.update("jax_persistent_cache_min_compile_time_secs", 0)
except Exception:
    pass

import concourse.bacc as bacc
import concourse.bass as bass
import concourse.mybir as mybir
import concourse.tile as tile
from concourse import bass_utils
from concourse.masks import make_identity

bf16 = ml_dtypes.bfloat16
F32 = mybir.dt.float32
BF = mybir.dt.bfloat16
I16 = mybir.dt.int16
I8 = mybir.dt.int8
AL = mybir.AluOpType
ACT = mybir.ActivationFunctionType

NCORES = 8
G = 128
N = 100000
FIN = 64
H = 64
NEG = 0.2
NBUCKET = 4
GPC = G // NCORES  # graphs per core = 16
NEG_BIG = -1.0e30
X_INT8 = True  # ship x as int8 (scale folded into waug1 on host)


# ---------------------------------------------------------------- host prep
def _host_prep(inputs):
    x = np.asarray(inputs["x"], np.float32)
    ei = np.asarray(inputs["edge_index"]).astype(np.int64)
    bid = np.asarray(inputs["batch_ids"]).astype(np.int64)

    cnt = np.bincount(bid, minlength=G).astype(np.int64)
    SLOT = int(np.ceil(max(cnt.max(), 128) / 128) * 128)
    NSLOT = GPC * SLOT
    NB = NSLOT // 128
    assert NB % 16 == 0 and NB % 4 == 0, NB
    NGRP = NB // 16
    NSLOT_G = NCORES * NSLOT
    BUCKET = NSLOT_G // NBUCKET
    assert BUCKET * NBUCKET == NSLOT_G and BUCKET <= 32768

    gstart = np.zeros(G + 1, np.int64)
    gstart[1:] = np.cumsum(cnt)
    rank = np.arange(N, dtype=np.int64) - gstart[bid]
    slot_of = bid * SLOT + rank  # graph-padded slot, 0..NSLOT_G

    def pi(s):
        c, sl = s // NSLOT, s % NSLOT
        return c * NSLOT, (sl % NB) * 128 + sl // NB

    core_base, loc = pi(slot_of)
    pi_of = core_base + loc  # global pi row of each node

    # permuted x per core (pad rows zero); int8 with per-column scales,
    # folded into the rows of waug1 below (h = sum_c xq[:,c] * s_c * W1[c,:])
    if X_INT8:
        s_q = np.abs(x).max(axis=0) / 127.0  # [FIN]
        xq = np.clip(np.rint(x / s_q), -127, 127).astype(np.int8)
        x_pi = np.zeros((NCORES, NSLOT, FIN), np.int8)
        x_pi[pi_of // NSLOT, pi_of % NSLOT] = xq
    else:
        s_q = np.ones(FIN, np.float32)
        x_pi = np.zeros((NCORES, NSLOT, FIN), np.float32)
        x_pi[pi_of // NSLOT, pi_of % NSLOT] = x

    # masks / counts per core
    # local slot (block i, partition p) <-> graph-slot p*NB + i
    pp, ii = np.meshgrid(np.arange(128), np.arange(NB), indexing="ij")
    gslot = pp * NB + ii  # [128, NB] graph-padded local slot
    within = gslot % SLOT  # rank within graph
    lg = gslot // SLOT  # local graph 0..15 (== pp//8)
    mask01 = np.zeros((NCORES, 128, NB), np.float32)
    for c in range(NCORES):
        real = within < cnt[c * GPC + lg]
        mask01[c] = real.astype(np.float32)

    # edges (no self loops in the gather path)
    src, dst = ei[0], ei[1]
    ps = pi_of[src]
    pd = pi_of[dst]
    core = pd // NSLOT
    blk = (pd % NSLOT) // 128
    bkt = ps // BUCKET
    grp = blk // 16

    cnts = np.zeros((NCORES, NB, NBUCKET), np.int64)
    np.add.at(cnts, (core, blk, bkt), 1)
    # uniform tiles-per-block within each (group, bucket)
    tpb = np.zeros((NGRP, NBUCKET), np.int64)  # tiles per block
    for g in range(NGRP):
        for b in range(NBUCKET):
            m = cnts[:, 16 * g:16 * g + 16, b].max()
            tpb[g, b] = max(1, int(np.ceil(m / 128)))
    seg_tiles = (16 * tpb).astype(np.int64)  # tiles per (g,b) segment
    TT = int(seg_tiles.sum())  # total tiles per core per layer
    assert TT % 2 == 0, TT
    TOTSLOT = TT * 128

    # slot offsets: order (g, b, block-within-group, slot)
    seg_off = np.zeros((NGRP, NBUCKET), np.int64)
    acc = 0
    for g in range(NGRP):
        for b in range(NBUCKET):
            seg_off[g, b] = acc
            acc += seg_tiles[g, b] * 128

    order = np.lexsort((bkt, blk))  # edges sorted by (blk, bkt); core split below
    src_local = np.zeros((NCORES, 128, TT), np.int16)
    dst_i8 = np.full((NCORES, 128, TT), -1, np.int8)
    for c in range(NCORES):
        sel = order[core[order] == c]
        sblk, sbkt = blk[sel], bkt[sel]
        sps, spd = ps[sel], pd[sel]
        # slot index for each edge: within its (g,b,block) run
        # run start: seg_off[g,b] + (blk%16)*tpb[g,b]*128; position = rank in run
        key = sblk * NBUCKET + sbkt
        # stable order already (blk, bkt); rank within run:
        runstart_mark = np.r_[True, key[1:] != key[:-1]]
        runid = np.cumsum(runstart_mark) - 1
        nruns = int(runid[-1]) + 1 if len(runid) else 0
        first = np.full(nruns, len(sel), np.int64)
        np.minimum.at(first, runid, np.arange(len(sel)))
        pos = np.arange(len(sel)) - first[runid]
        gg = sblk // 16
        slot = (seg_off[gg, sbkt] + (sblk % 16) * tpb[gg, sbkt] * 128 + pos)
        assert (pos < tpb[gg, sbkt] * 128).all()
        p_ = slot % 128
        t_ = slot // 128
        src_local[c, p_, t_] = (sps - sbkt * BUCKET).astype(np.int16)
        dst_i8[c, p_, t_] = (spd % NSLOT % 128).astype(np.int8)

    # wrapped int16 gather indices, stored compactly: per (g,b) segment the
    # [16, ntile*8] wrap is folded into [128, ntile] (row 16a+r holds
    # wseg[r, a*ntile : (a+1)*ntile]); the 8x replication the gather engine
    # wants is re-created on device with 8 DMAs per segment.
    idx_pack = np.zeros((NCORES, 128, TT), np.int16)
    for c in range(NCORES):
        flat = np.zeros(TOTSLOT, np.int16)
        sl = src_local[c]
        flat[np.arange(TOTSLOT)] = sl[np.arange(TOTSLOT) % 128,
                                      np.arange(TOTSLOT) // 128]
        w = flat.reshape(TOTSLOT // 16, 16).T  # [16, TT*8]
        for g in range(NGRP):
            for b in range(NBUCKET):
                off_t = int(seg_off[g, b]) // 128
                ntile = int(seg_tiles[g, b])
                wseg = w[:, off_t * 8:(off_t + ntile) * 8]
                idx_pack[c][:, off_t:off_t + ntile] = (
                    wseg.reshape(16, 8, ntile).transpose(1, 0, 2)
                    .reshape(128, ntile))

    # weights
    W1 = np.asarray(inputs["W1"], np.float32)
    W2 = np.asarray(inputs["W2"], np.float32)
    waug1 = s_q[:, None] * np.concatenate(
        [W1, (W1 @ np.asarray(inputs["a_src1"], np.float32))[:, None],
         (W1 @ np.asarray(inputs["a_dst1"], np.float32))[:, None]], axis=1)
    waug2 = np.concatenate(
        [W2, (W2 @ np.asarray(inputs["a_src2"], np.float32))[:, None],
         (W2 @ np.asarray(inputs["a_dst2"], np.float32))[:, None]], axis=1)

    b1 = np.asarray(inputs["b1"], np.float32)
    b2v = np.asarray(inputs["b2"], np.float32)
    lin1_W = np.asarray(inputs["lin1_W"], np.float32)
    lin1_b = np.asarray(inputs["lin1_b"], np.float32)
    lin2_W = np.asarray(inputs["lin2_W"], np.float32)
    lin2_b = np.asarray(inputs["lin2_b"], np.float32)

    invcnt = np.zeros((NCORES, 64, GPC), np.float32)
    for c in range(NCORES):
        invcnt[c] = np.broadcast_to(
            1.0 / np.maximum(cnt[c * GPC:(c + 1) * GPC], 1.0), (64, GPC))

    # ---- blob layout (bf16 columns); x is stored as two 64-row halves
    XH = NSLOT // 2  # x columns per half
    XCOLS = (NSLOT // 4) if X_INT8 else (NSLOT // 2)  # bf16 cols of X region
    IOFF = XCOLS
    DOFF = XCOLS + TT
    MOFF = DOFF + TT // 2  # int8 mask01 [128, NB]
    SOFF = MOFF + NB // 2
    S = 277  # f32 cols in the small region
    CB = SOFF + 2 * S

    per_core = []
    for c in range(NCORES):
        blob = np.zeros((128, CB * 2), np.uint8)
        if X_INT8:
            xT_q = np.ascontiguousarray(x_pi[c].T)  # [64, NSLOT] int8
            blob[0:64, 0:XH] = np.ascontiguousarray(
                xT_q[:, 0:XH]).view(np.uint8)
            blob[64:128, 0:XH] = np.ascontiguousarray(
                xT_q[:, XH:]).view(np.uint8)
        else:
            xT_bf = np.ascontiguousarray(x_pi[c].T).astype(bf16)
            blob[0:64, 0:XH * 2] = np.ascontiguousarray(
                xT_bf[:, 0:XH]).view(np.uint8)
            blob[64:128, 0:XH * 2] = np.ascontiguousarray(
                xT_bf[:, XH:]).view(np.uint8)
        blob[:, IOFF * 2:(IOFF + TT) * 2] = idx_pack[c].view(np.uint8)
        blob[:, DOFF * 2:DOFF * 2 + TT] = dst_i8[c].view(np.uint8)
        blob[:, MOFF * 2:MOFF * 2 + NB] = mask01[c].astype(np.int8)
        sm = np.zeros((128, S), np.float32)
        sm[:, 0:64] = lin1_W
        sm[0:64, 64:130] = waug1
        sm[0:64, 130:196] = waug2
        sm[0:64, 196:196 + GPC] = invcnt[c]
        SM_B = 196 + GPC
        sm[0, SM_B:SM_B + 64] = b1
        sm[1, SM_B:SM_B + 64] = b2v
        sm[2, SM_B:SM_B + 64] = lin1_b
        sm[3, SM_B:SM_B + 64] = lin2_W[:, 0]
        sm[0, SM_B + 64] = lin2_b[0]
        blob[:, SOFF * 2:SOFF * 2 + S * 4] = sm.view(np.uint8)
        per_core.append({"blob": blob.view(bf16)})

    tpl = dict(SLOT=SLOT, NSLOT=NSLOT, NB=NB, NGRP=NGRP, NSLOT_G=NSLOT_G,
               BUCKET=BUCKET, tpb=tpb, seg_tiles=seg_tiles, seg_off=seg_off,
               TT=TT, XH=XH, IOFF=IOFF, DOFF=DOFF, MOFF=MOFF, SOFF=SOFF,
               S=S, CB=CB)
    return tpl, per_core


# ---------------------------------------------------------------- device bld
def _build(tpl):
    NSLOT = tpl["NSLOT"]
    NB = tpl["NB"]
    NGRP = tpl["NGRP"]
    NSLOT_G = tpl["NSLOT_G"]
    BUCKET = tpl["BUCKET"]
    tpb = tpl["tpb"]
    seg_tiles = tpl["seg_tiles"]
    TT = tpl["TT"]
    XH = tpl["XH"]
    IOFF = tpl["IOFF"]
    DOFF = tpl["DOFF"]
    MOFF = tpl["MOFF"]
    SOFF = tpl["SOFF"]
    S = tpl["S"]
    CB = tpl["CB"]
    SM_L1W = 0
    SM_WG1 = 64
    SM_WG2 = 130
    SM_ICN = 196
    SM_B = 196 + GPC
    SM_B2 = SM_B + 64

    nc = bacc.Bacc("TRN2", target_bir_lowering=False, debug=False,
                   num_devices=NCORES)

    blob = nc.dram_tensor("blob", [128, CB], BF, kind="ExternalInput")
    out_final = nc.dram_tensor("out_final", [128, 1], F32,
                               kind="ExternalOutput")

    with tile.TileContext(nc) as tc:
        with (
            tc.tile_pool(name="const", bufs=1) as cp,
            tc.tile_pool(name="stage", bufs=1) as stp,
            tc.tile_pool(name="dram", bufs=1, space="DRAM") as dr,
        ):
            # ---- constants in SBUF
            iota_i = cp.tile([128, 128], mybir.dt.int32)
            nc.gpsimd.iota(iota_i[:], pattern=[[1, 128]], base=0,
                           channel_multiplier=0)
            iota_bf = cp.tile([128, 128], BF)
            nc.vector.tensor_copy(iota_bf[:], iota_i[:])
            ident = cp.tile([128, 128], F32)
            make_identity(nc, ident[:])
            ones_bf = cp.tile([128, 1], BF)
            nc.gpsimd.memset(ones_bf[:], 1.0)
            onec = cp.tile([128, 1], F32)
            nc.gpsimd.memset(onec[:], 1.0)

            # ---- unpack the blob
            smalls = cp.tile([128, S], F32)
            nc.sync.dma_start(
                smalls[:], blob[:, SOFF:SOFF + 2 * S].bitcast(F32))
            m8 = stp.tile([128, NB], I8, name="m8")
            nc.sync.dma_start(m8[:], blob[:, MOFF:MOFF + NB // 2].bitcast(I8))
            m01 = cp.tile([128, NB], F32)
            nc.vector.tensor_copy(m01[:], m8[:])
            mpad = cp.tile([128, NB], F32)
            nc.vector.scalar_tensor_tensor(
                out=mpad[:], in0=m01[:], scalar=-1.0,
                in1=onec[:].to_broadcast([128, NB]),
                op0=AL.mult, op1=AL.add)
            waug1_sb = cp.tile([64, 66], BF)
            nc.vector.tensor_copy(waug1_sb[:], smalls[0:64, SM_WG1:SM_WG1 + 66])
            waug2_sb = cp.tile([64, 66], BF)
            nc.vector.tensor_copy(waug2_sb[:], smalls[0:64, SM_WG2:SM_WG2 + 66])
            btiles = cp.tile([128, 4, 64], F32)
            nc.sync.dma_start(
                btiles[:],
                blob[0:4, SOFF + 2 * SM_B:SOFF + 2 * SM_B + 128].bitcast(
                    F32).rearrange("(o r) c -> o r c", o=1).to_broadcast(
                    [128, 4, 64]))
            b1t = btiles[:, 0, :]
            b2t = btiles[:, 1, :]
            l1bt = btiles[:, 2, :]
            l2rt = btiles[:, 3, :]
            b2c = cp.tile([128, 1], F32)
            nc.sync.dma_start(
                b2c[:],
                blob[0:1, SOFF + 2 * SM_B2:SOFF + 2 * SM_B2 + 2].bitcast(
                    F32).to_broadcast([128, 1]))
            dst_i8 = stp.tile([128, TT], I8, name="dsti8")
            nc.sync.dma_start(dst_i8[:], blob[:, DOFF:DOFF + TT // 2].bitcast(I8))
            dl_all = cp.tile([128, TT], F32)
            nc.vector.tensor_copy(dl_all[:], dst_i8[:])

            # persistent staging
            h_T = stp.tile([64, NSLOT], BF, tag="h_T", name="h_T")  # layer1 out, pi order
            dstage = [stp.tile([128, NB], F32, tag=f"dstage{l}", name=f"dstage{l}")
                      for l in range(2)]
            pool_mx = [stp.tile([64, GPC], F32, tag=f"pmx{l}", name=f"pmx{l}")
                       for l in range(2)]
            pool_sm = [stp.tile([64, GPC], F32, tag=f"psm{l}", name=f"psm{l}")
                       for l in range(2)]
            for l in range(2):
                nc.vector.memset(pool_mx[l][:], NEG_BIG)
                nc.vector.memset(pool_sm[l][:], 0.0)

            # DRAM scratch
            table = [dr.tile([NSLOT_G, 128], BF, tag=f"tab{l}", name=f"tab{l}")
                     for l in range(2)]
            tab_own = [dr.tile([NSLOT, 128], BF, tag=f"tabown{l}", name=f"tabown{l}")
                       for l in range(2)]
            d2d = [dr.tile([NB, 128], F32, tag=f"d2d{l}", name=f"d2d{l}") for l in range(2)]
            pool_bounce_in = dr.tile([GPC, 128], F32)
            pool_bounce_out = dr.tile([G, 128], F32)

            # ===== table 1 build: own shard of [h|s] rows + d column =====
            with (
                tc.tile_pool(name="p1", bufs=3) as p1,
                tc.tile_pool(name="p1x", bufs=3) as p1x,
                tc.tile_pool(name="p1ps", bufs=2, space="PSUM") as p1ps,
            ):
                QB = NB // 4
                for s in range(4):
                    half = (s * QB) // (NB // 2)
                    within = (s * QB) % (NB // 2)
                    rows0 = 64 * half
                    xT = p1x.tile([64, QB * 128], BF, tag="xTs")
                    if X_INT8:
                        coff = within * 64
                        xq8 = p1x.tile([64, QB * 128], I8, tag="xq8")
                        nc.sync.dma_start(
                            xq8[:],
                            blob[rows0:rows0 + 64,
                                 coff:coff + QB * 64].bitcast(I8))
                        nc.vector.tensor_copy(xT[:], xq8[:])
                    else:
                        coff = within * 128
                        nc.sync.dma_start(
                            xT[:],
                            blob[rows0:rows0 + 64, coff:coff + QB * 128])
                    rows = p1.tile([128, QB, 65], BF, tag="rows")
                    for j in range(QB):
                        blk_id = s * QB + j
                        hps = p1ps.tile([128, 66], F32, space="PSUM",
                                        tag="hps")
                        nc.tensor.matmul(
                            hps[:], lhsT=xT[:, 128 * j:128 * (j + 1)],
                            rhs=waug1_sb[:], start=True, stop=True)
                        nc.scalar.activation(rows[:, j, :], hps[:, 0:65],
                                             ACT.Copy)
                        nc.vector.tensor_copy(
                            dstage[0][:, blk_id:blk_id + 1],
                            hps[:, 65:66])
                    nc.sync.dma_start(
                        tab_own[0].rearrange("(s r) c -> r s c", r=128)[
                            :, s * QB:(s + 1) * QB, 0:65],
                        rows[:, 0:QB, :])
                # d transpose -> DRAM [NB, 128]
                dt_ps = p1ps.tile([NB, 128], F32, space="PSUM", tag="dt")
                nc.tensor.transpose(dt_ps[:], dstage[0][:], ident[:])
                dt_sb = p1.tile([NB, 128], F32, tag="dts")
                nc.vector.tensor_copy(dt_sb[:], dt_ps[:])
                nc.sync.dma_start(d2d[0][:, :], dt_sb[:])

            nc.gpsimd.collective_compute(
                "AllGather", AL.bypass,
                replica_groups=[list(range(NCORES))],
                ins=[tab_own[0].opt()], outs=[table[0].opt()])

            # ================= edge pass (both layers) =================
            def edge_pass(l):
                tab = table[l]
                with (
                    tc.tile_pool(name=f"eg{l}", bufs=2) as eg,
                    tc.tile_pool(name=f"eq{l}", bufs=2) as eqp,
                    tc.tile_pool(name=f"ep{l}", bufs=1, space="PSUM") as eps,
                    tc.tile_pool(name=f"et{l}", bufs=2, space="PSUM") as ept,
                    tc.tile_pool(name=f"ef{l}", bufs=3) as ef,
                ):
                    for g in range(NGRP):
                        # broadcast d rows of the group's 16 blocks
                        dbc = ef.tile([128, 16, 128], F32, tag="dbc")
                        nc.sync.dma_start(
                            dbc[:],
                            d2d[l][16 * g:16 * (g + 1), :].rearrange(
                                "(o r) c -> o r c", o=1).to_broadcast(
                                [128, 16, 128]))
                        dbc_bf = ef.tile([128, 16, 128], BF, tag="dbcb")
                        nc.vector.tensor_copy(dbc_bf[:], dbc[:])

                        psums = [eps.tile([128, 260], F32, space="PSUM",
                                          tag=f"ps{k}", name=f"ps{k}")
                                 for k in range(4)]
                        for ps_ in psums:
                            nc.vector.memset(ps_[:], 0.0)

                        for b in range(NBUCKET):
                            ntile = int(seg_tiles[g, b])
                            Tb = int(tpb[g, b])
                            off = int(tpl["seg_off"][g, b]) // 128  # tile off
                            # gather the whole (g,b) segment; indices come
                            # from the compact blob region, replicated 8x
                            idx_sb = eg.tile([128, ntile * 8], I16, tag="idx")
                            for k in range(8):
                                nc.sync.dma_start(
                                    idx_sb[16 * k:16 * (k + 1), :].rearrange(
                                        "p (a c) -> p a c", a=8),
                                    blob[:, IOFF + off:IOFF + off + ntile]
                                    .bitcast(I16).rearrange(
                                        "(a p) c -> p a c", p=16))
                            ch = eg.tile([128, ntile, 128], BF, tag="ch")
                            nc.gpsimd.dma_gather(
                                out_ap=ch[:],
                                in_ap=tab[b * BUCKET:(b + 1) * BUCKET, :],
                                idxs_ap=idx_sb[:],
                                num_idxs=ntile * 128,
                                num_idxs_reg=ntile * 128,
                                elem_size=128,
                                single_packet=False)
                            dl = dl_all[:, off:off + ntile]
                            # per tile: lhsT = (iota==dstloc)*|d|  (the |d|
                            # row-scale cancels in the softmax division);
                            # accum of (iota==dstloc)*d gives d_edge.
                            lhsT = eqp.tile([128, ntile, 128], BF, tag="eq")
                            dedge = ef.tile([128, ntile], F32, tag="dedge")
                            for tt in range(ntile):
                                k16 = tt // Tb
                                nc.vector.scalar_tensor_tensor(
                                    out=lhsT[:, tt, :], in0=iota_bf[:],
                                    scalar=dl[:, tt:tt + 1],
                                    in1=dbc_bf[:, k16, :],
                                    op0=AL.is_equal, op1=AL.mult,
                                    accum_out=dedge[:, tt:tt + 1])
                            # z = s + d ; w = exp(max(0.2 z, z))
                            z = ef.tile([128, ntile], F32, tag="z")
                            nc.vector.tensor_tensor(
                                out=z[:], in0=ch[:, :, 64], in1=dedge[:],
                                op=AL.add)
                            nc.vector.scalar_tensor_tensor(
                                out=z[:], in0=z[:], scalar=NEG, in1=z[:],
                                op0=AL.mult, op1=AL.max)
                            w = ef.tile([128, ntile], F32, tag="w")
                            nc.scalar.activation(w[:], z[:], ACT.Exp)
                            # ones into s slot for the denominator column
                            nc.vector.tensor_copy(
                                ch[:, :, 64],
                                ones_bf[:].to_broadcast([128, ntile]))
                            # rhs2 = ch[:, :, 0:65] * w (bulk)
                            ch2 = eqp.tile([128, ntile, 65], BF, tag="dmul",
                                           name="ch2")
                            nc.vector.tensor_tensor(
                                out=ch2[:], in0=ch[:, :, 0:65],
                                in1=w[:].to_broadcast([128, ntile, 65]),
                                op=AL.mult)
                            # matmuls
                            for i in range(16):
                                ps = psums[i // 4]
                                csl = slice(65 * (i % 4), 65 * (i % 4) + 65)
                                for t in range(Tb):
                                    tt = i * Tb + t
                                    nc.tensor.matmul(
                                        ps[:, csl],
                                        lhsT=lhsT[:, tt, :],
                                        rhs=ch2[:, tt, :],
                                        start=False,
                                        stop=(b == NBUCKET - 1 and t == Tb - 1))

                        # ---- finalize the group's 16 blocks
                        for i in range(16):
                            blk_id = 16 * g + i
                            ps = psums[i // 4]
                            csl = slice(65 * (i % 4), 65 * (i % 4) + 65)
                            # self loops (dense): rows of own dst block
                            row = ef.tile([128, 65], BF, tag="slrow")
                            nc.sync.dma_start(
                                row[:], tab_own[l][128 * blk_id:
                                                   128 * blk_id + 128, 0:65])
                            zs = ef.tile([128, 1], F32, tag="zs")
                            nc.vector.tensor_tensor(
                                out=zs[:], in0=row[:, 64:65],
                                in1=dstage[l][:, blk_id:blk_id + 1],
                                op=AL.add)
                            nc.vector.scalar_tensor_tensor(
                                out=zs[:], in0=zs[:], scalar=NEG, in1=zs[:],
                                op0=AL.mult, op1=AL.max)
                            ws = ef.tile([128, 1], F32, tag="ws")
                            nc.scalar.activation(ws[:], zs[:], ACT.Exp)
                            nc.vector.tensor_tensor(
                                out=ws[:], in0=ws[:],
                                in1=m01[:, blk_id:blk_id + 1], op=AL.mult)
                            nc.vector.tensor_tensor(
                                out=ws[:], in0=ws[:],
                                in1=dstage[l][:, blk_id:blk_id + 1],
                                op=AL.mult)
                            nc.vector.tensor_copy(
                                row[:, 64:65], ones_bf[:])
                            nc.vector.scalar_tensor_tensor(
                                out=ps[:, csl], in0=row[:, :],
                                scalar=ws[:], in1=ps[:, csl],
                                op0=AL.mult, op1=AL.add)
                            # divide + bias + relu
                            den = ef.tile([128, 1], F32, tag="den")
                            nc.vector.tensor_tensor(
                                out=den[:],
                                in0=ps[:, csl.start + 64:csl.start + 65],
                                in1=mpad[:, blk_id:blk_id + 1], op=AL.add)
                            rec = ef.tile([128, 1], F32, tag="rec")
                            nc.vector.reciprocal(rec[:], den[:])
                            hmid = ef.tile([128, 64], F32, tag="hmid")
                            nc.vector.scalar_tensor_tensor(
                                out=hmid[:],
                                in0=ps[:, csl.start:csl.start + 64],
                                scalar=rec[:], in1=b1t if l == 0 else b2t,
                                op0=AL.mult, op1=AL.add)
                            hout = ef.tile([128, 64], F32, tag="hout")
                            nc.scalar.activation(hout[:], hmid[:], ACT.Relu)
                            # mask pads to exactly 0 (safe for max: h1 >= 0)
                            nc.vector.tensor_scalar_mul(
                                hout[:], hout[:], m01[:, blk_id:blk_id + 1])
                            # transpose (PE) -> [64, 128]
                            hT = ept.tile([64, 128], F32, space="PSUM",
                                          tag="hT")
                            nc.tensor.transpose(hT[:], hout[:], ident[:])
                            if l == 0:
                                nc.vector.tensor_copy(
                                    h_T[:, 128 * blk_id:128 * (blk_id + 1)],
                                    hT[:])
                            # sum pool
                            red = ef.tile([64, GPC], F32, tag="red")
                            nc.vector.tensor_reduce(
                                red[:],
                                hT[:].rearrange("f (g e) -> f g e", g=GPC),
                                axis=mybir.AxisListType.X, op=AL.add)
                            nc.vector.tensor_tensor(
                                out=pool_sm[l][:], in0=pool_sm[l][:],
                                in1=red[:], op=AL.add)
                            # max pool
                            redm = ef.tile([64, GPC], F32, tag="redm")
                            nc.vector.tensor_reduce(
                                redm[:],
                                hT[:].rearrange("f (g e) -> f g e", g=GPC),
                                axis=mybir.AxisListType.X, op=AL.max)
                            nc.vector.tensor_tensor(
                                out=pool_mx[l][:], in0=pool_mx[l][:],
                                in1=redm[:], op=AL.max)

            edge_pass(0)

            # ================= table 2 build =================
            with (
                tc.tile_pool(name="p3", bufs=3) as p3,
                tc.tile_pool(name="p3ps", bufs=2, space="PSUM") as p3ps,
            ):
                for i in range(NB):
                    hps = p3ps.tile([128, 66], F32, space="PSUM", tag="hps2")
                    nc.tensor.matmul(
                        hps[:], lhsT=h_T[:, 128 * i:128 * (i + 1)],
                        rhs=waug2_sb[:], start=True, stop=True)
                    row = p3.tile([128, 65], BF, tag="row2")
                    nc.vector.tensor_copy(row[:], hps[:, 0:65])
                    nc.sync.dma_start(
                        tab_own[1][128 * i:128 * (i + 1), 0:65], row[:])
                    nc.vector.tensor_copy(dstage[1][:, i:i + 1], hps[:, 65:66])
                dt_ps = p3ps.tile([NB, 128], F32, space="PSUM", tag="dt2")
                nc.tensor.transpose(dt_ps[:], dstage[1][:], ident[:])
                dt_sb = p3.tile([NB, 128], F32, tag="dts2")
                nc.vector.tensor_copy(dt_sb[:], dt_ps[:])
                nc.sync.dma_start(d2d[1][:, :], dt_sb[:])

            nc.gpsimd.collective_compute(
                "AllGather", AL.bypass,
                replica_groups=[list(range(NCORES))],
                ins=[tab_own[1].opt()], outs=[table[1].opt()])

            edge_pass(1)

            # ================= pooling combine + MLP =================
            with (
                tc.tile_pool(name="p5", bufs=2) as p5,
                tc.tile_pool(name="p5ps", bufs=1, space="PSUM") as p5ps,
            ):
                icn = smalls[0:64, SM_ICN:SM_ICN + GPC]
                mxh = p5.tile([64, GPC], F32)
                nc.vector.tensor_tensor(out=mxh[:], in0=pool_mx[0][:],
                                        in1=pool_mx[1][:], op=AL.add)
                smh = p5.tile([64, GPC], F32)
                nc.vector.tensor_tensor(out=smh[:], in0=pool_sm[0][:],
                                        in1=pool_sm[1][:], op=AL.add)
                nc.vector.tensor_tensor(out=smh[:], in0=smh[:], in1=icn,
                                        op=AL.mult)
                # transpose to graph-major [GPC, 128] and AllGather
                zloc = p5.tile([GPC, 128], F32)
                mxT = p5ps.tile([GPC, 64], F32, space="PSUM", tag="mxT")
                nc.tensor.transpose(mxT[:], mxh[:], ident[0:64, 0:64])
                nc.vector.tensor_copy(zloc[:, 0:64], mxT[:])
                smT = p5ps.tile([GPC, 64], F32, space="PSUM", tag="smT")
                nc.tensor.transpose(smT[:], smh[:], ident[0:64, 0:64])
                nc.vector.tensor_copy(zloc[:, 64:128], smT[:])
                nc.sync.dma_start(pool_bounce_in[:, :], zloc[:])
                nc.gpsimd.collective_compute(
                    "AllGather", AL.bypass,
                    replica_groups=[list(range(NCORES))],
                    ins=[pool_bounce_in.opt()], outs=[pool_bounce_out.opt()])
                zg = p5.tile([G, 128], F32)
                nc.sync.dma_start(zg[:], pool_bounce_out[:, :])
                zT_ps = p5ps.tile([128, G], F32, space="PSUM", tag="zT")
                nc.tensor.transpose(zT_ps[:], zg[:], ident[:])
                zT = p5.tile([128, G], F32)
                nc.vector.tensor_copy(zT[:], zT_ps[:])
                mlp_ps = p5ps.tile([G, 64], F32, space="PSUM", tag="mlp")
                nc.tensor.matmul(mlp_ps[:], lhsT=zT[:],
                                 rhs=smalls[:, SM_L1W:SM_L1W + 64],
                                 start=True, stop=True)
                z1 = p5.tile([G, 64], F32)
                nc.vector.tensor_tensor(out=z1[:], in0=mlp_ps[:], in1=l1bt,
                                        op=AL.add)
                nc.scalar.activation(z1[:], z1[:], ACT.Relu)
                z2 = p5.tile([G, 64], F32)
                nc.vector.tensor_tensor(out=z2[:], in0=z1[:], in1=l2rt,
                                        op=AL.mult)
                ored = p5.tile([G, 1], F32)
                nc.vector.tensor_reduce(ored[:], z2[:],
                                        axis=mybir.AxisListType.X, op=AL.add)
                nc.vector.tensor_tensor(out=ored[:], in0=ored[:], in1=b2c[:],
                                        op=AL.add)
                nc.sync.dma_start(out_final[:, :], ored[:])

    nc.compile()
    return nc


# ---------------------------------------------------------------- runner
# run_bass_kernel_spmd rebuilds jax.jit(shard_map(...)) on every call, which
# re-deserializes and re-loads the NEFF executable (~0.3s/call under axon).
# Build the same jit once and reuse it; the execution path is identical.
def _make_runner(nc):
    import jax
    from jax.sharding import Mesh, PartitionSpec
    from jax.experimental.shard_map import shard_map
    from concourse.bass2jax import (_bass_exec_p, partition_id_tensor,
                                    install_neuronx_cc_hook)

    install_neuronx_cc_hook()
    partition_name = (nc.partition_id_tensor.name
                      if nc.partition_id_tensor else None)
    in_names, out_names, out_avals, zero_shapes = [], [], [], []
    for alloc in nc.m.functions[0].allocations:
        if not isinstance(alloc, mybir.MemoryLocationSet):
            continue
        name = alloc.memorylocations[0].name
        if alloc.kind == "ExternalInput":
            if name != partition_name:
                in_names.append(name)
        elif alloc.kind == "ExternalOutput":
            out_names.append(name)
            shape = tuple(alloc.tensor_shape)
            dtype = mybir.dt.np(alloc.dtype)
            out_avals.append(jax.core.ShapedArray(shape, dtype))
            zero_shapes.append((shape, dtype))
    n_params = len(in_names)
    n_outs = len(out_avals)
    all_in_names = list(in_names) + list(out_names)
    if partition_name is not None:
        all_in_names.append(partition_name)

    def _body(*args):
        operands = list(args)
        if partition_name is not None:
            operands.append(partition_id_tensor())
        outs = _bass_exec_p.bind(
            *operands, out_avals=tuple(out_avals),
            in_names=tuple(all_in_names), out_names=tuple(out_names),
            lowering_input_output_aliases=(), sim_require_finite=True,
            sim_require_nnan=True, nc=nc)
        return tuple(outs)

    devices = jax.devices()[:NCORES]
    mesh = Mesh(np.asarray(devices), ("core",))
    sharded = jax.jit(
        shard_map(_body, mesh=mesh,
                  in_specs=(PartitionSpec("core"),) * (n_params + n_outs),
                  out_specs=(PartitionSpec("core"),) * n_outs,
                  check_rep=False),
        donate_argnums=tuple(range(n_params, n_params + n_outs)),
        keep_unused=True)

    def run(in_maps):
        concat_in = [
            np.concatenate([np.asarray(m[n]) for m in in_maps], axis=0)
            for n in in_names]
        concat_zeros = [np.zeros((NCORES * s[0], *s[1:]), d)
                        for s, d in zero_shapes]
        outs = sharded(*concat_in, *concat_zeros)
        return [
            {name: np.asarray(outs[i]).reshape(NCORES, *out_avals[i].shape)[c]
             for i, name in enumerate(out_names)}
            for c in range(NCORES)]

    return run


def _get_runner(nc):
    try:
        return _make_runner(nc)
    except Exception:
        return lambda in_maps: bass_utils.run_bass_kernel_spmd(
            nc, in_maps, core_ids=list(range(NCORES))).results


# ---------------------------------------------------------------- entry
def kernel(**inputs) -> np.ndarray:
    tpl, per_core = _host_prep(inputs)
    nc = _build(tpl)
    in_maps = [{"blob": per_core[c]["blob"]} for c in range(NCORES)]
    results = _get_runner(nc)(in_maps)
    out = np.asarray(results[0]["out_final"]).reshape(G)
    return out.astype(np.float32)


# revision 31
# speedup vs baseline: 7.4955x; 1.0375x over previous
"""GAT (2-layer, heads=1) + pooling + MLP on 8 Trainium2 NeuronCores.

Strategy:
- Nodes are mapped to per-graph padded slots (SLOT = align128(max graph size)),
  graphs are sharded 16-per-core, and within each core slots are striped so
  that dst-block i holds slots s with s % NB == i.  Every 128-slot block then
  contains exactly 8 slots of each of the core's 16 graphs (partition p ->
  local graph p//8), which makes pooling segment boundaries compile-time.
- Edge aggregation: per-edge rows [h|s] are fetched with dma_gather (int16
  indices -> 4 src buckets); attention weights w = exp(leakyrelu(s_src +
  d_dst)) are computed on-chip (d expanded per edge via a one-hot *
  broadcast-d reduce); the segment softmax + feature sum is one matmul per
  128-edge tile accumulating [sum(w*h) | sum(w)] into a per-block PSUM.
- Self-loop edges are applied densely at finalize (no gather).
- Node tables ([h|s] rows, bf16) are built sharded and AllGathered; pooled
  [G,2H] is assembled with a single small AllGather; the final MLP is
  replicated.
- All host inputs are packed into ONE bf16 blob per core (x in bf16 split
  into two 64-row halves, gather indices stored once instead of 8x, dst
  rows as int8, small tensors bit-packed f32) and unpacked on device --
  the axon tunnel transfer (~46MB/s + ~70ms/array) dominates wall-clock.
"""
import sys

sys.path.insert(0, "/opt/trn_rl_repo")

import numpy as np
import ml_dtypes

try:  # persistent XLA compile cache: repeat calls skip the NEFF re-wrap
    import jax

    jax.config.update("jax_compilation_cache_dir", "/tmp/jaxcache")
    jax.config.update("jax_persistent_cache_min_entry_size_bytes", -1)
    jax.config.update("jax_persistent_cache_min_compile_time_secs", 0)
except Exception:
    pass

import concourse.bacc as bacc
import concourse.bass as bass
import concourse.mybir as mybir
import concourse.tile as tile
from concourse import bass_utils
from concourse.masks import make_identity

bf16 = ml_dtypes.bfloat16
F32 = mybir.dt.float32
BF = mybir.dt.bfloat16
I16 = mybir.dt.int16
I8 = mybir.dt.int8
AL = mybir.AluOpType
ACT = mybir.ActivationFunctionType

NCORES = 8
G = 128
N = 100000
FIN = 64
H = 64
NEG = 0.2
NBUCKET = 4
GPC = G // NCORES  # graphs per core = 16
NEG_BIG = -1.0e30
X_INT8 = True  # ship x as int8 (scale folded into waug1 on host)


# ---------------------------------------------------------------- host prep
def _host_prep(inputs):
    x = np.asarray(inputs["x"], np.float32)
    ei = np.asarray(inputs["edge_index"]).astype(np.int64)
    bid = np.asarray(inputs["batch_ids"]).astype(np.int64)

    cnt = np.bincount(bid, minlength=G).astype(np.int64)
    SLOT = int(np.ceil(max(cnt.max(), 128) / 128) * 128)
    NSLOT = GPC * SLOT
    NB = NSLOT // 128
    assert NB % 16 == 0 and NB % 4 == 0, NB
    NGRP = NB // 16
    NSLOT_G = NCORES * NSLOT
    BUCKET = NSLOT_G // NBUCKET
    assert BUCKET * NBUCKET == NSLOT_G and BUCKET <= 32768

    gstart = np.zeros(G + 1, np.int64)
    gstart[1:] = np.cumsum(cnt)
    rank = np.arange(N, dtype=np.int64) - gstart[bid]
    slot_of = bid * SLOT + rank  # graph-padded slot, 0..NSLOT_G

    def pi(s):
        c, sl = s // NSLOT, s % NSLOT
        return c * NSLOT, (sl % NB) * 128 + sl // NB

    core_base, loc = pi(slot_of)
    pi_of = core_base + loc  # global pi row of each node

    # permuted x per core (pad rows zero); int8 with per-column scales,
    # folded into the rows of waug1 below (h = sum_c xq[:,c] * s_c * W1[c,:])
    if X_INT8:
        s_q = np.abs(x).max(axis=0) / 127.0  # [FIN]
        xq = np.clip(np.rint(x / s_q), -127, 127).astype(np.int8)
        x_pi = np.zeros((NCORES, NSLOT, FIN), np.int8)
        x_pi[pi_of // NSLOT, pi_of % NSLOT] = xq
    else:
        s_q = np.ones(FIN, np.float32)
        x_pi = np.zeros((NCORES, NSLOT, FIN), np.float32)
        x_pi[pi_of // NSLOT, pi_of % NSLOT] = x

    # masks / counts per core
    # local slot (block i, partition p) <-> graph-slot p*NB + i
    pp, ii = np.meshgrid(np.arange(128), np.arange(NB), indexing="ij")
    gslot = pp * NB + ii  # [128, NB] graph-padded local slot
    within = gslot % SLOT  # rank within graph
    lg = gslot // SLOT  # local graph 0..15 (== pp//8)
    mask01 = np.zeros((NCORES, 128, NB), np.float32)
    for c in range(NCORES):
        real = within < cnt[c * GPC + lg]
        mask01[c] = real.astype(np.float32)

    # edges (no self loops in the gather path)
    src, dst = ei[0], ei[1]
    ps = pi_of[src]
    pd = pi_of[dst]
    core = pd // NSLOT
    blk = (pd % NSLOT) // 128
    bkt = ps // BUCKET
    grp = blk // 16

    cnts = np.zeros((NCORES, NB, NBUCKET), np.int64)
    np.add.at(cnts, (core, blk, bkt), 1)
    # per-block tiles (uniform across cores only -- same SPMD program)
    tpb = np.zeros((NGRP, NBUCKET, 16), np.int64)  # tiles per block
    for g in range(NGRP):
        for b in range(NBUCKET):
            m = cnts[:, 16 * g:16 * g + 16, b].max(axis=0)  # [16]
            tpb[g, b] = np.maximum(1, np.ceil(m / 128).astype(np.int64))
    seg_tiles = tpb.sum(axis=2)  # tiles per (g,b) segment
    blk_off = np.zeros((NGRP, NBUCKET, 16), np.int64)  # tile offset in seg
    blk_off[:, :, 1:] = np.cumsum(tpb, axis=2)[:, :, :-1]
    TT = int(seg_tiles.sum())  # total tiles per core per layer
    TOTSLOT = TT * 128

    # slot offsets: order (g, b, block-within-group, slot)
    seg_off = np.zeros((NGRP, NBUCKET), np.int64)
    acc = 0
    for g in range(NGRP):
        for b in range(NBUCKET):
            seg_off[g, b] = acc
            acc += seg_tiles[g, b] * 128

    order = np.lexsort((bkt, blk))  # edges sorted by (blk, bkt); core split below
    src_local = np.zeros((NCORES, 128, TT), np.int16)
    dst_i8 = np.full((NCORES, 128, TT), -1, np.int8)
    for c in range(NCORES):
        sel = order[core[order] == c]
        sblk, sbkt = blk[sel], bkt[sel]
        sps, spd = ps[sel], pd[sel]
        # slot index for each edge: within its (g,b,block) run
        # run start: seg_off[g,b] + (blk%16)*tpb[g,b]*128; position = rank in run
        key = sblk * NBUCKET + sbkt
        # stable order already (blk, bkt); rank within run:
        runstart_mark = np.r_[True, key[1:] != key[:-1]]
        runid = np.cumsum(runstart_mark) - 1
        nruns = int(runid[-1]) + 1 if len(runid) else 0
        first = np.full(nruns, len(sel), np.int64)
        np.minimum.at(first, runid, np.arange(len(sel)))
        pos = np.arange(len(sel)) - first[runid]
        gg = sblk // 16
        slot = (seg_off[gg, sbkt] + blk_off[gg, sbkt, sblk % 16] * 128 + pos)
        assert (pos < tpb[gg, sbkt, sblk % 16] * 128).all()
        p_ = slot % 128
        t_ = slot // 128
        src_local[c, p_, t_] = (sps - sbkt * BUCKET).astype(np.int16)
        dst_i8[c, p_, t_] = (spd % NSLOT % 128).astype(np.int8)

    # wrapped int16 gather indices, stored compactly: per (g,b) segment the
    # [16, ntile*8] wrap is folded into [128, ntile] (row 16a+r holds
    # wseg[r, a*ntile : (a+1)*ntile]); the 8x replication the gather engine
    # wants is re-created on device with 8 DMAs per segment.
    idx_pack = np.zeros((NCORES, 128, TT), np.int16)
    for c in range(NCORES):
        flat = np.zeros(TOTSLOT, np.int16)
        sl = src_local[c]
        flat[np.arange(TOTSLOT)] = sl[np.arange(TOTSLOT) % 128,
                                      np.arange(TOTSLOT) // 128]
        w = flat.reshape(TOTSLOT // 16, 16).T  # [16, TT*8]
        for g in range(NGRP):
            for b in range(NBUCKET):
                off_t = int(seg_off[g, b]) // 128
                ntile = int(seg_tiles[g, b])
                wseg = w[:, off_t * 8:(off_t + ntile) * 8]
                idx_pack[c][:, off_t:off_t + ntile] = (
                    wseg.reshape(16, 8, ntile).transpose(1, 0, 2)
                    .reshape(128, ntile))

    # weights
    W1 = np.asarray(inputs["W1"], np.float32)
    W2 = np.asarray(inputs["W2"], np.float32)
    waug1 = s_q[:, None] * np.concatenate(
        [W1, (W1 @ np.asarray(inputs["a_src1"], np.float32))[:, None],
         (W1 @ np.asarray(inputs["a_dst1"], np.float32))[:, None]], axis=1)
    waug2 = np.concatenate(
        [W2, (W2 @ np.asarray(inputs["a_src2"], np.float32))[:, None],
         (W2 @ np.asarray(inputs["a_dst2"], np.float32))[:, None]], axis=1)

    b1 = np.asarray(inputs["b1"], np.float32)
    b2v = np.asarray(inputs["b2"], np.float32)
    lin1_W = np.asarray(inputs["lin1_W"], np.float32)
    lin1_b = np.asarray(inputs["lin1_b"], np.float32)
    lin2_W = np.asarray(inputs["lin2_W"], np.float32)
    lin2_b = np.asarray(inputs["lin2_b"], np.float32)

    invcnt = np.zeros((NCORES, 64, GPC), np.float32)
    for c in range(NCORES):
        invcnt[c] = np.broadcast_to(
            1.0 / np.maximum(cnt[c * GPC:(c + 1) * GPC], 1.0), (64, GPC))

    # ---- blob layout (bf16 columns); x is stored as two 64-row halves
    XH = NSLOT // 2  # x columns per half
    XCOLS = (NSLOT // 4) if X_INT8 else (NSLOT // 2)  # bf16 cols of X region
    IOFF = XCOLS
    DOFF = XCOLS + TT
    DCOLS = (TT + 1) // 2  # int8 dst rows, padded so SOFF lands f32-aligned
    if (TT + DCOLS) % 2:
        DCOLS += 1
    MOFF = DOFF + DCOLS  # int8 mask01 [128, NB]
    SOFF = MOFF + NB // 2
    S = 277  # f32 cols in the small region
    CB = SOFF + 2 * S

    per_core = []
    for c in range(NCORES):
        blob = np.zeros((128, CB * 2), np.uint8)
        if X_INT8:
            xT_q = np.ascontiguousarray(x_pi[c].T)  # [64, NSLOT] int8
            blob[0:64, 0:XH] = np.ascontiguousarray(
                xT_q[:, 0:XH]).view(np.uint8)
            blob[64:128, 0:XH] = np.ascontiguousarray(
                xT_q[:, XH:]).view(np.uint8)
        else:
            xT_bf = np.ascontiguousarray(x_pi[c].T).astype(bf16)
            blob[0:64, 0:XH * 2] = np.ascontiguousarray(
                xT_bf[:, 0:XH]).view(np.uint8)
            blob[64:128, 0:XH * 2] = np.ascontiguousarray(
                xT_bf[:, XH:]).view(np.uint8)
        blob[:, IOFF * 2:(IOFF + TT) * 2] = idx_pack[c].view(np.uint8)
        blob[:, DOFF * 2:DOFF * 2 + TT] = dst_i8[c].view(np.uint8)
        blob[:, MOFF * 2:MOFF * 2 + NB] = mask01[c].astype(np.int8)
        sm = np.zeros((128, S), np.float32)
        sm[:, 0:64] = lin1_W
        sm[0:64, 64:130] = waug1
        sm[0:64, 130:196] = waug2
        sm[0:64, 196:196 + GPC] = invcnt[c]
        SM_B = 196 + GPC
        sm[0, SM_B:SM_B + 64] = b1
        sm[1, SM_B:SM_B + 64] = b2v
        sm[2, SM_B:SM_B + 64] = lin1_b
        sm[3, SM_B:SM_B + 64] = lin2_W[:, 0]
        sm[0, SM_B + 64] = lin2_b[0]
        blob[:, SOFF * 2:SOFF * 2 + S * 4] = sm.view(np.uint8)
        per_core.append({"blob": blob.view(bf16)})

    tpl = dict(SLOT=SLOT, NSLOT=NSLOT, NB=NB, NGRP=NGRP, NSLOT_G=NSLOT_G,
               BUCKET=BUCKET, tpb=tpb, blk_off=blk_off, seg_tiles=seg_tiles,
               seg_off=seg_off, TT=TT, XH=XH, IOFF=IOFF, DOFF=DOFF,
               MOFF=MOFF, SOFF=SOFF, S=S, CB=CB)
    return tpl, per_core


# ---------------------------------------------------------------- device bld
def _build(tpl):
    NSLOT = tpl["NSLOT"]
    NB = tpl["NB"]
    NGRP = tpl["NGRP"]
    NSLOT_G = tpl["NSLOT_G"]
    BUCKET = tpl["BUCKET"]
    tpb = tpl["tpb"]
    blk_off = tpl["blk_off"]
    seg_tiles = tpl["seg_tiles"]
    TT = tpl["TT"]
    XH = tpl["XH"]
    IOFF = tpl["IOFF"]
    DOFF = tpl["DOFF"]
    MOFF = tpl["MOFF"]
    SOFF = tpl["SOFF"]
    S = tpl["S"]
    CB = tpl["CB"]
    SM_L1W = 0
    SM_WG1 = 64
    SM_WG2 = 130
    SM_ICN = 196
    SM_B = 196 + GPC
    SM_B2 = SM_B + 64

    nc = bacc.Bacc("TRN2", target_bir_lowering=False, debug=False,
                   num_devices=NCORES)

    blob = nc.dram_tensor("blob", [128, CB], BF, kind="ExternalInput")
    out_final = nc.dram_tensor("out_final", [128, 1], F32,
                               kind="ExternalOutput")

    with tile.TileContext(nc) as tc:
        with (
            tc.tile_pool(name="const", bufs=1) as cp,
            tc.tile_pool(name="stage", bufs=1) as stp,
            tc.tile_pool(name="dram", bufs=1, space="DRAM") as dr,
        ):
            # ---- constants in SBUF
            iota_i = cp.tile([128, 128], mybir.dt.int32)
            nc.gpsimd.iota(iota_i[:], pattern=[[1, 128]], base=0,
                           channel_multiplier=0)
            iota_bf = cp.tile([128, 128], BF)
            nc.vector.tensor_copy(iota_bf[:], iota_i[:])
            ident = cp.tile([128, 128], F32)
            make_identity(nc, ident[:])
            ones_bf = cp.tile([128, 1], BF)
            nc.gpsimd.memset(ones_bf[:], 1.0)
            onec = cp.tile([128, 1], F32)
            nc.gpsimd.memset(onec[:], 1.0)

            # ---- unpack the blob
            smalls = cp.tile([128, S], F32)
            nc.sync.dma_start(
                smalls[:], blob[:, SOFF:SOFF + 2 * S].bitcast(F32))
            m8 = stp.tile([128, NB], I8, name="m8")
            nc.sync.dma_start(m8[:], blob[:, MOFF:MOFF + NB // 2].bitcast(I8))
            m01 = cp.tile([128, NB], F32)
            nc.vector.tensor_copy(m01[:], m8[:])
            mpad = cp.tile([128, NB], F32)
            nc.vector.scalar_tensor_tensor(
                out=mpad[:], in0=m01[:], scalar=-1.0,
                in1=onec[:].to_broadcast([128, NB]),
                op0=AL.mult, op1=AL.add)
            waug1_sb = cp.tile([64, 66], BF)
            nc.vector.tensor_copy(waug1_sb[:], smalls[0:64, SM_WG1:SM_WG1 + 66])
            waug2_sb = cp.tile([64, 66], BF)
            nc.vector.tensor_copy(waug2_sb[:], smalls[0:64, SM_WG2:SM_WG2 + 66])
            btiles = cp.tile([128, 4, 64], F32)
            nc.sync.dma_start(
                btiles[:],
                blob[0:4, SOFF + 2 * SM_B:SOFF + 2 * SM_B + 128].bitcast(
                    F32).rearrange("(o r) c -> o r c", o=1).to_broadcast(
                    [128, 4, 64]))
            b1t = btiles[:, 0, :]
            b2t = btiles[:, 1, :]
            l1bt = btiles[:, 2, :]
            l2rt = btiles[:, 3, :]
            b2c = cp.tile([128, 1], F32)
            nc.sync.dma_start(
                b2c[:],
                blob[0:1, SOFF + 2 * SM_B2:SOFF + 2 * SM_B2 + 2].bitcast(
                    F32).to_broadcast([128, 1]))
            dst_i8 = stp.tile([128, TT], I8, name="dsti8")
            nc.sync.dma_start(
                dst_i8[:],
                blob[:, DOFF:MOFF].bitcast(I8)[:, 0:TT])
            dl_all = cp.tile([128, TT], F32)
            nc.vector.tensor_copy(dl_all[:], dst_i8[:])

            # persistent staging
            h_T = stp.tile([64, NSLOT], BF, tag="h_T", name="h_T")  # layer1 out, pi order
            dstage = [stp.tile([128, NB], F32, tag=f"dstage{l}", name=f"dstage{l}")
                      for l in range(2)]
            pool_mx = [stp.tile([64, GPC], F32, tag=f"pmx{l}", name=f"pmx{l}")
                       for l in range(2)]
            pool_sm = [stp.tile([64, GPC], F32, tag=f"psm{l}", name=f"psm{l}")
                       for l in range(2)]
            for l in range(2):
                nc.vector.memset(pool_mx[l][:], NEG_BIG)
                nc.vector.memset(pool_sm[l][:], 0.0)

            # DRAM scratch
            table = [dr.tile([NSLOT_G, 128], BF, tag=f"tab{l}", name=f"tab{l}")
                     for l in range(2)]
            tab_own = [dr.tile([NSLOT, 128], BF, tag=f"tabown{l}", name=f"tabown{l}")
                       for l in range(2)]
            d2d = [dr.tile([NB, 128], F32, tag=f"d2d{l}", name=f"d2d{l}") for l in range(2)]
            pool_bounce_in = dr.tile([GPC, 128], F32)
            pool_bounce_out = dr.tile([G, 128], F32)

            # ===== table 1 build: own shard of [h|s] rows + d column =====
            with (
                tc.tile_pool(name="p1", bufs=3) as p1,
                tc.tile_pool(name="p1x", bufs=3) as p1x,
                tc.tile_pool(name="p1ps", bufs=2, space="PSUM") as p1ps,
            ):
                QB = NB // 4
                for s in range(4):
                    half = (s * QB) // (NB // 2)
                    within = (s * QB) % (NB // 2)
                    rows0 = 64 * half
                    xT = p1x.tile([64, QB * 128], BF, tag="xTs")
                    if X_INT8:
                        coff = within * 64
                        xq8 = p1x.tile([64, QB * 128], I8, tag="xq8")
                        nc.sync.dma_start(
                            xq8[:],
                            blob[rows0:rows0 + 64,
                                 coff:coff + QB * 64].bitcast(I8))
                        nc.vector.tensor_copy(xT[:], xq8[:])
                    else:
                        coff = within * 128
                        nc.sync.dma_start(
                            xT[:],
                            blob[rows0:rows0 + 64, coff:coff + QB * 128])
                    rows = p1.tile([128, QB, 65], BF, tag="rows")
                    for j in range(QB):
                        blk_id = s * QB + j
                        hps = p1ps.tile([128, 66], F32, space="PSUM",
                                        tag="hps")
                        nc.tensor.matmul(
                            hps[:], lhsT=xT[:, 128 * j:128 * (j + 1)],
                            rhs=waug1_sb[:], start=True, stop=True)
                        nc.scalar.activation(rows[:, j, :], hps[:, 0:65],
                                             ACT.Copy)
                        nc.vector.tensor_copy(
                            dstage[0][:, blk_id:blk_id + 1],
                            hps[:, 65:66])
                    nc.sync.dma_start(
                        tab_own[0].rearrange("(s r) c -> r s c", r=128)[
                            :, s * QB:(s + 1) * QB, 0:65],
                        rows[:, 0:QB, :])
                # d transpose -> DRAM [NB, 128]
                dt_ps = p1ps.tile([NB, 128], F32, space="PSUM", tag="dt")
                nc.tensor.transpose(dt_ps[:], dstage[0][:], ident[:])
                dt_sb = p1.tile([NB, 128], F32, tag="dts")
                nc.vector.tensor_copy(dt_sb[:], dt_ps[:])
                nc.sync.dma_start(d2d[0][:, :], dt_sb[:])

            nc.gpsimd.collective_compute(
                "AllGather", AL.bypass,
                replica_groups=[list(range(NCORES))],
                ins=[tab_own[0].opt()], outs=[table[0].opt()])

            # ================= edge pass (both layers) =================
            def edge_pass(l):
                tab = table[l]
                with (
                    tc.tile_pool(name=f"eg{l}", bufs=2) as eg,
                    tc.tile_pool(name=f"eq{l}", bufs=2) as eqp,
                    tc.tile_pool(name=f"ep{l}", bufs=1, space="PSUM") as eps,
                    tc.tile_pool(name=f"et{l}", bufs=2, space="PSUM") as ept,
                    tc.tile_pool(name=f"ef{l}", bufs=3) as ef,
                ):
                    for g in range(NGRP):
                        # broadcast d rows of the group's 16 blocks
                        dbc = ef.tile([128, 16, 128], F32, tag="dbc")
                        nc.sync.dma_start(
                            dbc[:],
                            d2d[l][16 * g:16 * (g + 1), :].rearrange(
                                "(o r) c -> o r c", o=1).to_broadcast(
                                [128, 16, 128]))
                        dbc_bf = ef.tile([128, 16, 128], BF, tag="dbcb")
                        nc.vector.tensor_copy(dbc_bf[:], dbc[:])

                        psums = [eps.tile([128, 260], F32, space="PSUM",
                                          tag=f"ps{k}", name=f"ps{k}")
                                 for k in range(4)]
                        for ps_ in psums:
                            nc.vector.memset(ps_[:], 0.0)

                        for b in range(NBUCKET):
                            ntile = int(seg_tiles[g, b])
                            tpb_i = [int(t) for t in tpb[g, b]]
                            boff_i = [int(t) for t in blk_off[g, b]]
                            blk_of_tile = [i for i in range(16)
                                           for _ in range(tpb_i[i])]
                            off = int(tpl["seg_off"][g, b]) // 128  # tile off
                            # gather the whole (g,b) segment; indices come
                            # from the compact blob region, replicated 8x
                            idx_sb = eg.tile([128, ntile * 8], I16, tag="idx")
                            for k in range(8):
                                nc.sync.dma_start(
                                    idx_sb[16 * k:16 * (k + 1), :].rearrange(
                                        "p (a c) -> p a c", a=8),
                                    blob[:, IOFF + off:IOFF + off + ntile]
                                    .bitcast(I16).rearrange(
                                        "(a p) c -> p a c", p=16))
                            ch = eg.tile([128, ntile, 128], BF, tag="ch")
                            nc.gpsimd.dma_gather(
                                out_ap=ch[:],
                                in_ap=tab[b * BUCKET:(b + 1) * BUCKET, :],
                                idxs_ap=idx_sb[:],
                                num_idxs=ntile * 128,
                                num_idxs_reg=ntile * 128,
                                elem_size=128,
                                single_packet=False)
                            dl = dl_all[:, off:off + ntile]
                            # per tile: lhsT = (iota==dstloc)*|d|  (the |d|
                            # row-scale cancels in the softmax division);
                            # accum of (iota==dstloc)*d gives d_edge.
                            lhsT = eqp.tile([128, ntile, 128], BF, tag="eq")
                            dedge = ef.tile([128, ntile], F32, tag="dedge")
                            for tt in range(ntile):
                                k16 = blk_of_tile[tt]
                                nc.vector.scalar_tensor_tensor(
                                    out=lhsT[:, tt, :], in0=iota_bf[:],
                                    scalar=dl[:, tt:tt + 1],
                                    in1=dbc_bf[:, k16, :],
                                    op0=AL.is_equal, op1=AL.mult,
                                    accum_out=dedge[:, tt:tt + 1])
                            # z = s + d ; w = exp(max(0.2 z, z))
                            z = ef.tile([128, ntile], F32, tag="z")
                            nc.vector.tensor_tensor(
                                out=z[:], in0=ch[:, :, 64], in1=dedge[:],
                                op=AL.add)
                            nc.vector.scalar_tensor_tensor(
                                out=z[:], in0=z[:], scalar=NEG, in1=z[:],
                                op0=AL.mult, op1=AL.max)
                            w = ef.tile([128, ntile], F32, tag="w")
                            nc.scalar.activation(w[:], z[:], ACT.Exp)
                            # ones into s slot for the denominator column
                            nc.vector.tensor_copy(
                                ch[:, :, 64],
                                ones_bf[:].to_broadcast([128, ntile]))
                            # rhs2 = ch[:, :, 0:65] * w (bulk)
                            ch2 = eqp.tile([128, ntile, 65], BF, tag="dmul",
                                           name="ch2")
                            nc.vector.tensor_tensor(
                                out=ch2[:], in0=ch[:, :, 0:65],
                                in1=w[:].to_broadcast([128, ntile, 65]),
                                op=AL.mult)
                            # matmuls
                            for i in range(16):
                                ps = psums[i // 4]
                                csl = slice(65 * (i % 4), 65 * (i % 4) + 65)
                                for t in range(tpb_i[i]):
                                    tt = boff_i[i] + t
                                    nc.tensor.matmul(
                                        ps[:, csl],
                                        lhsT=lhsT[:, tt, :],
                                        rhs=ch2[:, tt, :],
                                        start=False,
                                        stop=(b == NBUCKET - 1
                                              and t == tpb_i[i] - 1))

                        # ---- finalize the group's 16 blocks
                        for i in range(16):
                            blk_id = 16 * g + i
                            ps = psums[i // 4]
                            csl = slice(65 * (i % 4), 65 * (i % 4) + 65)
                            # self loops (dense): rows of own dst block
                            row = ef.tile([128, 65], BF, tag="slrow")
                            nc.sync.dma_start(
                                row[:], tab_own[l][128 * blk_id:
                                                   128 * blk_id + 128, 0:65])
                            zs = ef.tile([128, 1], F32, tag="zs")
                            nc.vector.tensor_tensor(
                                out=zs[:], in0=row[:, 64:65],
                                in1=dstage[l][:, blk_id:blk_id + 1],
                                op=AL.add)
                            nc.vector.scalar_tensor_tensor(
                                out=zs[:], in0=zs[:], scalar=NEG, in1=zs[:],
                                op0=AL.mult, op1=AL.max)
                            ws = ef.tile([128, 1], F32, tag="ws")
                            nc.scalar.activation(ws[:], zs[:], ACT.Exp)
                            nc.vector.tensor_tensor(
                                out=ws[:], in0=ws[:],
                                in1=m01[:, blk_id:blk_id + 1], op=AL.mult)
                            nc.vector.tensor_tensor(
                                out=ws[:], in0=ws[:],
                                in1=dstage[l][:, blk_id:blk_id + 1],
                                op=AL.mult)
                            nc.vector.tensor_copy(
                                row[:, 64:65], ones_bf[:])
                            nc.vector.scalar_tensor_tensor(
                                out=ps[:, csl], in0=row[:, :],
                                scalar=ws[:], in1=ps[:, csl],
                                op0=AL.mult, op1=AL.add)
                            # divide + bias + relu
                            den = ef.tile([128, 1], F32, tag="den")
                            nc.vector.tensor_tensor(
                                out=den[:],
                                in0=ps[:, csl.start + 64:csl.start + 65],
                                in1=mpad[:, blk_id:blk_id + 1], op=AL.add)
                            rec = ef.tile([128, 1], F32, tag="rec")
                            nc.vector.reciprocal(rec[:], den[:])
                            hmid = ef.tile([128, 64], F32, tag="hmid")
                            nc.vector.scalar_tensor_tensor(
                                out=hmid[:],
                                in0=ps[:, csl.start:csl.start + 64],
                                scalar=rec[:], in1=b1t if l == 0 else b2t,
                                op0=AL.mult, op1=AL.add)
                            hout = ef.tile([128, 64], F32, tag="hout")
                            nc.scalar.activation(hout[:], hmid[:], ACT.Relu)
                            # mask pads to exactly 0 (safe for max: h1 >= 0)
                            nc.vector.tensor_scalar_mul(
                                hout[:], hout[:], m01[:, blk_id:blk_id + 1])
                            # transpose (PE) -> [64, 128]
                            hT = ept.tile([64, 128], F32, space="PSUM",
                                          tag="hT")
                            nc.tensor.transpose(hT[:], hout[:], ident[:])
                            if l == 0:
                                nc.vector.tensor_copy(
                                    h_T[:, 128 * blk_id:128 * (blk_id + 1)],
                                    hT[:])
                            # sum pool
                            red = ef.tile([64, GPC], F32, tag="red")
                            nc.vector.tensor_reduce(
                                red[:],
                                hT[:].rearrange("f (g e) -> f g e", g=GPC),
                                axis=mybir.AxisListType.X, op=AL.add)
                            nc.vector.tensor_tensor(
                                out=pool_sm[l][:], in0=pool_sm[l][:],
                                in1=red[:], op=AL.add)
                            # max pool
                            redm = ef.tile([64, GPC], F32, tag="redm")
                            nc.vector.tensor_reduce(
                                redm[:],
                                hT[:].rearrange("f (g e) -> f g e", g=GPC),
                                axis=mybir.AxisListType.X, op=AL.max)
                            nc.vector.tensor_tensor(
                                out=pool_mx[l][:], in0=pool_mx[l][:],
                                in1=redm[:], op=AL.max)

            edge_pass(0)

            # ================= table 2 build =================
            with (
                tc.tile_pool(name="p3", bufs=3) as p3,
                tc.tile_pool(name="p3ps", bufs=2, space="PSUM") as p3ps,
            ):
                for i in range(NB):
                    hps = p3ps.tile([128, 66], F32, space="PSUM", tag="hps2")
                    nc.tensor.matmul(
                        hps[:], lhsT=h_T[:, 128 * i:128 * (i + 1)],
                        rhs=waug2_sb[:], start=True, stop=True)
                    row = p3.tile([128, 65], BF, tag="row2")
                    nc.vector.tensor_copy(row[:], hps[:, 0:65])
                    nc.sync.dma_start(
                        tab_own[1][128 * i:128 * (i + 1), 0:65], row[:])
                    nc.vector.tensor_copy(dstage[1][:, i:i + 1], hps[:, 65:66])
                dt_ps = p3ps.tile([NB, 128], F32, space="PSUM", tag="dt2")
                nc.tensor.transpose(dt_ps[:], dstage[1][:], ident[:])
                dt_sb = p3.tile([NB, 128], F32, tag="dts2")
                nc.vector.tensor_copy(dt_sb[:], dt_ps[:])
                nc.sync.dma_start(d2d[1][:, :], dt_sb[:])

            nc.gpsimd.collective_compute(
                "AllGather", AL.bypass,
                replica_groups=[list(range(NCORES))],
                ins=[tab_own[1].opt()], outs=[table[1].opt()])

            edge_pass(1)

            # ================= pooling combine + MLP =================
            with (
                tc.tile_pool(name="p5", bufs=2) as p5,
                tc.tile_pool(name="p5ps", bufs=1, space="PSUM") as p5ps,
            ):
                icn = smalls[0:64, SM_ICN:SM_ICN + GPC]
                mxh = p5.tile([64, GPC], F32)
                nc.vector.tensor_tensor(out=mxh[:], in0=pool_mx[0][:],
                                        in1=pool_mx[1][:], op=AL.add)
                smh = p5.tile([64, GPC], F32)
                nc.vector.tensor_tensor(out=smh[:], in0=pool_sm[0][:],
                                        in1=pool_sm[1][:], op=AL.add)
                nc.vector.tensor_tensor(out=smh[:], in0=smh[:], in1=icn,
                                        op=AL.mult)
                # transpose to graph-major [GPC, 128] and AllGather
                zloc = p5.tile([GPC, 128], F32)
                mxT = p5ps.tile([GPC, 64], F32, space="PSUM", tag="mxT")
                nc.tensor.transpose(mxT[:], mxh[:], ident[0:64, 0:64])
                nc.vector.tensor_copy(zloc[:, 0:64], mxT[:])
                smT = p5ps.tile([GPC, 64], F32, space="PSUM", tag="smT")
                nc.tensor.transpose(smT[:], smh[:], ident[0:64, 0:64])
                nc.vector.tensor_copy(zloc[:, 64:128], smT[:])
                nc.sync.dma_start(pool_bounce_in[:, :], zloc[:])
                nc.gpsimd.collective_compute(
                    "AllGather", AL.bypass,
                    replica_groups=[list(range(NCORES))],
                    ins=[pool_bounce_in.opt()], outs=[pool_bounce_out.opt()])
                zg = p5.tile([G, 128], F32)
                nc.sync.dma_start(zg[:], pool_bounce_out[:, :])
                zT_ps = p5ps.tile([128, G], F32, space="PSUM", tag="zT")
                nc.tensor.transpose(zT_ps[:], zg[:], ident[:])
                zT = p5.tile([128, G], F32)
                nc.vector.tensor_copy(zT[:], zT_ps[:])
                mlp_ps = p5ps.tile([G, 64], F32, space="PSUM", tag="mlp")
                nc.tensor.matmul(mlp_ps[:], lhsT=zT[:],
                                 rhs=smalls[:, SM_L1W:SM_L1W + 64],
                                 start=True, stop=True)
                z1 = p5.tile([G, 64], F32)
                nc.vector.tensor_tensor(out=z1[:], in0=mlp_ps[:], in1=l1bt,
                                        op=AL.add)
                nc.scalar.activation(z1[:], z1[:], ACT.Relu)
                z2 = p5.tile([G, 64], F32)
                nc.vector.tensor_tensor(out=z2[:], in0=z1[:], in1=l2rt,
                                        op=AL.mult)
                ored = p5.tile([G, 1], F32)
                nc.vector.tensor_reduce(ored[:], z2[:],
                                        axis=mybir.AxisListType.X, op=AL.add)
                nc.vector.tensor_tensor(out=ored[:], in0=ored[:], in1=b2c[:],
                                        op=AL.add)
                nc.sync.dma_start(out_final[:, :], ored[:])

    nc.compile()
    return nc


# ---------------------------------------------------------------- runner
# run_bass_kernel_spmd rebuilds jax.jit(shard_map(...)) on every call, which
# re-deserializes and re-loads the NEFF executable (~0.3s/call under axon).
# Build the same jit once and reuse it; the execution path is identical.
def _make_runner(nc):
    import jax
    from jax.sharding import Mesh, PartitionSpec
    from jax.experimental.shard_map import shard_map
    from concourse.bass2jax import (_bass_exec_p, partition_id_tensor,
                                    install_neuronx_cc_hook)

    install_neuronx_cc_hook()
    partition_name = (nc.partition_id_tensor.name
                      if nc.partition_id_tensor else None)
    in_names, out_names, out_avals, zero_shapes = [], [], [], []
    for alloc in nc.m.functions[0].allocations:
        if not isinstance(alloc, mybir.MemoryLocationSet):
            continue
        name = alloc.memorylocations[0].name
        if alloc.kind == "ExternalInput":
            if name != partition_name:
                in_names.append(name)
        elif alloc.kind == "ExternalOutput":
            out_names.append(name)
            shape = tuple(alloc.tensor_shape)
            dtype = mybir.dt.np(alloc.dtype)
            out_avals.append(jax.core.ShapedArray(shape, dtype))
            zero_shapes.append((shape, dtype))
    n_params = len(in_names)
    n_outs = len(out_avals)
    all_in_names = list(in_names) + list(out_names)
    if partition_name is not None:
        all_in_names.append(partition_name)

    def _body(*args):
        operands = list(args)
        if partition_name is not None:
            operands.append(partition_id_tensor())
        outs = _bass_exec_p.bind(
            *operands, out_avals=tuple(out_avals),
            in_names=tuple(all_in_names), out_names=tuple(out_names),
            lowering_input_output_aliases=(), sim_require_finite=True,
            sim_require_nnan=True, nc=nc)
        return tuple(outs)

    devices = jax.devices()[:NCORES]
    mesh = Mesh(np.asarray(devices), ("core",))
    sharded = jax.jit(
        shard_map(_body, mesh=mesh,
                  in_specs=(PartitionSpec("core"),) * (n_params + n_outs),
                  out_specs=(PartitionSpec("core"),) * n_outs,
                  check_rep=False),
        donate_argnums=tuple(range(n_params, n_params + n_outs)),
        keep_unused=True)

    def run(in_maps):
        concat_in = [
            np.concatenate([np.asarray(m[n]) for m in in_maps], axis=0)
            for n in in_names]
        concat_zeros = [np.zeros((NCORES * s[0], *s[1:]), d)
                        for s, d in zero_shapes]
        outs = sharded(*concat_in, *concat_zeros)
        return [
            {name: np.asarray(outs[i]).reshape(NCORES, *out_avals[i].shape)[c]
             for i, name in enumerate(out_names)}
            for c in range(NCORES)]

    return run


def _get_runner(nc):
    try:
        return _make_runner(nc)
    except Exception:
        return lambda in_maps: bass_utils.run_bass_kernel_spmd(
            nc, in_maps, core_ids=list(range(NCORES))).results


# ---------------------------------------------------------------- entry
def kernel(**inputs) -> np.ndarray:
    tpl, per_core = _host_prep(inputs)
    nc = _build(tpl)
    in_maps = [{"blob": per_core[c]["blob"]} for c in range(NCORES)]
    results = _get_runner(nc)(in_maps)
    out = np.asarray(results[0]["out_final"]).reshape(G)
    return out.astype(np.float32)
